# revision 1
# baseline (speedup 1.0000x reference)
"""Trainium2 Bass kernel for batched single-head attention with seq-sum pooling.

Reference computation (B=16, S=2048, D=512, fp32):
    q = x @ W_q ; k = x @ W_k ; v = x @ W_v          per batch  [S, D]
    scores = q @ k.T / sqrt(D)                        [S, S]
    attn = softmax(scores, axis=-1)
    out_b = sum_s (attn @ v)[s, :]                    [D]

Two algebraic restructures carry most of the speedup:
1. The final sum over query positions commutes through both trailing
   matmuls: out_b = (sum_q attn[q,:]) @ (x @ W_v) = ((r^T E) @ x) @ W_v,
   where E = exp(scores/sqrt(D)) and r[q] = 1/rowsum_q(E) — removes the
   [S,S]x[S,D] attention-value matmul AND the V projection.
2. scores = (x W_q)(x W_k)^T = x M x^T with M = W_q W_k^T computed ONCE per
   core (~6us) — replaces both per-batch Q/K projections with a single
   G = x M projection, and x^T itself becomes the scores key operand.
Net: of the reference's four [*,D]x[D,*] weight matmuls per batch, only one
(G = x M) survives, plus the scores matmul, one column-sum pass over E, a
[1,S]x[S,D] matvec against x, and a [1,D]x[D,D] epilogue.

Sharding: pure data parallelism over batch — 2 batch elements per core on 8
NeuronCores, weights replicated, no collectives.  Host concatenates per-core
[2, D] outputs.

Matmul operands are bf16 (fp32 PSUM accumulation), which streams the PE at
~215 ns per [128x128]x[128x512] matmul.  The X transpose runs on the PE as a
REGULAR identity matmul (out = x_tile.T @ I): transpose-mode matmuls don't
pipeline weight loads and don't count as PE activity for the HAM clock gate
(the array would re-throttle 2.4 -> 1.2 GHz), and the DMA-crossbar transpose
serializes the whole DMA subsystem against ordinary copies.  Row vectors
(w, y) are transposed to columns via K=1 outer-product matmuls against a
[1,1] ones tile, then broadcast across a 128-wide stationary tile so the
accumulation matmuls run at the full-width issue rate.

Emission is software-pipelined around a dense PE instruction stream: x-chunk
cast-DMAs (SWDGE f32->bf16) are ordered so each lands just before use,
transposes run one s-chunk ahead woven between projection groups, batch 0's
w-phase weaves into batch 1's projections, and PSUM banks are split
scores(2) + transposes/epilogue(2) + colsum accumulator(4).

Measured: HW exec ~240 us on 8 cores (unthrottled), rel error ~3.4e-3
(tolerance 2e-2).
"""

import sys

sys.path.insert(0, "/opt/trn_rl_repo")

import numpy as np

import concourse.bass as bass
import concourse.mybir as mybir
import concourse.tile as tile
from concourse import bacc
from concourse.bass_utils import run_bass_kernel_spmd
from concourse.masks import make_identity

B, S, D = 16, 2048, 512
P = 128
N_CORES = 8
B_PER_CORE = B // N_CORES  # 2
SCALE = 1.0 / float(np.sqrt(D))

F32 = mybir.dt.float32
BF16 = mybir.dt.bfloat16

N_ST = S // P  # 16 s-tiles (partition tiles of the sequence dim)
N_DT = D // P  # 4 d-tiles (partition tiles of the feature dim)
NCH = 512  # moving free dim per matmul (one fp32 PSUM bank)
N_SC = S // NCH  # 4 s-chunks of the sequence dim
N_KC = S // NCH  # 4 k-chunks of the key dim


def build_nc():
    nc = bacc.Bacc("TRN2", target_bir_lowering=False, debug=False, num_devices=N_CORES)
    x_ext = nc.dram_tensor(
        "inputs", [B_PER_CORE, S, D], F32, kind="ExternalInput"
    ).ap()
    wq_ext = nc.dram_tensor("W_q", [D, D], F32, kind="ExternalInput").ap()
    wk_ext = nc.dram_tensor("W_k", [D, D], F32, kind="ExternalInput").ap()
    wv_ext = nc.dram_tensor("W_v", [D, D], F32, kind="ExternalInput").ap()
    out_ext = nc.dram_tensor("out", [B_PER_CORE, D], F32, kind="ExternalOutput").ap()

    with tile.TileContext(nc) as tc:
        with (
            tc.tile_pool(name="const", bufs=1) as const_pool,
            tc.tile_pool(name="w", bufs=1) as w_pool,
            tc.tile_pool(name="xnat", bufs=2) as xnat_pool,
            tc.tile_pool(name="xt", bufs=2) as xt_pool,
            tc.tile_pool(name="qkv", bufs=2) as qkv_pool,
            tc.tile_pool(name="e", bufs=5) as e_pool,
            tc.tile_pool(name="soft", bufs=4) as soft_pool,
            tc.tile_pool(name="wvec", bufs=2) as wvec_pool,
            tc.tile_pool(name="scps", bufs=2, space="PSUM") as sc_psum,
            tc.tile_pool(name="gpps", bufs=2, space="PSUM") as gp_psum,
            tc.tile_pool(name="wps", bufs=1, space="PSUM") as w_psum,
        ):
            one_t = const_pool.tile([1, 1], BF16)
            nc.gpsimd.memset(one_t[:], 1.0)
            ident_f = const_pool.tile([P, P], F32)
            make_identity(nc, ident_f[:])
            ident = const_pool.tile([P, P], BF16)
            nc.vector.tensor_copy(ident[:], ident_f[:])

            # x arrives via SWDGE cast-DMA (f32 -> bf16) into natural-layout
            # staging tiles; the transpose to xT happens on the PE as a
            # REGULAR identity matmul (out = x_tile.T @ I).  Unlike
            # transpose-mode matmuls, these pipeline their weight loads and
            # count as PE activity for the HAM clock gate, and unlike the DMA
            # crossbar transpose they don't serialize the DMA subsystem.
            def dma_x_chunk(b, sc, xnat_s):
                nc.gpsimd.dma_start(
                    out=xnat_s[:, sc * 4 : (sc + 1) * 4, :],
                    in_=x_ext[b, sc * NCH : (sc + 1) * NCH, :].rearrange(
                        "(t p) d -> p t d", p=P
                    ),
                )

            w_tiles = {}

            def dma_w(name, ext):
                w_s = w_pool.tile([P, N_DT, D], BF16, tag=name)
                nc.gpsimd.dma_start(
                    out=w_s[:], in_=ext.rearrange("(t p) e -> p t e", p=P)
                )
                w_tiles[name] = w_s

            # Batch 0's x chunks and the weight loads share the SWDGE queue;
            # order so each lands just before the PE needs it.
            xnat0_s = xnat_pool.tile([P, N_ST, D], BF16, tag="xnat")
            x0_loaded = [False] * N_SC
            # s-tile 0 rides the parallel HWDGE queue as f32 (lands ~3us
            # before the SWDGE cast chain's first byte); its transposes run
            # as fp32 identity matmuls and a DVE downcast provides the bf16
            # natural-layout copy the final matvec needs.
            xf0 = xnat_pool.tile([P, D], F32, tag="xf0")
            nc.sync.dma_start(out=xf0[:], in_=x_ext[0, 0:P, :])
            nc.vector.tensor_copy(xnat0_s[:, 0, :], xf0[:])
            nc.gpsimd.dma_start(
                out=xnat0_s[:, 1:4, :],
                in_=x_ext[0, P:NCH, :].rearrange("(t p) d -> p t d", p=P),
            )
            x0_loaded[0] = True
            dma_w("wk", wk_ext)
            dma_w("wq", wq_ext)
            dma_x_chunk(0, 1, xnat0_s)
            x0_loaded[1] = True
            dma_x_chunk(0, 2, xnat0_s)
            x0_loaded[2] = True
            dma_x_chunk(0, 3, xnat0_s)
            x0_loaded[3] = True
            dma_w("wv", wv_ext)
            wk_s, wq_s, wv_s = w_tiles["wk"], w_tiles["wq"], w_tiles["wv"]

            # One-time prework: scores = (X Wq)(X Wk)^T = X M X^T with
            # M = Wq Wk^T [D, D].  Computing M once (per core) replaces the
            # two per-batch Q/K projections with a single G = X M projection.
            wqT_s = w_pool.tile([P, N_DT, D], BF16, tag="wqT")
            wkT_s = w_pool.tile([P, N_DT, D], BF16, tag="wkT")
            m_s = w_pool.tile([P, N_DT, D], BF16, tag="m")

            def m_prework_thunks():
                thunks = []

                def make_wtrans_unit(src_w, dst, t_e):
                    def th():
                        tp = sc_psum.tile([P, N_DT * P], F32, tag="sc")
                        for t_a in range(N_DT):
                            nc.tensor.matmul(
                                tp[:, t_a * P : (t_a + 1) * P],
                                src_w[:, t_a, t_e * P : (t_e + 1) * P],
                                ident[:],
                                start=True,
                                stop=True,
                                skip_group_check=True,
                            )
                        nc.vector.tensor_copy(
                            dst[:, t_e, :],
                            tp[:],
                        )

                    return th

                def make_m_group(t_a):
                    def th():
                        mp = gp_psum.tile([P, NCH], F32, tag="gp")
                        for t_e in range(N_DT):
                            nc.tensor.matmul(
                                mp[:],
                                wqT_s[:, t_e, t_a * P : (t_a + 1) * P],
                                wkT_s[:, t_e, :],
                                start=(t_e == 0),
                                stop=(t_e == N_DT - 1),
                            )
                        nc.vector.tensor_copy(m_s[:, t_a, :], mp[:])

                    return th

                for t_e in range(N_DT):
                    thunks.append(make_wtrans_unit(wk_s, wkT_s, t_e))
                for t_e in range(N_DT):
                    thunks.append(make_wtrans_unit(wq_s, wqT_s, t_e))
                for t_a in range(N_DT):
                    thunks.append(make_m_group(t_a))
                return thunks

            # ---------- thunk builders (emission deferred for interleaving) --

            def proj_thunks(b, xnat_s, loaded):
                """Transpose + G = X M projection thunks for batch b."""
                xt_s = xt_pool.tile([P, N_DT, S], BF16, tag="xt")
                gt_s = qkv_pool.tile([P, N_DT, S], BF16, tag="gt")

                def make_dma(sc):
                    def th():
                        dma_x_chunk(b, sc, xnat_s)

                    return th

                dma_th = [
                    None if loaded[sc] else make_dma(sc) for sc in range(N_SC)
                ]

                def make_trans_unit(sc, t_i):
                    def th():
                        st = sc * 4 + t_i
                        tp = sc_psum.tile([P, N_DT * P], F32, tag="sc")
                        for dt_i in range(N_DT):
                            nc.tensor.matmul(
                                tp[:, dt_i * P : (dt_i + 1) * P],
                                xnat_s[:, st, dt_i * P : (dt_i + 1) * P],
                                ident[:],
                                start=True,
                                stop=True,
                                skip_group_check=True,
                            )
                        nc.vector.tensor_copy(
                            xt_s[:, :, st * P : (st + 1) * P],
                            tp[:].rearrange("p (t c) -> p t c", t=N_DT),
                        )

                    return th

                trans_th = [
                    [make_trans_unit(sc, t_i) for t_i in range(4)]
                    for sc in range(N_SC)
                ]

                def make_g(sc, ct):
                    def th():
                        mp = gp_psum.tile([P, NCH], F32, tag="gp")
                        for kd in range(N_DT):
                            nc.tensor.matmul(
                                mp[:],
                                m_s[:, kd, ct * P : (ct + 1) * P],
                                xt_s[:, kd, sc * NCH : (sc + 1) * NCH],
                                start=(kd == 0),
                                stop=(kd == N_DT - 1),
                            )
                        nc.vector.tensor_copy(
                            gt_s[:, ct, sc * NCH : (sc + 1) * NCH], mp[:]
                        )

                    return th

                kq_th = [
                    [make_g(sc, ct) for ct in range(N_DT)]
                    for sc in range(N_SC)
                ]
                return (gt_s, xt_s), dma_th, trans_th, kq_th

            def emit_ltp(dma_th, trans_th, kq_th, extra=None):
                """Emit the transpose/projection stream: chunk sc+1's
                transposes weave between chunk sc's projection groups so the
                PE stream stays dense."""
                extra = list(extra) if extra else []
                ei = 0
                if dma_th[0] is not None:
                    dma_th[0]()
                    dma_th[0] = None
                for th in trans_th[0]:
                    th()
                for sc in range(N_SC):
                    for j in (sc + 1, sc + 2):
                        if j < N_SC and dma_th[j] is not None:
                            dma_th[j]()
                            dma_th[j] = None
                    nxt = trans_th[sc + 1] if sc + 1 < N_SC else []
                    groups = list(kq_th[sc])
                    ti = 0
                    for g_i, g in enumerate(groups):
                        g()
                        while ti < len(nxt) and ti * len(groups) < (g_i + 1) * len(nxt):
                            nxt[ti]()
                            ti += 1
                        if ei < len(extra):
                            extra[ei]()
                            ei += 1
                    while ti < len(nxt):
                        nxt[ti]()
                        ti += 1
                while ei < len(extra):
                    extra[ei]()
                    ei += 1

            def emit_scores_qt(gt_s, xt_s, qt):
                """scores + exp + rowsum + reciprocal for one q-tile."""
                e_t = e_pool.tile([P, S], BF16, tag="e")
                rsum = soft_pool.tile([P, N_KC], F32, tag="rsum")
                for kc in range(N_KC):
                    sp = sc_psum.tile([P, NCH], F32, tag="sc")
                    for et in range(N_DT):
                        nc.tensor.matmul(
                            sp[:],
                            gt_s[:, et, qt * P : (qt + 1) * P],
                            xt_s[:, et, kc * NCH : (kc + 1) * NCH],
                            start=(et == 0),
                            stop=(et == N_DT - 1),
                        )
                    nc.scalar.activation(
                        e_t[:, kc * NCH : (kc + 1) * NCH],
                        sp[:],
                        mybir.ActivationFunctionType.Exp,
                        scale=SCALE,
                        accum_out=rsum[:, kc : kc + 1],
                    )
                rtot = soft_pool.tile([P, 1], F32, tag="rtot")
                nc.vector.reduce_sum(rtot[:], rsum[:], axis=mybir.AxisListType.X)
                rrec = soft_pool.tile([P, 1], F32, tag="rrec")
                nc.vector.reciprocal(rrec[:], rtot[:])
                # M=1 matmuls issue ~25% slower than M=128 ones; broadcast r
                # across a full 128-wide stationary tile (every PSUM row then
                # equals r^T E) to keep the colsum at full rate.
                r_t = soft_pool.tile([P, P], BF16, tag="r")
                nc.vector.tensor_copy(r_t[:], rrec[:, 0:1].broadcast_to([P, P]))
                return e_t, r_t

            def emit_colsum_qt(w_ps, e_t, r_t, qt):
                """w_ps[:, kc, :] += bcast(r_qt)^T @ E_qt (every row = colsum)."""
                for kc in range(N_KC):
                    nc.tensor.matmul(
                        w_ps[:, kc, :],
                        r_t[:],
                        e_t[:, kc * NCH : (kc + 1) * NCH],
                        start=(qt == 0),
                        stop=(qt == N_ST - 1),
                        skip_group_check=True,
                    )

            def phase_scores(b, gt_s, xt_s, per_qt_extra=None):
                w_ps = w_psum.tile([P, N_KC, NCH], F32, tag="w")
                pending = []
                for qt in range(N_ST):
                    cur = emit_scores_qt(gt_s, xt_s, qt)
                    # emit colsums in PAIRS so the scores<->colsum stationary
                    # swap (an LDWEIGHTS pipeline break) happens half as often
                    if len(pending) == 2:
                        for pqt, (pe, pr) in pending:
                            emit_colsum_qt(w_ps, pe, pr, pqt)
                        pending = []
                    pending.append((qt, cur))
                    if per_qt_extra is not None and qt < len(per_qt_extra):
                        per_qt_extra[qt]()
                for pqt, (pe, pr) in pending:
                    emit_colsum_qt(w_ps, pe, pr, pqt)
                return w_ps

            def final_thunks(b, w_ps, xnat_s):
                """w-phase thunks, using out = (w @ X) @ W_v so no V
                projection is ever materialized: 4 ACT copies of w, 16 (PE
                row->column transpose + DVE broadcast), 16 y-accumulation
                matmuls y = w @ X, then the tiny epilogue y @ W_v and the
                output copy + DMA.  Emitted interleaved by the caller."""
                w_sb = wvec_pool.tile([1, S], BF16, tag="wsb")
                y_ps = sc_psum.tile([P, NCH], F32, tag="sc")
                wt_pads = {}
                yt_pads = {}
                thunks = []

                def make_wcopy(kc):
                    def th():
                        eng = nc.scalar.copy if kc % 2 == 0 else nc.vector.tensor_copy
                        eng(w_sb[:, kc * NCH : (kc + 1) * NCH], w_ps[0:1, kc, :])

                    return th

                def row_to_bcast_cols(src_row, pads, key, tag):
                    """[1,128] SBUF row chunk -> K=1 matmul -> [128,1] PSUM
                    column -> DVE broadcast to a [128,128] stationary tile."""
                    tp = gp_psum.tile([P, 1], F32, tag="gp")
                    nc.tensor.matmul(
                        tp[:], src_row, one_t[0:1, 0:1], start=True, stop=True
                    )
                    pad = wvec_pool.tile([P, P], BF16, tag=tag)
                    nc.vector.tensor_copy(pad[:], tp[:, 0:1].broadcast_to([P, P]))
                    pads[key] = pad

                def make_wtrans(kt):
                    def th():
                        row_to_bcast_cols(
                            w_sb[0:1, kt * P : (kt + 1) * P],
                            wt_pads, kt, f"wtp{kt % 4}",
                        )

                    return th

                def make_ymm(st):
                    def th():
                        nc.tensor.matmul(
                            y_ps[:],
                            wt_pads[st][:],
                            xnat_s[:, st, :],
                            start=(st == 0),
                            stop=(st == N_ST - 1),
                            skip_group_check=True,
                        )

                    return th

                def epilogue_th():
                    # y [1, D] -> o = y @ W_v  (4 K=1 transposes + 4 matmuls)
                    y_sb = wvec_pool.tile([1, NCH], BF16, tag="ysb")
                    nc.scalar.copy(y_sb[:], y_ps[0:1, :])
                    o_ps = gp_psum.tile([P, NCH], F32, tag="gp")
                    for c in range(N_DT):
                        row_to_bcast_cols(
                            y_sb[0:1, c * P : (c + 1) * P], yt_pads, c, f"ytp{c}"
                        )
                    for c in range(N_DT):
                        nc.tensor.matmul(
                            o_ps[:],
                            yt_pads[c][:],
                            wv_s[:, c, :],
                            start=(c == 0),
                            stop=(c == N_DT - 1),
                            skip_group_check=True,
                        )
                    o_sb = wvec_pool.tile([1, NCH], F32, tag="osb")
                    nc.scalar.copy(o_sb[:], o_ps[0:1, :])
                    nc.sync.dma_start(out=out_ext[b : b + 1, :], in_=o_sb[:])

                for kc in range(N_KC):
                    thunks.append(make_wcopy(kc))
                for kt in range(N_ST):
                    thunks.append(make_wtrans(kt))
                    if kt >= 3:
                        thunks.append(make_ymm(kt - 3))
                for st in range(N_ST - 3, N_ST):
                    thunks.append(make_ymm(st))
                thunks.append(epilogue_th)
                return thunks

            # ------------------------- emission ------------------------------

            # batch 0: M prework + transposes woven into the G projection
            h0, dma0, trans0, kq0 = proj_thunks(0, xnat0_s, x0_loaded)
            g0, xt0 = h0
            if dma0[0] is not None:
                dma0[0]()
                dma0[0] = None

            def first_tile_trans_f32():
                tp = sc_psum.tile([P, N_DT * P], F32, tag="sc")
                for dt_i in range(N_DT):
                    nc.tensor.matmul(
                        tp[:, dt_i * P : (dt_i + 1) * P],
                        xf0[:, dt_i * P : (dt_i + 1) * P],
                        ident_f[:],
                        start=True,
                        stop=True,
                        skip_group_check=True,
                    )
                nc.vector.tensor_copy(
                    xt0[:, :, 0:P],
                    tp[:].rearrange("p (t c) -> p t c", t=N_DT),
                )

            first_tile_trans_f32()
            for th in trans0[0][1:]:
                th()
            for th in m_prework_thunks():
                th()
            trans0 = [[], *trans0[1:]]
            emit_ltp(dma0, trans0, kq0)

            wps0 = phase_scores(0, g0, xt0)

            # batch 1 transposes/projections with batch 0's w-phase woven in
            xnat1_s = xnat_pool.tile([P, N_ST, D], BF16, tag="xnat")
            h1, dma1, trans1, kq1 = proj_thunks(1, xnat1_s, [False] * N_SC)
            g1, xt1 = h1
            emit_ltp(dma1, trans1, kq1, extra=final_thunks(0, wps0, xnat0_s))

            wps1 = phase_scores(1, g1, xt1)

            for th in final_thunks(1, wps1, xnat1_s):
                th()

    nc.compile()
    return nc


_NC_CACHE = None


def _get_nc():
    global _NC_CACHE
    if _NC_CACHE is None:
        _NC_CACHE = build_nc()
    return _NC_CACHE


def make_in_maps(inputs, W_q, W_k, W_v):
    inputs = np.ascontiguousarray(np.asarray(inputs, dtype=np.float32))
    W_q = np.ascontiguousarray(np.asarray(W_q, dtype=np.float32))
    W_k = np.ascontiguousarray(np.asarray(W_k, dtype=np.float32))
    W_v = np.ascontiguousarray(np.asarray(W_v, dtype=np.float32))
    return [
        {
            "inputs": inputs[i * B_PER_CORE : (i + 1) * B_PER_CORE],
            "W_q": W_q,
            "W_k": W_k,
            "W_v": W_v,
        }
        for i in range(N_CORES)
    ]


def kernel(**inputs) -> np.ndarray:
    nc = _get_nc()
    in_maps = make_in_maps(
        inputs["inputs"], inputs["W_q"], inputs["W_k"], inputs["W_v"]
    )
    res = run_bass_kernel_spmd(nc, in_maps, core_ids=list(range(N_CORES)))
    return np.concatenate(
        [res.results[i]["out"] for i in range(N_CORES)], axis=0
    ).astype(np.float32)



# revision 6
# speedup vs baseline: 1.2113x; 1.2113x over previous
"""Trainium2 Bass kernel for batched single-head attention with seq-sum pooling.

Reference computation (B=16, S=2048, D=512, fp32):
    q = x @ W_q ; k = x @ W_k ; v = x @ W_v          per batch  [S, D]
    scores = q @ k.T / sqrt(D)                        [S, S]
    attn = softmax(scores, axis=-1)
    out_b = sum_s (attn @ v)[s, :]                    [D]

Algebraic restructures (carried over from the bf16 version):
1. The final sum over query positions commutes through both trailing
   matmuls: out_b = (sum_q attn[q,:]) @ (x @ W_v) = ((r^T E) @ x) @ W_v,
   where E = exp(scores/sqrt(D)) and r[q] = 1/rowsum_q(E) — removes the
   [S,S]x[S,D] attention-value matmul AND the V projection.
2. scores = (x W_q)(x W_k)^T = x M x^T with M = W_q W_k^T computed ONCE per
   core — replaces both per-batch Q/K projections with a single G = x M
   projection, and x^T itself becomes the scores key operand.

fp8 acceleration (this version): the three bulk matmuls — G = M^T X^T, the
scores matmul G X^T, and the softmax column-sum — run with float8e4 operands
in DoubleRow perf mode (K=256 per instruction, 0.5 PE cycles per output
column = 4x bf16 matmul throughput).  Numerics are kept in range with three
exact power-of-2/constant foldings:
  * M is stored as 16*M (M entries ~N(0,1/512) would be fp8-subnormal);
    the exp activation scale divides the 16 back out.
  * E is computed as exp(s/sqrt(D) - 2.5): the global offset cancels through
    the softmax normalization and keeps E below the fp8e4 max of 240.
  * The row-normalizer r=1/Z is stored as 128*r (r~4e-3 would be subnormal);
    the 1/128 is folded into the final [1,D] output copy.
Simulated end-to-end rel err of this mix vs the f32 reference: 7.6e-3
(tolerance 2e-2).  With the PE 4x faster, the exp activation over the [S,S]
score matrix (~2.2us per 128-row q-tile on the ACT engine) becomes the
critical path; the emission interleaves batch 1's transpose/projection PE
work into batch 0's ACT-bound scores phase, and batch 0's w-phase into batch
1's scores phase (y_ps lives in the gp PSUM pool so the scores double-buffer
keeps both sc banks).

Sharding: pure data parallelism over batch — 2 batch elements per core on 8
NeuronCores, weights replicated, no collectives.  Host concatenates per-core
[2, D] outputs.
"""

import sys

sys.path.insert(0, "/opt/trn_rl_repo")

import numpy as np

import concourse.bass as bass
import concourse.mybir as mybir
import concourse.tile as tile
from concourse import bacc
from concourse.bass_utils import run_bass_kernel_spmd
from concourse.masks import make_identity

B, S, D = 16, 2048, 512
P = 128
N_CORES = 8
B_PER_CORE = B // N_CORES  # 2
SCALE = 1.0 / float(np.sqrt(D))
KM = 16.0  # M pre-scale (exact power of 2)
KR = 128.0  # r pre-scale (exact power of 2)
C_OFF = 2.5  # global exp offset; cancels through softmax normalization

F32 = mybir.dt.float32
BF16 = mybir.dt.bfloat16
F8 = mybir.dt.float8e4
DR = mybir.MatmulPerfMode.DoubleRow

N_ST = S // P  # 16 s-tiles (partition tiles of the sequence dim)
N_DT = D // P  # 4 d-tiles (partition tiles of the feature dim)
NCH = 512  # moving free dim per matmul (one fp32 PSUM bank)
N_SC = S // NCH  # 4 s-chunks of the sequence dim
N_KC = S // NCH  # 4 k-chunks of the key dim


def build_nc():
    nc = bacc.Bacc("TRN2", target_bir_lowering=False, debug=False, num_devices=N_CORES)
    x_ext = nc.dram_tensor(
        "inputs", [B_PER_CORE, S, D], F32, kind="ExternalInput"
    ).ap()
    wq_ext = nc.dram_tensor("W_q", [D, D], F32, kind="ExternalInput").ap()
    wk_ext = nc.dram_tensor("W_k", [D, D], F32, kind="ExternalInput").ap()
    wv_ext = nc.dram_tensor("W_v", [D, D], F32, kind="ExternalInput").ap()
    out_ext = nc.dram_tensor("out", [B_PER_CORE, D], F32, kind="ExternalOutput").ap()

    with tile.TileContext(nc) as tc:
        with (
            tc.tile_pool(name="const", bufs=1) as const_pool,
            tc.tile_pool(name="w", bufs=1) as w_pool,
            tc.tile_pool(name="xnat", bufs=2) as xnat_pool,
            tc.tile_pool(name="xt", bufs=2) as xt_pool,
            tc.tile_pool(name="qkv", bufs=2) as qkv_pool,
            tc.tile_pool(name="e", bufs=3) as e_pool,
            tc.tile_pool(name="soft", bufs=4) as soft_pool,
            tc.tile_pool(name="r2", bufs=3) as r2_pool,
            tc.tile_pool(name="wvec", bufs=2) as wvec_pool,
            tc.tile_pool(name="scps", bufs=2, space="PSUM") as sc_psum,
            tc.tile_pool(name="gpps", bufs=2, space="PSUM") as gp_psum,
            tc.tile_pool(name="wps", bufs=1, space="PSUM") as w_psum,
        ):
            one_t = const_pool.tile([1, 1], BF16)
            nc.gpsimd.memset(one_t[:], 1.0)
            ident_f = const_pool.tile([P, P], F32)
            make_identity(nc, ident_f[:])
            ident = const_pool.tile([P, P], BF16)
            nc.vector.tensor_copy(ident[:], ident_f[:])
            negc_t = const_pool.tile([P, 1], F32)
            nc.gpsimd.memset(negc_t[:], -C_OFF)

            # x arrives via SWDGE cast-DMA (f32 -> bf16) into natural-layout
            # staging tiles; the transpose to xT happens on the PE as a
            # REGULAR identity matmul (out = x_tile.T @ I), cast to fp8 on the
            # PSUM->SBUF copy.
            def dma_x_chunk(b, sc, xnat_s):
                nc.gpsimd.dma_start(
                    out=xnat_s[:, sc * 4 : (sc + 1) * 4, :],
                    in_=x_ext[b, sc * NCH : (sc + 1) * NCH, :].rearrange(
                        "(t p) d -> p t d", p=P
                    ),
                )

            w_tiles = {}

            def dma_w(name, ext):
                w_s = w_pool.tile([P, N_DT, D], BF16, tag=name)
                nc.gpsimd.dma_start(
                    out=w_s[:], in_=ext.rearrange("(t p) e -> p t e", p=P)
                )
                w_tiles[name] = w_s

            # Batch 0's x chunks and the weight loads share the SWDGE queue;
            # order so each lands just before the PE needs it.
            xnat0_s = xnat_pool.tile([P, N_ST, D], BF16, tag="xnat")
            x0_loaded = [False] * N_SC
            # s-tile 0 rides the parallel HWDGE queue as f32 (lands ~3us
            # before the SWDGE cast chain's first byte); its transposes run
            # as fp32 identity matmuls and a DVE downcast provides the bf16
            # natural-layout copy the final matvec needs.
            xf0 = xnat_pool.tile([P, D], F32, tag="xf0")
            nc.sync.dma_start(out=xf0[:], in_=x_ext[0, 0:P, :])
            nc.vector.tensor_copy(xnat0_s[:, 0, :], xf0[:])
            nc.gpsimd.dma_start(
                out=xnat0_s[:, 1:4, :],
                in_=x_ext[0, P:NCH, :].rearrange("(t p) d -> p t d", p=P),
            )
            x0_loaded[0] = True
            dma_w("wk", wk_ext)
            dma_w("wq", wq_ext)
            dma_x_chunk(0, 1, xnat0_s)
            x0_loaded[1] = True
            dma_x_chunk(0, 2, xnat0_s)
            x0_loaded[2] = True
            dma_x_chunk(0, 3, xnat0_s)
            x0_loaded[3] = True
            dma_w("wv", wv_ext)
            wk_s, wq_s, wv_s = w_tiles["wk"], w_tiles["wq"], w_tiles["wv"]

            # One-time prework: scores = (X Wq)(X Wk)^T = X M X^T with
            # M = Wq Wk^T [D, D].  m8 stores 16*M in fp8 (the raw M entries
            # ~N(0, 1/512) would land in fp8-subnormal range); the 16 is
            # divided back out by the exp activation scale.  wqT is scaled by
            # 16 at its PSUM->SBUF copy (ACT, idle during prework) so the M
            # matmul's PSUM result is already 16*M and the m8 copy is a plain
            # DVE cast.
            wqT_s = w_pool.tile([P, N_DT, D], BF16, tag="wqT")
            wkT_s = w_pool.tile([P, N_DT, D], BF16, tag="wkT")
            m8_s = w_pool.tile([P, N_DT, D], F8, tag="m8")

            def m_prework_thunks():
                thunks = []

                def make_wtrans_unit(src_w, dst, t_e, scale):
                    def th():
                        tp = sc_psum.tile([P, N_DT * P], F32, tag="sc")
                        for t_a in range(N_DT):
                            nc.tensor.matmul(
                                tp[:, t_a * P : (t_a + 1) * P],
                                src_w[:, t_a, t_e * P : (t_e + 1) * P],
                                ident[:],
                                start=True,
                                stop=True,
                                skip_group_check=True,
                            )
                        if scale is None:
                            nc.vector.tensor_copy(dst[:, t_e, :], tp[:])
                        else:
                            nc.scalar.mul(dst[:, t_e, :], tp[:], scale)

                    return th

                def make_m_group(t_a):
                    def th():
                        mp = gp_psum.tile([P, NCH], F32, tag="gp")
                        for t_e in range(N_DT):
                            nc.tensor.matmul(
                                mp[:],
                                wqT_s[:, t_e, t_a * P : (t_a + 1) * P],
                                wkT_s[:, t_e, :],
                                start=(t_e == 0),
                                stop=(t_e == N_DT - 1),
                            )
                        nc.vector.tensor_copy(m8_s[:, t_a, :], mp[:])

                    return th

                for t_e in range(N_DT):
                    thunks.append(make_wtrans_unit(wk_s, wkT_s, t_e, None))
                for t_e in range(N_DT):
                    thunks.append(make_wtrans_unit(wq_s, wqT_s, t_e, KM))
                for t_a in range(N_DT):
                    thunks.append(make_m_group(t_a))
                return thunks

            # ---------- thunk builders (emission deferred for interleaving) --

            def proj_thunks(b, xnat_s, loaded):
                """Transpose + G = X M projection thunks for batch b.
                xt8/gt8 are fp8; the G matmul runs fp8 DoubleRow."""
                xt8_s = xt_pool.tile([P, N_DT, S], F8, tag="xt")
                gt8_s = qkv_pool.tile([P, N_DT, S], F8, tag="gt")

                def make_dma(sc):
                    def th():
                        dma_x_chunk(b, sc, xnat_s)

                    return th

                dma_th = [
                    None if loaded[sc] else make_dma(sc) for sc in range(N_SC)
                ]

                def make_trans_unit(sc, t_i):
                    def th():
                        st = sc * 4 + t_i
                        tp = sc_psum.tile([P, N_DT * P], F32, tag="sc")
                        for dt_i in range(N_DT):
                            nc.tensor.matmul(
                                tp[:, dt_i * P : (dt_i + 1) * P],
                                xnat_s[:, st, dt_i * P : (dt_i + 1) * P],
                                ident[:],
                                start=True,
                                stop=True,
                                skip_group_check=True,
                            )
                        nc.vector.tensor_copy(
                            xt8_s[:, :, st * P : (st + 1) * P],
                            tp[:].rearrange("p (t c) -> p t c", t=N_DT),
                        )

                    return th

                trans_th = [
                    [make_trans_unit(sc, t_i) for t_i in range(4)]
                    for sc in range(N_SC)
                ]

                def make_g(sc, ct):
                    def th():
                        mp = gp_psum.tile([P, NCH], F32, tag="gp")
                        for j in range(2):
                            nc.tensor.matmul(
                                mp[:],
                                m8_s[:, 2 * j : 2 * j + 2, ct * P : (ct + 1) * P],
                                xt8_s[:, 2 * j : 2 * j + 2, sc * NCH : (sc + 1) * NCH],
                                start=(j == 0),
                                stop=(j == 1),
                                perf_mode=DR,
                            )
                        nc.vector.tensor_copy(
                            gt8_s[:, ct, sc * NCH : (sc + 1) * NCH], mp[:]
                        )

                    return th

                kq_th = [
                    [make_g(sc, ct) for ct in range(N_DT)]
                    for sc in range(N_SC)
                ]
                return (gt8_s, xt8_s), dma_th, trans_th, kq_th

            def flat_proj_thunks(dma_th, trans_th, kq_th):
                """Dependency-ordered flat list of one batch's proj thunks,
                for weaving into another batch's scores phase: dma(sc+1) and
                trans(sc+1) interleave with the G groups of chunk sc."""
                out = []
                if dma_th[0] is not None:
                    out.append(dma_th[0])
                for th in trans_th[0]:
                    out.append(th)
                for sc in range(N_SC):
                    if sc + 1 < N_SC and dma_th[sc + 1] is not None:
                        out.append(dma_th[sc + 1])
                    nxt = trans_th[sc + 1] if sc + 1 < N_SC else []
                    for g_i, g in enumerate(kq_th[sc]):
                        out.append(g)
                        if g_i < len(nxt):
                            out.append(nxt[g_i])
                return out

            def emit_ltp(dma_th, trans_th, kq_th, extra=None):
                """Emit the transpose/projection stream densely (batch 0's
                unhidden proj phase)."""
                extra = list(extra) if extra else []
                ei = 0
                if dma_th[0] is not None:
                    dma_th[0]()
                    dma_th[0] = None
                for th in trans_th[0]:
                    th()
                for sc in range(N_SC):
                    for j in (sc + 1, sc + 2):
                        if j < N_SC and dma_th[j] is not None:
                            dma_th[j]()
                            dma_th[j] = None
                    nxt = trans_th[sc + 1] if sc + 1 < N_SC else []
                    groups = list(kq_th[sc])
                    ti = 0
                    for g_i, g in enumerate(groups):
                        g()
                        while ti < len(nxt) and ti * len(groups) < (g_i + 1) * len(nxt):
                            nxt[ti]()
                            ti += 1
                        if ei < len(extra):
                            extra[ei]()
                            ei += 1
                    while ti < len(nxt):
                        nxt[ti]()
                        ti += 1
                while ei < len(extra):
                    extra[ei]()
                    ei += 1

            def emit_scores_qt(gt8_s, xt8_s, qt, e2_t, r2_t):
                """scores (fp8 DoubleRow) + exp (fp8 out, offset -C_OFF) +
                rowsum + scaled reciprocal for one q-tile; writes into the
                qt%2 lane of the pair tiles e2_t / r2_t."""
                par = qt % 2
                rsum = soft_pool.tile([P, N_KC], F32, tag="rsum")
                for kc in range(N_KC):
                    sp = sc_psum.tile([P, NCH], F32, tag="sc")
                    for j in range(2):
                        nc.tensor.matmul(
                            sp[:],
                            gt8_s[:, 2 * j : 2 * j + 2, qt * P : (qt + 1) * P],
                            xt8_s[:, 2 * j : 2 * j + 2, kc * NCH : (kc + 1) * NCH],
                            start=(j == 0),
                            stop=(j == 1),
                            perf_mode=DR,
                        )
                    nc.scalar.activation(
                        e2_t[:, par, kc * NCH : (kc + 1) * NCH],
                        sp[:],
                        mybir.ActivationFunctionType.Exp,
                        scale=SCALE / KM,
                        bias=negc_t[:],
                        accum_out=rsum[:, kc : kc + 1],
                    )
                rtot = soft_pool.tile([P, 1], F32, tag="rtot")
                nc.vector.reduce_sum(rtot[:], rsum[:], axis=mybir.AxisListType.X)
                rtot_s = soft_pool.tile([P, 1], F32, tag="rtots")
                nc.vector.tensor_scalar_mul(rtot_s[:], rtot[:], 1.0 / KR)
                rrec = soft_pool.tile([P, 1], F32, tag="rrec")
                nc.vector.reciprocal(rrec[:], rtot_s[:])  # = KR / rowsum
                nc.vector.tensor_copy(
                    r2_t[:, par, :], rrec[:, 0:1].broadcast_to([P, P])
                )

            def emit_colsum_pair(w_ps, e2_t, r2_t, pair):
                """w_ps[:, kc, :] += bcast(r_pair)^T @ E_pair via ONE fp8
                DoubleRow matmul per kc (contracts both q-subtiles at once;
                every PSUM row ends up equal to the weighted column sum)."""
                for kc in range(N_KC):
                    nc.tensor.matmul(
                        w_ps[:, kc, :],
                        r2_t[:, 0:2, :],
                        e2_t[:, 0:2, kc * NCH : (kc + 1) * NCH],
                        start=(pair == 0),
                        stop=(pair == N_ST // 2 - 1),
                        perf_mode=DR,
                        skip_group_check=True,
                    )

            def phase_scores(b, gt8_s, xt8_s, per_qt_extra=None):
                """16 q-tiles of scores+exp, with pair colsums lagged by one
                q-tile; per_qt_extra[qt] is a list of thunks woven in after
                each q-tile's chunk stream.  The w tile is allocated lazily at
                the first colsum so a previous batch's w-phase copies (woven
                into this phase's early extras) are emitted before the
                bufs=1 pool hands the banks to this batch."""
                w_holder = {}

                def get_wps():
                    if "t" not in w_holder:
                        w_ps = w_psum.tile([P, N_KC, NCH], F32, tag="w")
                        w_holder["t"] = w_ps
                    return w_holder["t"]

                pend = None  # (pair_idx, e2_t, r2_t) awaiting colsum
                e2_t = r2_t = None
                for qt in range(N_ST):
                    if qt % 2 == 0:
                        e2_t = e_pool.tile([P, 2, S], F8, tag="e2")
                        r2_t = r2_pool.tile([P, 2, P], F8, tag="r2")
                    emit_scores_qt(gt8_s, xt8_s, qt, e2_t, r2_t)
                    if qt % 2 == 1:
                        if pend is not None:
                            emit_colsum_pair(get_wps(), pend[1], pend[2], pend[0])
                        pend = (qt // 2, e2_t, r2_t)
                    if per_qt_extra is not None and qt < len(per_qt_extra):
                        for th in per_qt_extra[qt]:
                            th()
                if pend is not None:
                    emit_colsum_pair(get_wps(), pend[1], pend[2], pend[0])
                return w_holder["t"]

            def final_thunks(b, w_ps, xnat_s):
                """w-phase thunks, using out = (w @ X) @ W_v so no V
                projection is ever materialized.  w carries the KR=128
                pre-scale; the 1/128 is folded into the final o_sb copy.
                y_ps and the row->column transposes live in the gp PSUM pool
                so a concurrent scores phase keeps both sc banks."""
                w_sb = wvec_pool.tile([1, S], BF16, tag="wsb")
                y_ps = gp_psum.tile([P, NCH], F32, tag="gp")
                wt_pads = {}
                yt_pads = {}
                thunks = []

                def make_wcopy(kc):
                    def th():
                        nc.vector.tensor_copy(
                            w_sb[:, kc * NCH : (kc + 1) * NCH], w_ps[0:1, kc, :]
                        )

                    return th

                def row_to_bcast_cols(src_row, pads, key, tag):
                    """[1,128] SBUF row chunk -> K=1 matmul -> [128,1] PSUM
                    column -> DVE broadcast to a [128,128] stationary tile."""
                    tp = gp_psum.tile([P, 1], F32, tag="gp")
                    nc.tensor.matmul(
                        tp[:], src_row, one_t[0:1, 0:1], start=True, stop=True
                    )
                    pad = wvec_pool.tile([P, P], BF16, tag=tag)
                    nc.vector.tensor_copy(pad[:], tp[:, 0:1].broadcast_to([P, P]))
                    pads[key] = pad

                def make_wtrans(kt):
                    def th():
                        row_to_bcast_cols(
                            w_sb[0:1, kt * P : (kt + 1) * P],
                            wt_pads, kt, f"wtp{kt % 4}",
                        )

                    return th

                def make_ymm(st):
                    def th():
                        nc.tensor.matmul(
                            y_ps[:],
                            wt_pads[st][:],
                            xnat_s[:, st, :],
                            start=(st == 0),
                            stop=(st == N_ST - 1),
                            skip_group_check=True,
                        )

                    return th

                def epilogue_th():
                    # y [1, D] -> o = y @ W_v  (4 K=1 transposes + 4 matmuls)
                    y_sb = wvec_pool.tile([1, NCH], BF16, tag="ysb")
                    nc.vector.tensor_copy(y_sb[:], y_ps[0:1, :])
                    o_ps = gp_psum.tile([P, NCH], F32, tag="gp")
                    for c in range(N_DT):
                        row_to_bcast_cols(
                            y_sb[0:1, c * P : (c + 1) * P], yt_pads, c, f"ytp{c}"
                        )
                    for c in range(N_DT):
                        nc.tensor.matmul(
                            o_ps[:],
                            yt_pads[c][:],
                            wv_s[:, c, :],
                            start=(c == 0),
                            stop=(c == N_DT - 1),
                            skip_group_check=True,
                        )
                    o_sb = wvec_pool.tile([1, NCH], F32, tag="osb")
                    nc.vector.tensor_scalar_mul(o_sb[:], o_ps[0:1, :], 1.0 / KR)
                    nc.sync.dma_start(out=out_ext[b : b + 1, :], in_=o_sb[:])

                for kc in range(N_KC):
                    thunks.append(make_wcopy(kc))
                for kt in range(N_ST):
                    thunks.append(make_wtrans(kt))
                    if kt >= 3:
                        thunks.append(make_ymm(kt - 3))
                for st in range(N_ST - 3, N_ST):
                    thunks.append(make_ymm(st))
                thunks.append(epilogue_th)
                return thunks

            def spread(thunks, n_slots):
                """Distribute a flat thunk list over n_slots per-qt extras,
                preserving order."""
                slots = [[] for _ in range(n_slots)]
                k = len(thunks)
                for i, th in enumerate(thunks):
                    slots[min(i * n_slots // k, n_slots - 1)].append(th)
                return slots

            # ------------------------- emission ------------------------------

            # batch 0: M prework + transposes woven into the G projection
            h0, dma0, trans0, kq0 = proj_thunks(0, xnat0_s, x0_loaded)
            g0, xt0 = h0
            if dma0[0] is not None:
                dma0[0]()
                dma0[0] = None

            def first_tile_trans_f32():
                tp = sc_psum.tile([P, N_DT * P], F32, tag="sc")
                for dt_i in range(N_DT):
                    nc.tensor.matmul(
                        tp[:, dt_i * P : (dt_i + 1) * P],
                        xf0[:, dt_i * P : (dt_i + 1) * P],
                        ident_f[:],
                        start=True,
                        stop=True,
                        skip_group_check=True,
                    )
                nc.vector.tensor_copy(
                    xt0[:, :, 0:P],
                    tp[:].rearrange("p (t c) -> p t c", t=N_DT),
                )

            first_tile_trans_f32()
            for th in trans0[0][1:]:
                th()
            for th in m_prework_thunks():
                th()
            trans0 = [[], *trans0[1:]]
            emit_ltp(dma0, trans0, kq0)

            # batch 1 proj thunks, woven into batch 0's ACT-bound scores phase
            xnat1_s = xnat_pool.tile([P, N_ST, D], BF16, tag="xnat")
            h1, dma1, trans1, kq1 = proj_thunks(1, xnat1_s, [False] * N_SC)
            g1, xt1 = h1
            proj1_flat = flat_proj_thunks(dma1, trans1, kq1)
            wps0 = phase_scores(0, g0, xt0, per_qt_extra=spread(proj1_flat, N_ST))

            # batch 1 scores with batch 0's w-phase woven in
            wps1 = phase_scores(
                1, g1, xt1, per_qt_extra=spread(final_thunks(0, wps0, xnat0_s), N_ST)
            )

            for th in final_thunks(1, wps1, xnat1_s):
                th()

    nc.compile()
    return nc


_NC_CACHE = None


def _get_nc():
    global _NC_CACHE
    if _NC_CACHE is None:
        _NC_CACHE = build_nc()
    return _NC_CACHE


def make_in_maps(inputs, W_q, W_k, W_v):
    inputs = np.ascontiguousarray(np.asarray(inputs, dtype=np.float32))
    W_q = np.ascontiguousarray(np.asarray(W_q, dtype=np.float32))
    W_k = np.ascontiguousarray(np.asarray(W_k, dtype=np.float32))
    W_v = np.ascontiguousarray(np.asarray(W_v, dtype=np.float32))
    return [
        {
            "inputs": inputs[i * B_PER_CORE : (i + 1) * B_PER_CORE],
            "W_q": W_q,
            "W_k": W_k,
            "W_v": W_v,
        }
        for i in range(N_CORES)
    ]


def kernel(**inputs) -> np.ndarray:
    nc = _get_nc()
    in_maps = make_in_maps(
        inputs["inputs"], inputs["W_q"], inputs["W_k"], inputs["W_v"]
    )
    res = run_bass_kernel_spmd(nc, in_maps, core_ids=list(range(N_CORES)))
    return np.concatenate(
        [res.results[i]["out"] for i in range(N_CORES)], axis=0
    ).astype(np.float32)


# revision 21
# speedup vs baseline: 1.2172x; 1.0049x over previous
"""Trainium2 Bass kernel for batched single-head attention with seq-sum pooling.

Reference computation (B=16, S=2048, D=512, fp32):
    q = x @ W_q ; k = x @ W_k ; v = x @ W_v          per batch  [S, D]
    scores = q @ k.T / sqrt(D)                        [S, S]
    attn = softmax(scores, axis=-1)
    out_b = sum_s (attn @ v)[s, :]                    [D]

Algebraic restructures (carried over from the bf16 version):
1. The final sum over query positions commutes through both trailing
   matmuls: out_b = (sum_q attn[q,:]) @ (x @ W_v) = ((r^T E) @ x) @ W_v,
   where E = exp(scores/sqrt(D)) and r[q] = 1/rowsum_q(E) — removes the
   [S,S]x[S,D] attention-value matmul AND the V projection.
2. scores = (x W_q)(x W_k)^T = x M x^T with M = W_q W_k^T computed ONCE per
   core — replaces both per-batch Q/K projections with a single G = x M
   projection, and x^T itself becomes the scores key operand.

fp8 acceleration: the three bulk matmuls — G = M^T X^T, the scores matmul
G X^T, and the softmax column-sum — run with float8e4 operands in DoubleRow
perf mode (K=256 per instruction, 2 PE rows per cycle).  Numerics stay in
range via exact foldings: M is stored as 16*M (raw entries would be
fp8-subnormal; the exp activation scale divides it back out), E is computed
as exp(s/sqrt(D) - 2.5) (the global offset cancels through the softmax
normalization and keeps E below fp8e4's max of 240), and r=1/Z is stored as
128*r (folded back in the final output copy).  Measured end-to-end rel err
7.5e-3 (tolerance 2e-2).

Engine-balance structure (the v2 trace showed ACT exp+accum-drain and a
1.2GHz-throttled PE in lockstep):
  * scores run j-major into a [P,2,512] two-bank PSUM tile — the stationary
    gt8 pair is loaded 4x per q-tile instead of 8x, and the two-bank tile
    feeds ONE 1024-wide exp (2 ACT instructions per q-tile instead of 4,
    saving the per-instruction fixed cost).
  * exp carries no accum_out: the 182ns ACTIVATION_READ_ACCUMULATOR drain is
    gone; row sums of the fp8 E run on the otherwise-idle Pool/GpSimd engine.
  * transposes and G chunks use the gp PSUM pool so the sc pool's two
    2-bank buffers stay dedicated to the scores->exp rotation.
  * PSUM budget (8 banks): sc 2x2 + gp 2x1 + w 2.  The colsum therefore
    splits: key-chunks {0,1} accumulate inline (one DoubleRow matmul per
    q-tile pair per chunk, lagged one pair); chunks {2,3} run as a 16-matmul
    sweep after the phase, woven into the next phase's slack, with all eight
    E-pair tiles and r-pair tiles kept live in SBUF.
  * batch 1's DMA/transpose/projection weaves into batch 0's scores phase;
    batch 0's colsum sweep + w-phase weaves into batch 1's scores phase.

Sharding: pure data parallelism over batch — 2 batch elements per core on 8
NeuronCores, weights replicated, no collectives.  Host concatenates per-core
[2, D] outputs.
"""

import sys

sys.path.insert(0, "/opt/trn_rl_repo")

import numpy as np

import concourse.bass as bass
import concourse.mybir as mybir
import concourse.tile as tile
from concourse import bacc
from concourse.bass_utils import run_bass_kernel_spmd
from concourse.masks import make_identity

B, S, D = 16, 2048, 512
P = 128
N_CORES = 8
B_PER_CORE = B // N_CORES  # 2
SCALE = 1.0 / float(np.sqrt(D))
KM = 16.0  # M pre-scale (exact power of 2)
KR = 128.0  # r pre-scale (exact power of 2)
C_OFF = 2.5  # global exp offset; cancels through softmax normalization

F32 = mybir.dt.float32
BF16 = mybir.dt.bfloat16
F8 = mybir.dt.float8e4
DR = mybir.MatmulPerfMode.DoubleRow

N_ST = S // P  # 16 s-tiles (partition tiles of the sequence dim)
N_DT = D // P  # 4 d-tiles (partition tiles of the feature dim)
NCH = 512  # moving free dim per matmul (one fp32 PSUM bank)
N_SC = S // NCH  # 4 s-chunks of the sequence dim
N_KC = S // NCH  # 4 k-chunks of the key dim
N_PAIR = N_ST // 2  # 8 q-tile pairs


def build_nc():
    nc = bacc.Bacc("TRN2", target_bir_lowering=False, debug=False, num_devices=N_CORES)
    x_ext = nc.dram_tensor(
        "inputs", [B_PER_CORE, S, D], F32, kind="ExternalInput"
    ).ap()
    wq_ext = nc.dram_tensor("W_q", [D, D], F32, kind="ExternalInput").ap()
    wk_ext = nc.dram_tensor("W_k", [D, D], F32, kind="ExternalInput").ap()
    wv_ext = nc.dram_tensor("W_v", [D, D], F32, kind="ExternalInput").ap()
    out_ext = nc.dram_tensor("out", [B_PER_CORE, D], F32, kind="ExternalOutput").ap()

    with tile.TileContext(nc) as tc:
        with (
            tc.tile_pool(name="const", bufs=1) as const_pool,
            tc.tile_pool(name="w", bufs=1) as w_pool,
            tc.tile_pool(name="xnat", bufs=2) as xnat_pool,
            tc.tile_pool(name="xt", bufs=2) as xt_pool,
            tc.tile_pool(name="qkv", bufs=2) as qkv_pool,
            tc.tile_pool(name="e", bufs=10) as e_pool,
            tc.tile_pool(name="soft", bufs=4) as soft_pool,
            tc.tile_pool(name="r2", bufs=10) as r2_pool,
            tc.tile_pool(name="wvec", bufs=2) as wvec_pool,
            tc.tile_pool(name="scps", bufs=2, space="PSUM") as sc_psum,
            tc.tile_pool(name="gpps", bufs=2, space="PSUM") as gp_psum,
            tc.tile_pool(name="wps", bufs=1, space="PSUM") as w_psum,
        ):
            one_t = const_pool.tile([1, 1], BF16)
            nc.gpsimd.memset(one_t[:], 1.0)
            ident_f = const_pool.tile([P, P], F32)
            make_identity(nc, ident_f[:])
            ident = const_pool.tile([P, P], BF16)
            nc.vector.tensor_copy(ident[:], ident_f[:])
            negc_t = const_pool.tile([P, 1], F32)
            nc.gpsimd.memset(negc_t[:], -C_OFF)

            # x arrives via SWDGE cast-DMA (f32 -> bf16) into natural-layout
            # staging tiles; the transpose to xT happens on the PE as a
            # REGULAR identity matmul (out = x_tile.T @ I), cast to fp8 on the
            # PSUM->SBUF copy.
            def dma_x_chunk(b, sc, xnat_s):
                nc.gpsimd.dma_start(
                    out=xnat_s[:, sc * 4 : (sc + 1) * 4, :],
                    in_=x_ext[b, sc * NCH : (sc + 1) * NCH, :].rearrange(
                        "(t p) d -> p t d", p=P
                    ),
                )

            w_tiles = {}

            def dma_w(name, ext):
                w_s = w_pool.tile([P, N_DT, D], BF16, tag=name)
                nc.gpsimd.dma_start(
                    out=w_s[:], in_=ext.rearrange("(t p) e -> p t e", p=P)
                )
                w_tiles[name] = w_s

            # Batch 0's x chunks and the weight loads share the SWDGE queue;
            # order so each lands just before the PE needs it.
            xnat0_s = xnat_pool.tile([P, N_ST, D], BF16, tag="xnat")
            x0_loaded = [False] * N_SC
            # s-tile 0 rides the parallel HWDGE queue as f32 (lands ~3us
            # before the SWDGE cast chain's first byte); its transposes run
            # as fp32 identity matmuls and a DVE downcast provides the bf16
            # natural-layout copy the final matvec needs.
            xf0 = xnat_pool.tile([P, D], F32, tag="xf0")
            nc.sync.dma_start(out=xf0[:], in_=x_ext[0, 0:P, :])
            nc.vector.tensor_copy(xnat0_s[:, 0, :], xf0[:])
            nc.gpsimd.dma_start(
                out=xnat0_s[:, 1:4, :],
                in_=x_ext[0, P:NCH, :].rearrange("(t p) d -> p t d", p=P),
            )
            x0_loaded[0] = True
            dma_w("wk", wk_ext)
            dma_w("wq", wq_ext)
            dma_x_chunk(0, 1, xnat0_s)
            x0_loaded[1] = True
            dma_x_chunk(0, 2, xnat0_s)
            x0_loaded[2] = True
            dma_x_chunk(0, 3, xnat0_s)
            x0_loaded[3] = True
            dma_w("wv", wv_ext)
            wk_s, wq_s, wv_s = w_tiles["wk"], w_tiles["wq"], w_tiles["wv"]

            # One-time prework: scores = (X Wq)(X Wk)^T = X M X^T with
            # M = Wq Wk^T [D, D].  m8 stores 16*M in fp8 (raw M entries
            # ~N(0, 1/512) would land in fp8-subnormal range); wqT is scaled
            # by 16 at its PSUM->SBUF copy (ACT, idle during prework) so the
            # M matmul's PSUM result is already 16*M.
            wqT_s = w_pool.tile([P, N_DT, D], BF16, tag="wqT")
            wkT_s = w_pool.tile([P, N_DT, D], BF16, tag="wkT")
            m8_s = w_pool.tile([P, N_DT, D], F8, tag="m8")

            def m_prework_thunks():
                thunks = []

                def make_wtrans_unit(src_w, dst, t_e, scale):
                    def th():
                        tp = gp_psum.tile([P, N_DT * P], F32, tag="gp")
                        for t_a in range(N_DT):
                            nc.tensor.matmul(
                                tp[:, t_a * P : (t_a + 1) * P],
                                src_w[:, t_a, t_e * P : (t_e + 1) * P],
                                ident[:],
                                start=True,
                                stop=True,
                                skip_group_check=True,
                            )
                        # bf16 ACT copies keep the prework off DVE, which is
                        # saturated by the batch-0 transpose casts during fill
                        if scale is None:
                            nc.scalar.copy(dst[:, t_e, :], tp[:])
                        else:
                            nc.scalar.mul(dst[:, t_e, :], tp[:], scale)

                    return th

                def make_m_group(t_a):
                    def th():
                        mp = gp_psum.tile([P, NCH], F32, tag="gp")
                        for t_e in range(N_DT):
                            nc.tensor.matmul(
                                mp[:],
                                wqT_s[:, t_e, t_a * P : (t_a + 1) * P],
                                wkT_s[:, t_e, :],
                                start=(t_e == 0),
                                stop=(t_e == N_DT - 1),
                            )
                        nc.vector.tensor_copy(m8_s[:, t_a, :], mp[:])

                    return th

                for t_e in range(N_DT):
                    thunks.append(make_wtrans_unit(wk_s, wkT_s, t_e, None))
                for t_e in range(N_DT):
                    thunks.append(make_wtrans_unit(wq_s, wqT_s, t_e, KM))
                for t_a in range(N_DT):
                    thunks.append(make_m_group(t_a))
                return thunks

            # ---------- thunk builders (emission deferred for interleaving) --

            def proj_thunks(b, xnat_s, loaded):
                """Transpose + G = X M projection thunks for batch b.  Both
                run through the gp PSUM pool (1-bank tiles); xt8/gt8 are fp8.
                All PSUM->SBUF casts are DVE (GPSIMD cannot access PSUM)."""
                xt8_s = xt_pool.tile([P, N_DT, S], F8, tag="xt")
                gt8_s = qkv_pool.tile([P, N_DT, S], F8, tag="gt")

                def make_dma(sc):
                    def th():
                        dma_x_chunk(b, sc, xnat_s)

                    return th

                dma_th = [
                    None if loaded[sc] else make_dma(sc) for sc in range(N_SC)
                ]

                def make_trans_unit(sc, t_i):
                    def th():
                        st = sc * 4 + t_i
                        tp = gp_psum.tile([P, N_DT * P], F32, tag="gp")
                        for dt_i in range(N_DT):
                            nc.tensor.matmul(
                                tp[:, dt_i * P : (dt_i + 1) * P],
                                xnat_s[:, st, dt_i * P : (dt_i + 1) * P],
                                ident[:],
                                start=True,
                                stop=True,
                                skip_group_check=True,
                            )
                        nc.vector.tensor_copy(
                            xt8_s[:, :, st * P : (st + 1) * P],
                            tp[:].rearrange("p (t c) -> p t c", t=N_DT),
                        )

                    return th

                trans_th = [
                    [make_trans_unit(sc, t_i) for t_i in range(4)]
                    for sc in range(N_SC)
                ]

                def make_g(sc, ct):
                    def th():
                        mp = gp_psum.tile([P, NCH], F32, tag="gp")
                        for j in range(2):
                            nc.tensor.matmul(
                                mp[:],
                                m8_s[:, 2 * j : 2 * j + 2, ct * P : (ct + 1) * P],
                                xt8_s[:, 2 * j : 2 * j + 2, sc * NCH : (sc + 1) * NCH],
                                start=(j == 0),
                                stop=(j == 1),
                                perf_mode=DR,
                            )
                        nc.vector.tensor_copy(
                            gt8_s[:, ct, sc * NCH : (sc + 1) * NCH], mp[:]
                        )

                    return th

                kq_th = [
                    [make_g(sc, ct) for ct in range(N_DT)]
                    for sc in range(N_SC)
                ]
                return (gt8_s, xt8_s), dma_th, trans_th, kq_th

            def flat_proj_thunks(dma_th, trans_th, kq_th):
                """Dependency-ordered flat list of one batch's proj thunks,
                for weaving into another batch's scores phase."""
                out = []
                if dma_th[0] is not None:
                    out.append(dma_th[0])
                for th in trans_th[0]:
                    out.append(th)
                for sc in range(N_SC):
                    for j in (sc + 1, sc + 2):
                        if j < N_SC and dma_th[j] is not None:
                            out.append(dma_th[j])
                            dma_th[j] = None
                    nxt = trans_th[sc + 1] if sc + 1 < N_SC else []
                    for g_i, g in enumerate(kq_th[sc]):
                        out.append(g)
                        if g_i < len(nxt):
                            out.append(nxt[g_i])
                return out

            def emit_ltp(dma_th, trans_th, kq_th, extra=None):
                """Emit the transpose/projection stream densely (batch 0's
                unhidden proj phase)."""
                extra = list(extra) if extra else []
                ei = 0
                if dma_th[0] is not None:
                    dma_th[0]()
                    dma_th[0] = None
                for th in trans_th[0]:
                    th()
                for sc in range(N_SC):
                    for j in (sc + 1, sc + 2):
                        if j < N_SC and dma_th[j] is not None:
                            dma_th[j]()
                            dma_th[j] = None
                    nxt = trans_th[sc + 1] if sc + 1 < N_SC else []
                    groups = list(kq_th[sc])
                    ti = 0
                    for g_i, g in enumerate(groups):
                        g()
                        while ti < len(nxt) and ti * len(groups) < (g_i + 1) * len(nxt):
                            nxt[ti]()
                            ti += 1
                        if ei < len(extra):
                            extra[ei]()
                            ei += 1
                    while ti < len(nxt):
                        nxt[ti]()
                        ti += 1
                while ei < len(extra):
                    extra[ei]()
                    ei += 1

            def emit_scores_qt(gt8_s, xt8_s, qt, e2_t, r2_t):
                """One q-tile: two half-row passes, each j-major into a
                [P,2,512] two-bank PSUM tile feeding a single 1024-wide exp
                (fp8 out, offset -C_OFF).  Row sums ride each exp's
                accum_out; the r pipeline stays on DVE."""
                par = qt % 2
                rsum = soft_pool.tile([P, 2], F32, tag="rsum")
                for h in range(2):
                    sp = sc_psum.tile([P, 2, NCH], F32, tag="sc")
                    for j in range(2):
                        for i in range(2):
                            kc = 2 * h + i
                            nc.tensor.matmul(
                                sp[:, i, :],
                                gt8_s[:, 2 * j : 2 * j + 2, qt * P : (qt + 1) * P],
                                xt8_s[:, 2 * j : 2 * j + 2, kc * NCH : (kc + 1) * NCH],
                                start=(j == 0),
                                stop=(j == 1),
                                perf_mode=DR,
                            )
                    eseg = e2_t[:, par, h * 2 * NCH : (h + 1) * 2 * NCH]
                    nc.scalar.activation(
                        eseg,
                        sp[:].rearrange("p a b -> p (a b)"),
                        mybir.ActivationFunctionType.Exp,
                        scale=SCALE / KM,
                        bias=negc_t[:],
                        accum_out=rsum[:, h : h + 1],
                    )
                rtot = soft_pool.tile([P, 1], F32, tag="rtot")
                nc.vector.reduce_sum(rtot[:], rsum[:], axis=mybir.AxisListType.X)
                rtot_s = soft_pool.tile([P, 1], F32, tag="rtots")
                nc.vector.tensor_scalar_mul(rtot_s[:], rtot[:], 1.0 / KR)
                rrec = soft_pool.tile([P, 1], F32, tag="rrec")
                nc.vector.reciprocal(rrec[:], rtot_s[:])  # = KR / rowsum
                nc.vector.tensor_copy(
                    r2_t[:, par, :], rrec[:, 0:1].broadcast_to([P, P])
                )

            def emit_colsum_pair(w_ps, e2_t, r2_t, pair, kcs, w_off):
                """w_ps[:, kc-w_off, :] += bcast(r_pair)^T @ E_pair via ONE
                fp8 DoubleRow matmul per key chunk (contracts both q-subtiles
                at once; every PSUM row ends up the weighted column sum)."""
                for kc in kcs:
                    nc.tensor.matmul(
                        w_ps[:, kc - w_off, :],
                        r2_t[:, 0:2, :],
                        e2_t[:, 0:2, kc * NCH : (kc + 1) * NCH],
                        start=(pair == 0),
                        stop=(pair == N_PAIR - 1),
                        perf_mode=DR,
                        skip_group_check=True,
                    )

            def phase_scores(b, gt8_s, xt8_s, per_qt_extra=None):
                """16 q-tiles of scores+exp.  Colsum for key chunks {0,1}
                accumulates inline (lagged one pair); chunks {2,3} are
                returned as pair thunks for the caller to weave into the next
                phase.  All E/r pair tiles stay live for the deferred sweep.
                The w tile is allocated lazily at the first inline colsum so
                thunks woven into early extras (a previous batch's w-copies)
                are emitted before the bufs=1 pool reassigns the banks."""
                w_holder = {}

                def get_wps():
                    if "a" not in w_holder:
                        w_ps_a = w_psum.tile([P, 2, NCH], F32, tag="w")
                        w_holder["a"] = w_ps_a
                    return w_holder["a"]

                pairs = []  # (e2_t, r2_t) per pair, kept live for the sweep
                pending = []  # pair indices awaiting inline colsum (lag 2)
                e2_t = r2_t = None
                for qt in range(N_ST):
                    if qt % 2 == 0:
                        e2_t = e_pool.tile([P, 2, S], F8, tag="e2")
                        r2_t = r2_pool.tile([P, 2, P], F8, tag="r2")
                    emit_scores_qt(gt8_s, xt8_s, qt, e2_t, r2_t)
                    if qt % 2 == 1:
                        pairs.append((e2_t, r2_t))
                        pending.append(qt // 2)
                        if len(pending) > 2:
                            p = pending.pop(0)
                            emit_colsum_pair(
                                get_wps(), pairs[p][0], pairs[p][1], p, (0, 1), 0
                            )
                    if per_qt_extra is not None and qt < len(per_qt_extra):
                        for th in per_qt_extra[qt]:
                            th()
                for p in pending:
                    emit_colsum_pair(get_wps(), pairs[p][0], pairs[p][1], p, (0, 1), 0)

                sweep_holder = {}

                def make_sweep_pair(pair):
                    def th():
                        if "b" not in sweep_holder:
                            w_ps_b = w_psum.tile([P, 2, NCH], F32, tag="w")
                            sweep_holder["b"] = w_ps_b
                        e2_p, r2_p = pairs[pair]
                        emit_colsum_pair(
                            sweep_holder["b"], e2_p, r2_p, pair, (2, 3), 2
                        )

                    return th

                sweep_th = [make_sweep_pair(p) for p in range(N_PAIR)]
                return w_holder["a"], sweep_holder, sweep_th

            def final_thunks(b, w_ps_a, sweep_holder, sweep_th, xnat_s):
                """Colsum sweep for key chunks {2,3} + w-phase, as a flat
                thunk list the caller weaves into the next scores phase.
                out = (w @ X) @ W_v; w carries the KR=128 pre-scale, folded
                into the final o_sb copy.  y_ps and the row->column
                transposes live in the gp PSUM pool."""
                w_sb = wvec_pool.tile([1, S], BF16, tag="wsb")
                y_ps = gp_psum.tile([P, NCH], F32, tag="gp")
                wt_pads = {}
                yt_pads = {}
                thunks = []

                def make_wcopy(kc):
                    def th():
                        src = w_ps_a if kc < 2 else sweep_holder["b"]
                        nc.vector.tensor_copy(
                            w_sb[:, kc * NCH : (kc + 1) * NCH],
                            src[0:1, kc % 2, :],
                        )

                    return th

                def row_to_bcast_cols(src_row, pads, key, tag):
                    """[1,128] SBUF row chunk -> K=1 matmul -> [128,1] PSUM
                    column -> DVE broadcast to a [128,128] stationary tile."""
                    tp = gp_psum.tile([P, 1], F32, tag="gp")
                    nc.tensor.matmul(
                        tp[:], src_row, one_t[0:1, 0:1], start=True, stop=True
                    )
                    pad = wvec_pool.tile([P, P], BF16, tag=tag)
                    nc.vector.tensor_copy(pad[:], tp[:, 0:1].broadcast_to([P, P]))
                    pads[key] = pad

                def make_wtrans(kt):
                    def th():
                        row_to_bcast_cols(
                            w_sb[0:1, kt * P : (kt + 1) * P],
                            wt_pads, kt, f"wtp{kt % 4}",
                        )

                    return th

                def make_ymm(st):
                    def th():
                        nc.tensor.matmul(
                            y_ps[:],
                            wt_pads[st][:],
                            xnat_s[:, st, :],
                            start=(st == 0),
                            stop=(st == N_ST - 1),
                            skip_group_check=True,
                        )

                    return th

                def epilogue_th():
                    # y [1, D] -> o = y @ W_v  (4 K=1 transposes + 4 matmuls)
                    y_sb = wvec_pool.tile([1, NCH], BF16, tag="ysb")
                    nc.vector.tensor_copy(y_sb[:], y_ps[0:1, :])
                    o_ps = gp_psum.tile([P, NCH], F32, tag="gp")
                    for c in range(N_DT):
                        row_to_bcast_cols(
                            y_sb[0:1, c * P : (c + 1) * P], yt_pads, c, f"ytp{c}"
                        )
                    for c in range(N_DT):
                        nc.tensor.matmul(
                            o_ps[:],
                            yt_pads[c][:],
                            wv_s[:, c, :],
                            start=(c == 0),
                            stop=(c == N_DT - 1),
                            skip_group_check=True,
                        )
                    o_sb = wvec_pool.tile([1, NCH], F32, tag="osb")
                    nc.vector.tensor_scalar_mul(o_sb[:], o_ps[0:1, :], 1.0 / KR)
                    nc.sync.dma_start(out=out_ext[b : b + 1, :], in_=o_sb[:])

                # wcopies of the inline half first (frees w_ps_a for the next
                # batch), then the {2,3} sweep, its wcopies, then the w-phase.
                thunks.append(make_wcopy(0))
                thunks.append(make_wcopy(1))
                thunks.extend(sweep_th)
                thunks.append(make_wcopy(2))
                thunks.append(make_wcopy(3))
                for kt in range(N_ST):
                    thunks.append(make_wtrans(kt))
                    if kt >= 3:
                        thunks.append(make_ymm(kt - 3))
                for st in range(N_ST - 3, N_ST):
                    thunks.append(make_ymm(st))
                thunks.append(epilogue_th)
                return thunks

            def spread(thunks, n_slots):
                """Distribute a flat thunk list over n_slots per-qt extras,
                preserving order."""
                slots = [[] for _ in range(n_slots)]
                k = len(thunks)
                for i, th in enumerate(thunks):
                    slots[min(i * n_slots // k, n_slots - 1)].append(th)
                return slots

            # ------------------------- emission ------------------------------

            # batch 0: M prework + transposes woven into the G projection.
            # Only G s-chunk 0 is emitted in the fill phase (q-tiles 0-3's
            # stationary slice); chunks 1-3 weave into the scores phase's
            # early slack, shortening the DVE-paced fill by ~8us.
            h0, dma0, trans0, kq0 = proj_thunks(0, xnat0_s, x0_loaded)
            g0, xt0 = h0
            if dma0[0] is not None:
                dma0[0]()
                dma0[0] = None

            def first_tile_trans_f32():
                tp = gp_psum.tile([P, N_DT * P], F32, tag="gp")
                for dt_i in range(N_DT):
                    nc.tensor.matmul(
                        tp[:, dt_i * P : (dt_i + 1) * P],
                        xf0[:, dt_i * P : (dt_i + 1) * P],
                        ident_f[:],
                        start=True,
                        stop=True,
                        skip_group_check=True,
                    )
                nc.vector.tensor_copy(
                    xt0[:, :, 0:P],
                    tp[:].rearrange("p (t c) -> p t c", t=N_DT),
                )

            first_tile_trans_f32()
            for th in trans0[0][1:]:
                th()
            for th in m_prework_thunks():
                th()
            trans0 = [[], *trans0[1:]]
            emit_ltp(dma0, trans0, [kq0[0], [], [], []])

            # batch 1 proj thunks, woven into batch 0's ACT-bound scores phase.
            # The first two x chunks DMA up front (SWDGE queue is free by
            # now) so the woven transpose matmuls never park the in-order PE
            # queue on a DMA semaphore.
            xnat1_s = xnat_pool.tile([P, N_ST, D], BF16, tag="xnat")
            h1, dma1, trans1, kq1 = proj_thunks(1, xnat1_s, [False] * N_SC)
            g1, xt1 = h1
            dma1[0]()
            dma1[0] = None
            dma1[1]()
            dma1[1] = None
            proj1_flat = flat_proj_thunks(dma1, trans1, kq1)
            slots0 = spread(proj1_flat, N_ST)
            # batch 0's deferred G chunks go 2-per-slot at the front of the
            # early slots: chunk sc lands a full pair of q-tiles before the
            # first q-tile (4*sc) whose stationary slice needs it.
            g0_rest = [th for sc in (1, 2, 3) for th in kq0[sc]]
            for i, th in enumerate(g0_rest):
                slots0[i // 2].insert(i % 2, th)
            wa0, swh0, swth0 = phase_scores(0, g0, xt0, per_qt_extra=slots0)

            # batch 1 scores with batch 0's colsum sweep + w-phase woven in
            fin0 = final_thunks(0, wa0, swh0, swth0, xnat0_s)
            wa1, swh1, swth1 = phase_scores(
                1, g1, xt1, per_qt_extra=spread(fin0, N_ST)
            )

            for th in final_thunks(1, wa1, swh1, swth1, xnat1_s):
                th()

    nc.compile()
    return nc


_NC_CACHE = None


def _get_nc():
    global _NC_CACHE
    if _NC_CACHE is None:
        _NC_CACHE = build_nc()
    return _NC_CACHE


def make_in_maps(inputs, W_q, W_k, W_v):
    inputs = np.ascontiguousarray(np.asarray(inputs, dtype=np.float32))
    W_q = np.ascontiguousarray(np.asarray(W_q, dtype=np.float32))
    W_k = np.ascontiguousarray(np.asarray(W_k, dtype=np.float32))
    W_v = np.ascontiguousarray(np.asarray(W_v, dtype=np.float32))
    return [
        {
            "inputs": inputs[i * B_PER_CORE : (i + 1) * B_PER_CORE],
            "W_q": W_q,
            "W_k": W_k,
            "W_v": W_v,
        }
        for i in range(N_CORES)
    ]


def kernel(**inputs) -> np.ndarray:
    nc = _get_nc()
    in_maps = make_in_maps(
        inputs["inputs"], inputs["W_q"], inputs["W_k"], inputs["W_v"]
    )
    res = run_bass_kernel_spmd(nc, in_maps, core_ids=list(range(N_CORES)))
    return np.concatenate(
        [res.results[i]["out"] for i in range(N_CORES)], axis=0
    ).astype(np.float32)


# revision 22
# speedup vs baseline: 1.2387x; 1.0176x over previous
"""Trainium2 Bass kernel for batched single-head attention with seq-sum pooling.

Reference computation (B=16, S=2048, D=512, fp32):
    q = x @ W_q ; k = x @ W_k ; v = x @ W_v          per batch  [S, D]
    scores = q @ k.T / sqrt(D)                        [S, S]
    attn = softmax(scores, axis=-1)
    out_b = sum_s (attn @ v)[s, :]                    [D]

Algebraic restructures:
1. The final sum over query positions commutes through both trailing
   matmuls: out_b = ((r^T E) @ x) @ W_v with E = exp(scores/sqrt(D)) and
   r[q] = 1/rowsum_q(E) — removes the [S,S]x[S,D] attention-value matmul
   AND the V projection.
2. scores = x M x^T with M = W_q W_k^T computed once per core — one
   G = x M projection replaces both per-batch Q/K projections.

fp8: the G projection, scores, and softmax column-sum matmuls run with
float8e4 operands in DoubleRow mode (K=256/instruction).  Exact foldings
keep fp8 in range: M stored as 16*M, E = exp(s/sqrt(D) - 2.5) (global
offset cancels through the softmax), r stored as 128*r (folded into the
final output copy).  Measured rel err 7.5e-3 (tolerance 2e-2).

Scores/colsum stationaries are stored PRE-INTERLEAVED for
DoubleRowSwInterleave: the HW weight load then reads contiguously instead
of DoubleRow's strided interleave (which disables fast-weight-load and
makes LDWEIGHTS the ~260ns/matmul bottleneck).  The interleave's column
reversal permutes scores rows (q) within each 128-block — harmless, since
every consumer (exp, row-sum, r broadcast, weighted column-sum) is
q-order-free, and all per-q tensors carry the same permutation.  The G
matmul keeps plain DoubleRow (a reversed G would misalign the scores
contraction).

Schedule (from trace analysis: the PE re-throttles 2.4->1.2GHz on idle
windows, so density is everything): each batch's score rows are computed in
two half-row passes — phase A covers key chunks {0,1}, phase B {2,3} —
which lets phase A start after only 4 transposes + 4 G chunks (~6us) and
hides the entire 8MB x DMA under compute.  Row sums accumulate per-half
via exp's accum_out into persistent per-q-tile tiles; r and the colsums
happen in phase B.  Colsum kc{0,1} accumulates inline (2-pair lag);
kc{2,3} runs as a deferred sweep.  Weave plan: batch0's remaining
transposes/G chunks fill phase A0's slack; batch1's projection fills B0;
batch0's sweep+w-phase fills A1; the only serial tails are ~8us of fill
and batch1's w-phase.  PSUM: sc 2x[P,2,512] + gp 2x1 + w 2 banks = 8.

Sharding: pure data parallelism over batch — 2 batch elements per core on
8 NeuronCores, weights replicated, no collectives.
"""

import sys

sys.path.insert(0, "/opt/trn_rl_repo")

import numpy as np

import concourse.bass as bass
import concourse.mybir as mybir
import concourse.tile as tile
from concourse import bacc
from concourse.bass_utils import run_bass_kernel_spmd
from concourse.masks import make_identity

B, S, D = 16, 2048, 512
P = 128
N_CORES = 8
B_PER_CORE = B // N_CORES  # 2
SCALE = 1.0 / float(np.sqrt(D))
KM = 16.0  # M pre-scale (exact power of 2)
KR = 128.0  # r pre-scale (exact power of 2)
C_OFF = 2.5  # global exp offset; cancels through softmax normalization

F32 = mybir.dt.float32
BF16 = mybir.dt.bfloat16
F8 = mybir.dt.float8e4
DR = mybir.MatmulPerfMode.DoubleRow
SWI = mybir.MatmulPerfMode.DoubleRowSwInterleave
USE_SWI = True  # pre-interleaved scores/colsum stationaries

N_ST = S // P  # 16 s-tiles
N_DT = D // P  # 4 d-tiles
NCH = 512  # moving free dim per matmul (one fp32 PSUM bank)
N_SC = S // NCH  # 4 s-chunks
N_KC = S // NCH  # 4 k-chunks
N_PAIR = N_ST // 2  # 8 q-tile pairs


def build_nc():
    nc = bacc.Bacc("TRN2", target_bir_lowering=False, debug=False, num_devices=N_CORES)
    x_ext = nc.dram_tensor(
        "inputs", [B_PER_CORE, S, D], F32, kind="ExternalInput"
    ).ap()
    wq_ext = nc.dram_tensor("W_q", [D, D], F32, kind="ExternalInput").ap()
    wk_ext = nc.dram_tensor("W_k", [D, D], F32, kind="ExternalInput").ap()
    wv_ext = nc.dram_tensor("W_v", [D, D], F32, kind="ExternalInput").ap()
    out_ext = nc.dram_tensor("out", [B_PER_CORE, D], F32, kind="ExternalOutput").ap()

    with tile.TileContext(nc) as tc:
        with (
            tc.tile_pool(name="const", bufs=1) as const_pool,
            tc.tile_pool(name="w", bufs=1) as w_pool,
            tc.tile_pool(name="xnat", bufs=2) as xnat_pool,
            tc.tile_pool(name="xt", bufs=2) as xt_pool,
            tc.tile_pool(name="qkv", bufs=2) as qkv_pool,
            tc.tile_pool(name="e", bufs=17) as e_pool,
            tc.tile_pool(name="soft", bufs=4) as soft_pool,
            tc.tile_pool(name="rs", bufs=36) as rs_pool,
            tc.tile_pool(name="r2", bufs=18) as r2_pool,
            tc.tile_pool(name="wvec", bufs=2) as wvec_pool,
            tc.tile_pool(name="scps", bufs=2, space="PSUM") as sc_psum,
            tc.tile_pool(name="gpps", bufs=2, space="PSUM") as gp_psum,
            tc.tile_pool(name="wps", bufs=1, space="PSUM") as w_psum,
        ):
            one_t = const_pool.tile([1, 1], BF16)
            nc.gpsimd.memset(one_t[:], 1.0)
            ident_f = const_pool.tile([P, P], F32)
            make_identity(nc, ident_f[:])
            ident = const_pool.tile([P, P], BF16)
            nc.vector.tensor_copy(ident[:], ident_f[:])
            negc_t = const_pool.tile([P, 1], F32)
            nc.gpsimd.memset(negc_t[:], -C_OFF)

            def dma_x_chunk(b, sc, xnat_s):
                nc.gpsimd.dma_start(
                    out=xnat_s[:, sc * 4 : (sc + 1) * 4, :],
                    in_=x_ext[b, sc * NCH : (sc + 1) * NCH, :].rearrange(
                        "(t p) d -> p t d", p=P
                    ),
                )

            w_tiles = {}

            def dma_w(name, ext):
                w_s = w_pool.tile([P, N_DT, D], BF16, tag=name)
                nc.gpsimd.dma_start(
                    out=w_s[:], in_=ext.rearrange("(t p) e -> p t e", p=P)
                )
                w_tiles[name] = w_s

            # SWDGE order: weights first (M prework gates the scores start),
            # then batch 0's x chunks.  s-tile 0 rides HWDGE as f32.
            xnat0_s = xnat_pool.tile([P, N_ST, D], BF16, tag="xnat")
            xf0 = xnat_pool.tile([P, D], F32, tag="xf0")
            nc.sync.dma_start(out=xf0[:], in_=x_ext[0, 0:P, :])
            nc.vector.tensor_copy(xnat0_s[:, 0, :], xf0[:])
            dma_w("wk", wk_ext)
            dma_w("wq", wq_ext)
            nc.gpsimd.dma_start(
                out=xnat0_s[:, 1:4, :],
                in_=x_ext[0, P:NCH, :].rearrange("(t p) d -> p t d", p=P),
            )
            dma_x_chunk(0, 1, xnat0_s)
            dma_x_chunk(0, 2, xnat0_s)
            dma_x_chunk(0, 3, xnat0_s)
            dma_w("wv", wv_ext)
            x0_loaded = [True] * N_SC
            wk_s, wq_s, wv_s = w_tiles["wk"], w_tiles["wq"], w_tiles["wv"]

            # One-time prework: M = Wq Wk^T, stored as 16*M fp8 (raw entries
            # would be fp8-subnormal; the exp scale divides the 16 out).
            # wqT scaled by 16 at its ACT copy; wkT copies also on ACT so the
            # fill phase's DVE stays on the x transposes.
            wqT_s = w_pool.tile([P, N_DT, D], BF16, tag="wqT")
            wkT_s = w_pool.tile([P, N_DT, D], BF16, tag="wkT")
            m8_s = w_pool.tile([P, N_DT, D], F8, tag="m8")

            def m_prework_thunks():
                thunks = []

                def make_wtrans_unit(src_w, dst, t_e, scale):
                    def th():
                        tp = gp_psum.tile([P, N_DT * P], F32, tag="gp")
                        for t_a in range(N_DT):
                            nc.tensor.matmul(
                                tp[:, t_a * P : (t_a + 1) * P],
                                src_w[:, t_a, t_e * P : (t_e + 1) * P],
                                ident[:],
                                start=True,
                                stop=True,
                                skip_group_check=True,
                            )
                        if scale is None:
                            nc.scalar.copy(dst[:, t_e, :], tp[:])
                        else:
                            nc.scalar.mul(dst[:, t_e, :], tp[:], scale)

                    return th

                def make_m_group(t_a):
                    def th():
                        mp = gp_psum.tile([P, NCH], F32, tag="gp")
                        for t_e in range(N_DT):
                            nc.tensor.matmul(
                                mp[:],
                                wqT_s[:, t_e, t_a * P : (t_a + 1) * P],
                                wkT_s[:, t_e, :],
                                start=(t_e == 0),
                                stop=(t_e == N_DT - 1),
                            )
                        nc.vector.tensor_copy(m8_s[:, t_a, :], mp[:])

                    return th

                for t_e in range(N_DT):
                    thunks.append(make_wtrans_unit(wk_s, wkT_s, t_e, None))
                for t_e in range(N_DT):
                    thunks.append(make_wtrans_unit(wq_s, wqT_s, t_e, KM))
                for t_a in range(N_DT):
                    thunks.append(make_m_group(t_a))
                return thunks

            # ---------- thunk builders --------------------------------------

            def proj_thunks(b, xnat_s, loaded):
                """Transpose + G = X M projection thunks for batch b.  xt8 is
                [P, dtile, S] fp8.  gt8 layout depends on USE_SWI:
                  - SWI: [P, jpair, qt_block, 2*P] with the two d-subtiles of
                    a jpair interleaved along the last dim (stored UNreversed;
                    the HW's column reversal permutes q within blocks, which
                    every downstream consumer absorbs).
                  - plain DR: [P, dtile, S]."""
                xt8_s = xt_pool.tile([P, N_DT, S], F8, tag="xt")
                if USE_SWI:
                    gt8_s = qkv_pool.tile([P, 2, N_ST, 2 * P], F8, tag="gt")
                else:
                    gt8_s = qkv_pool.tile([P, N_DT, S], F8, tag="gt")

                def make_dma(sc):
                    def th():
                        dma_x_chunk(b, sc, xnat_s)

                    return th

                dma_th = [
                    None if loaded[sc] else make_dma(sc) for sc in range(N_SC)
                ]

                def make_trans_unit(sc, t_i):
                    def th():
                        st = sc * 4 + t_i
                        tp = gp_psum.tile([P, N_DT * P], F32, tag="gp")
                        for dt_i in range(N_DT):
                            nc.tensor.matmul(
                                tp[:, dt_i * P : (dt_i + 1) * P],
                                xnat_s[:, st, dt_i * P : (dt_i + 1) * P],
                                ident[:],
                                start=True,
                                stop=True,
                                skip_group_check=True,
                            )
                        nc.vector.tensor_copy(
                            xt8_s[:, :, st * P : (st + 1) * P],
                            tp[:].rearrange("p (t c) -> p t c", t=N_DT),
                        )

                    return th

                trans_th = [
                    [make_trans_unit(sc, t_i) for t_i in range(4)]
                    for sc in range(N_SC)
                ]

                def make_g(sc, ct):
                    def th():
                        mp = gp_psum.tile([P, NCH], F32, tag="gp")
                        for j in range(2):
                            nc.tensor.matmul(
                                mp[:],
                                m8_s[:, 2 * j : 2 * j + 2, ct * P : (ct + 1) * P],
                                xt8_s[:, 2 * j : 2 * j + 2, sc * NCH : (sc + 1) * NCH],
                                start=(j == 0),
                                stop=(j == 1),
                                perf_mode=DR,
                            )
                        if USE_SWI:
                            nc.vector.tensor_copy(
                                gt8_s[
                                    :,
                                    ct // 2,
                                    sc * 4 : (sc + 1) * 4,
                                    (ct % 2) :: 2,
                                ],
                                mp[:].rearrange("p (b q) -> p b q", b=4),
                            )
                        else:
                            nc.vector.tensor_copy(
                                gt8_s[:, ct, sc * NCH : (sc + 1) * NCH], mp[:]
                            )

                    return th

                kq_th = [
                    [make_g(sc, ct) for ct in range(N_DT)]
                    for sc in range(N_SC)
                ]
                return (gt8_s, xt8_s), dma_th, trans_th, kq_th

            def scores_stationary(gt8_s, j, qt):
                if USE_SWI:
                    return gt8_s[:, j, qt, :]
                return gt8_s[:, 2 * j : 2 * j + 2, qt * P : (qt + 1) * P]

            SC_MODE = SWI if USE_SWI else DR

            def emit_scores_half(gt8_s, xt8_s, qt, h, e2_t, rs_t):
                """One half-row pass for one q-tile: j-major into a [P,2,512]
                two-bank PSUM tile, one 1024-wide exp (fp8 out, offset
                -C_OFF), row-sum via accum_out into rs_t[:, h]."""
                par = qt % 2
                sp = sc_psum.tile([P, 2, NCH], F32, tag="sc")
                for j in range(2):
                    for i in range(2):
                        kc = 2 * h + i
                        nc.tensor.matmul(
                            sp[:, i, :],
                            scores_stationary(gt8_s, j, qt),
                            xt8_s[:, 2 * j : 2 * j + 2, kc * NCH : (kc + 1) * NCH],
                            start=(j == 0),
                            stop=(j == 1),
                            perf_mode=SC_MODE,
                        )
                nc.scalar.activation(
                    e2_t[:, par, h * 2 * NCH : (h + 1) * 2 * NCH],
                    sp[:].rearrange("p a b -> p (a b)"),
                    mybir.ActivationFunctionType.Exp,
                    scale=SCALE / KM,
                    bias=negc_t[:],
                    accum_out=rs_t[:, h : h + 1],
                )

            def emit_r(qt, rs_t, r2_t):
                """r = KR / (rowsumA + rowsumB), broadcast into the qt%2 lane
                of the pair's (interleaved) stationary tile."""
                par = qt % 2
                rtot = soft_pool.tile([P, 1], F32, tag="rtot")
                nc.vector.reduce_sum(rtot[:], rs_t[:], axis=mybir.AxisListType.X)
                rtot_s = soft_pool.tile([P, 1], F32, tag="rtots")
                nc.vector.tensor_scalar_mul(rtot_s[:], rtot[:], 1.0 / KR)
                rrec = soft_pool.tile([P, 1], F32, tag="rrec")
                nc.vector.reciprocal(rrec[:], rtot_s[:])
                if USE_SWI:
                    dst = r2_t[:, par::2]
                else:
                    dst = r2_t[:, par, :]
                nc.vector.tensor_copy(dst, rrec[:, 0:1].broadcast_to([P, P]))

            def colsum_stationary(r2_t):
                if USE_SWI:
                    return r2_t[:, :]
                return r2_t[:, 0:2, :]

            def emit_colsum_pair(w_ps, e2_t, r2_t, pair, kcs, w_off):
                for kc in kcs:
                    nc.tensor.matmul(
                        w_ps[:, kc - w_off, :],
                        colsum_stationary(r2_t),
                        e2_t[:, 0:2, kc * NCH : (kc + 1) * NCH],
                        start=(pair == 0),
                        stop=(pair == N_PAIR - 1),
                        perf_mode=SC_MODE,
                        skip_group_check=True,
                    )

            def phase_A(gt8_s, xt8_s, e2_list, rs_list, extras):
                """Key chunks {0,1} for all 16 q-tiles."""
                for qt in range(N_ST):
                    if qt % 2 == 0:
                        e2_t = e_pool.tile([P, 2, S], F8, tag="e2")
                        e2_list.append(e2_t)
                    rs_t = rs_pool.tile([P, 2], F32, tag="rs")
                    rs_list.append(rs_t)
                    emit_scores_half(gt8_s, xt8_s, qt, 0, e2_list[qt // 2], rs_t)
                    for th in extras[qt]:
                        th()

            def phase_B(gt8_s, xt8_s, e2_list, rs_list, extras):
                """Key chunks {2,3}, r pipeline, inline colsum kc{0,1}
                (2-pair lag, lazy w tile), deferred kc{2,3} sweep thunks."""
                w_holder = {}

                def get_wps():
                    if "a" not in w_holder:
                        w_ps_a = w_psum.tile([P, 2, NCH], F32, tag="w")
                        w_holder["a"] = w_ps_a
                    return w_holder["a"]

                r2_list = []
                pending = []
                for qt in range(N_ST):
                    pair = qt // 2
                    if qt % 2 == 0:
                        if USE_SWI:
                            r2_t = r2_pool.tile([P, 2 * P], F8, tag="r2")
                        else:
                            r2_t = r2_pool.tile([P, 2, P], F8, tag="r2")
                        r2_list.append(r2_t)
                    emit_scores_half(gt8_s, xt8_s, qt, 1, e2_list[pair], rs_list[qt])
                    emit_r(qt, rs_list[qt], r2_list[pair])
                    if qt % 2 == 1:
                        pending.append(pair)
                        if len(pending) > 2:
                            p = pending.pop(0)
                            emit_colsum_pair(
                                get_wps(), e2_list[p], r2_list[p], p, (0, 1), 0
                            )
                    for th in extras[qt]:
                        th()
                for p in pending:
                    emit_colsum_pair(get_wps(), e2_list[p], r2_list[p], p, (0, 1), 0)

                sweep_holder = {}

                def make_sweep_pair(pair):
                    def th():
                        if "b" not in sweep_holder:
                            w_ps_b = w_psum.tile([P, 2, NCH], F32, tag="w")
                            sweep_holder["b"] = w_ps_b
                        emit_colsum_pair(
                            sweep_holder["b"],
                            e2_list[pair],
                            r2_list[pair],
                            pair,
                            (2, 3),
                            2,
                        )

                    return th

                sweep_th = [make_sweep_pair(p) for p in range(N_PAIR)]
                return w_holder["a"], sweep_holder, sweep_th

            def final_thunks(b, w_ps_a, sweep_holder, sweep_th, xnat_s):
                """Colsum sweep kc{2,3} + w-phase: out = (w @ X) @ W_v; the
                KR pre-scale is folded into the final o_sb copy.  y_ps and
                the row->column transposes live in the gp PSUM pool."""
                w_sb = wvec_pool.tile([1, S], BF16, tag="wsb")
                y_ps = gp_psum.tile([P, NCH], F32, tag="gp")
                wt_pads = {}
                yt_pads = {}
                thunks = []

                def make_wcopy(kc):
                    def th():
                        src = w_ps_a if kc < 2 else sweep_holder["b"]
                        nc.vector.tensor_copy(
                            w_sb[:, kc * NCH : (kc + 1) * NCH],
                            src[0:1, kc % 2, :],
                        )

                    return th

                def row_to_bcast_cols(src_row, pads, key, tag):
                    tp = gp_psum.tile([P, 1], F32, tag="gp")
                    nc.tensor.matmul(
                        tp[:], src_row, one_t[0:1, 0:1], start=True, stop=True
                    )
                    pad = wvec_pool.tile([P, P], BF16, tag=tag)
                    nc.vector.tensor_copy(pad[:], tp[:, 0:1].broadcast_to([P, P]))
                    pads[key] = pad

                def make_wtrans(kt):
                    def th():
                        row_to_bcast_cols(
                            w_sb[0:1, kt * P : (kt + 1) * P],
                            wt_pads, kt, f"wtp{kt % 4}",
                        )

                    return th

                def make_ymm(st):
                    def th():
                        nc.tensor.matmul(
                            y_ps[:],
                            wt_pads[st][:],
                            xnat_s[:, st, :],
                            start=(st == 0),
                            stop=(st == N_ST - 1),
                            skip_group_check=True,
                        )

                    return th

                def epilogue_th():
                    y_sb = wvec_pool.tile([1, NCH], BF16, tag="ysb")
                    nc.vector.tensor_copy(y_sb[:], y_ps[0:1, :])
                    o_ps = gp_psum.tile([P, NCH], F32, tag="gp")
                    for c in range(N_DT):
                        row_to_bcast_cols(
                            y_sb[0:1, c * P : (c + 1) * P], yt_pads, c, f"ytp{c}"
                        )
                    for c in range(N_DT):
                        nc.tensor.matmul(
                            o_ps[:],
                            yt_pads[c][:],
                            wv_s[:, c, :],
                            start=(c == 0),
                            stop=(c == N_DT - 1),
                            skip_group_check=True,
                        )
                    o_sb = wvec_pool.tile([1, NCH], F32, tag="osb")
                    nc.vector.tensor_scalar_mul(o_sb[:], o_ps[0:1, :], 1.0 / KR)
                    nc.sync.dma_start(out=out_ext[b : b + 1, :], in_=o_sb[:])

                thunks.append(make_wcopy(0))
                thunks.append(make_wcopy(1))
                thunks.extend(sweep_th)
                thunks.append(make_wcopy(2))
                thunks.append(make_wcopy(3))
                for kt in range(N_ST):
                    thunks.append(make_wtrans(kt))
                    if kt >= 3:
                        thunks.append(make_ymm(kt - 3))
                for st in range(N_ST - 3, N_ST):
                    thunks.append(make_ymm(st))
                thunks.append(epilogue_th)
                return thunks

            def spread(thunks, n_slots):
                slots = [[] for _ in range(n_slots)]
                k = len(thunks)
                for i, th in enumerate(thunks):
                    slots[min(i * n_slots // k, n_slots - 1)].append(th)
                return slots

            # ------------------------- emission ------------------------------

            # FILL: s-tile 0 transpose (f32 path), s-tiles 1-3 transposes,
            # M prework, G s-chunk 0 — just enough for phase A0's q-tile 0.
            h0, dma0, trans0, kq0 = proj_thunks(0, xnat0_s, x0_loaded)
            g0, xt0 = h0

            def first_tile_trans_f32():
                tp = gp_psum.tile([P, N_DT * P], F32, tag="gp")
                for dt_i in range(N_DT):
                    nc.tensor.matmul(
                        tp[:, dt_i * P : (dt_i + 1) * P],
                        xf0[:, dt_i * P : (dt_i + 1) * P],
                        ident_f[:],
                        start=True,
                        stop=True,
                        skip_group_check=True,
                    )
                nc.vector.tensor_copy(
                    xt0[:, :, 0:P],
                    tp[:].rearrange("p (t c) -> p t c", t=N_DT),
                )

            first_tile_trans_f32()
            pre_th = m_prework_thunks()
            # interleave prework (ACT copies) with c0 transposes (DVE copies)
            fill_stream = []
            fill_stream.extend(pre_th[:4])  # wkT units
            fill_stream.extend(trans0[0][1:])  # s-tiles 1-3
            fill_stream.extend(pre_th[4:8])  # wqT units
            fill_stream.extend(trans0[1])  # s-tiles 4-7 (phase A needs kc1)
            fill_stream.extend(pre_th[8:])  # M groups
            fill_stream.extend(kq0[0])  # G s-chunk 0
            for th in fill_stream:
                th()

            # batch 1 proj thunks (woven into B0)
            xnat1_s = xnat_pool.tile([P, N_ST, D], BF16, tag="xnat")
            h1, dma1, trans1, kq1 = proj_thunks(1, xnat1_s, [False] * N_SC)
            g1, xt1 = h1

            # --- phase A0: extras = batch0's remaining transposes/G + batch1
            # DMA kickoff.  Deadlines: G sc1 before qt4, sc2 before qt8,
            # sc3 before qt12; trans c2/c3 before phase B0.
            slots_a0 = [[] for _ in range(N_ST)]
            slots_a0[0] = [kq0[1][0], kq0[1][1]]
            slots_a0[1] = [kq0[1][2], kq0[1][3]]
            slots_a0[2] = [trans0[2][0], trans0[2][1]]
            slots_a0[3] = [trans0[2][2], trans0[2][3]]
            slots_a0[4] = [kq0[2][0], kq0[2][1]]
            slots_a0[5] = [kq0[2][2], kq0[2][3]]
            slots_a0[6] = [trans0[3][0], trans0[3][1]]
            slots_a0[7] = [trans0[3][2], trans0[3][3]]
            slots_a0[8] = [kq0[3][0], kq0[3][1]]
            slots_a0[9] = [kq0[3][2], kq0[3][3]]
            slots_a0[10] = [dma1[0]]
            slots_a0[11] = [dma1[1]]
            dma1[0] = dma1[1] = None

            e2_0, rs_0 = [], []
            phase_A(g0, xt0, e2_0, rs_0, slots_a0)

            # --- phase B0: extras = batch1's projection (DMA c2/c3 early).
            proj1_flat = [dma1[2], dma1[3]]
            dma1[2] = dma1[3] = None
            for sc in range(N_SC):
                proj1_flat.extend(trans1[sc])
                proj1_flat.extend(kq1[sc])
            wa0, swh0, swth0 = phase_B(
                g0, xt0, e2_0, rs_0, spread(proj1_flat, N_ST)
            )

            # --- phase A1: extras = batch0's colsum sweep + w-phase.
            fin0 = final_thunks(0, wa0, swh0, swth0, xnat0_s)
            e2_1, rs_1 = [], []
            phase_A(g1, xt1, e2_1, rs_1, spread(fin0, N_ST))

            # --- phase B1: no extras.
            wa1, swh1, swth1 = phase_B(
                g1, xt1, e2_1, rs_1, [[] for _ in range(N_ST)]
            )

            # --- tail: batch1's sweep + w-phase.
            for th in final_thunks(1, wa1, swh1, swth1, xnat1_s):
                th()

    nc.compile()
    return nc


_NC_CACHE = None


def _get_nc():
    global _NC_CACHE
    if _NC_CACHE is None:
        _NC_CACHE = build_nc()
    return _NC_CACHE


def make_in_maps(inputs, W_q, W_k, W_v):
    inputs = np.ascontiguousarray(np.asarray(inputs, dtype=np.float32))
    W_q = np.ascontiguousarray(np.asarray(W_q, dtype=np.float32))
    W_k = np.ascontiguousarray(np.asarray(W_k, dtype=np.float32))
    W_v = np.ascontiguousarray(np.asarray(W_v, dtype=np.float32))
    return [
        {
            "inputs": inputs[i * B_PER_CORE : (i + 1) * B_PER_CORE],
            "W_q": W_q,
            "W_k": W_k,
            "W_v": W_v,
        }
        for i in range(N_CORES)
    ]


def kernel(**inputs) -> np.ndarray:
    nc = _get_nc()
    in_maps = make_in_maps(
        inputs["inputs"], inputs["W_q"], inputs["W_k"], inputs["W_v"]
    )
    res = run_bass_kernel_spmd(nc, in_maps, core_ids=list(range(N_CORES)))
    return np.concatenate(
        [res.results[i]["out"] for i in range(N_CORES)], axis=0
    ).astype(np.float32)


# revision 23
# speedup vs baseline: 1.4744x; 1.1903x over previous
"""Trainium2 Bass kernel for batched single-head attention with seq-sum pooling.

Reference computation (B=16, S=2048, D=512, fp32):
    q = x @ W_q ; k = x @ W_k ; v = x @ W_v          per batch  [S, D]
    scores = q @ k.T / sqrt(D)                        [S, S]
    attn = softmax(scores, axis=-1)
    out_b = sum_s (attn @ v)[s, :]                    [D]

Algebraic restructures:
1. The final sum over query positions commutes through both trailing
   matmuls: out_b = ((r^T E) @ x) @ W_v with E = exp(scores/sqrt(D)) and
   r[q] = 1/rowsum_q(E) — removes the [S,S]x[S,D] attention-value matmul
   AND the V projection.
2. scores = x M x^T with M = W_q W_k^T computed once per core — one
   G = x M projection replaces both per-batch Q/K projections.

fp8: the G projection, scores, and softmax column-sum matmuls run with
float8e4 operands in DoubleRow mode (K=256/instruction).  Exact foldings
keep fp8 in range: M stored as 16*M, E = exp(s/sqrt(D) - 2.5) (global
offset cancels through the softmax), r stored as 128*r (folded into the
final output copy).  Measured rel err 7.5e-3 (tolerance 2e-2).

Scores/colsum stationaries are stored PRE-INTERLEAVED for
DoubleRowSwInterleave: the HW weight load then reads contiguously instead
of DoubleRow's strided interleave (which disables fast-weight-load and
makes LDWEIGHTS the ~260ns/matmul bottleneck).  The interleave's column
reversal permutes scores rows (q) within each 128-block — harmless, since
every consumer (exp, row-sum, r broadcast, weighted column-sum) is
q-order-free, and all per-q tensors carry the same permutation.  The G
matmul keeps plain DoubleRow (a reversed G would misalign the scores
contraction).

Schedule (from trace analysis: the PE re-throttles 2.4->1.2GHz on idle
windows, so density is everything): each batch's score rows are computed in
two half-row passes — phase A covers key chunks {0,1}, phase B {2,3} —
which lets phase A start after only 4 transposes + 4 G chunks (~6us) and
hides the entire 8MB x DMA under compute.  Row sums accumulate per-half
via exp's accum_out into persistent per-q-tile tiles; r and the colsums
happen in phase B.  Colsum kc{0,1} accumulates inline (2-pair lag);
kc{2,3} runs as a deferred sweep.  Weave plan: batch0's remaining
transposes/G chunks fill phase A0's slack; batch1's projection fills B0;
batch0's sweep+w-phase fills A1; the only serial tails are ~8us of fill
and batch1's w-phase.  PSUM: sc 2x[P,2,512] + gp 2x1 + w 2 banks = 8.

Sharding: pure data parallelism over batch — 2 batch elements per core on
8 NeuronCores, weights replicated, no collectives.
"""

import sys

sys.path.insert(0, "/opt/trn_rl_repo")

import numpy as np

import concourse.bass as bass
import concourse.mybir as mybir
import concourse.tile as tile
from concourse import bacc
from concourse.bass_utils import run_bass_kernel_spmd
from concourse.masks import make_identity

B, S, D = 16, 2048, 512
P = 128
N_CORES = 8
B_PER_CORE = B // N_CORES  # 2
SCALE = 1.0 / float(np.sqrt(D))
KM = 16.0  # M pre-scale (exact power of 2)
KR = 128.0  # r pre-scale (exact power of 2)
C_OFF = 2.5  # global exp offset; cancels through softmax normalization

F32 = mybir.dt.float32
BF16 = mybir.dt.bfloat16
F8 = mybir.dt.float8e4
DR = mybir.MatmulPerfMode.DoubleRow
SWI = mybir.MatmulPerfMode.DoubleRowSwInterleave
USE_SWI = True  # pre-interleaved scores/colsum stationaries

N_ST = S // P  # 16 s-tiles
N_DT = D // P  # 4 d-tiles
NCH = 512  # moving free dim per matmul (one fp32 PSUM bank)
N_SC = S // NCH  # 4 s-chunks
N_KC = S // NCH  # 4 k-chunks
N_PAIR = N_ST // 2  # 8 q-tile pairs


def build_nc():
    nc = bacc.Bacc("TRN2", target_bir_lowering=False, debug=False, num_devices=N_CORES)
    x_ext = nc.dram_tensor(
        "inputs", [B_PER_CORE, S, D], F32, kind="ExternalInput"
    ).ap()
    wq_ext = nc.dram_tensor("W_q", [D, D], F32, kind="ExternalInput").ap()
    wk_ext = nc.dram_tensor("W_k", [D, D], F32, kind="ExternalInput").ap()
    wv_ext = nc.dram_tensor("W_v", [D, D], F32, kind="ExternalInput").ap()
    out_ext = nc.dram_tensor("out", [B_PER_CORE, D], F32, kind="ExternalOutput").ap()

    with tile.TileContext(nc) as tc:
        with (
            tc.tile_pool(name="const", bufs=1) as const_pool,
            tc.tile_pool(name="w", bufs=1) as w_pool,
            tc.tile_pool(name="xnat", bufs=2) as xnat_pool,
            tc.tile_pool(name="xt", bufs=2) as xt_pool,
            tc.tile_pool(name="qkv", bufs=2) as qkv_pool,
            tc.tile_pool(name="e", bufs=17) as e_pool,
            tc.tile_pool(name="soft", bufs=4) as soft_pool,
            tc.tile_pool(name="rs", bufs=36) as rs_pool,
            tc.tile_pool(name="r2", bufs=18) as r2_pool,
            tc.tile_pool(name="wvec", bufs=2) as wvec_pool,
            tc.tile_pool(name="scps", bufs=2, space="PSUM") as sc_psum,
            tc.tile_pool(name="gpps", bufs=2, space="PSUM") as gp_psum,
            tc.tile_pool(name="wps", bufs=1, space="PSUM") as w_psum,
        ):
            one_t = const_pool.tile([1, 1], BF16)
            nc.gpsimd.memset(one_t[:], 1.0)
            ident_f = const_pool.tile([P, P], F32)
            make_identity(nc, ident_f[:])
            ident = const_pool.tile([P, P], BF16)
            nc.vector.tensor_copy(ident[:], ident_f[:])
            negc_t = const_pool.tile([P, 1], F32)
            nc.gpsimd.memset(negc_t[:], -C_OFF)

            # HAM warmup: the PE clock boots throttled to 1.2GHz and only
            # un-throttles to 2.4GHz after ~3.4us of sustained matmul
            # activity (one full busy window of the hardware activity
            # monitor) — and a DoubleRow/fp8 stream SUSTAINS the warm state
            # but was never observed to CREATE it.  Burn ~5us of dense
            # dependency-free bf16 matmuls up front (concurrent with the
            # weight/x DMAs) so every real matmul after runs at 2.4GHz.
            warm_mov = const_pool.tile([P, NCH], BF16)
            nc.gpsimd.memset(warm_mov[:], 0.0)
            warm_ps = sc_psum.tile([P, 2, NCH], F32, tag="sc")
            for i in range(12):
                nc.tensor.matmul(
                    warm_ps[:, i % 2, :],
                    ident[:],
                    warm_mov[:],
                    start=True,
                    stop=True,
                    skip_group_check=True,
                )

            def dma_x_chunk(b, sc, xnat_s):
                nc.gpsimd.dma_start(
                    out=xnat_s[:, sc * 4 : (sc + 1) * 4, :],
                    in_=x_ext[b, sc * NCH : (sc + 1) * NCH, :].rearrange(
                        "(t p) d -> p t d", p=P
                    ),
                )

            w_tiles = {}

            def dma_w(name, ext):
                w_s = w_pool.tile([P, N_DT, D], BF16, tag=name)
                nc.gpsimd.dma_start(
                    out=w_s[:], in_=ext.rearrange("(t p) e -> p t e", p=P)
                )
                w_tiles[name] = w_s

            # SWDGE order: weights first (M prework gates the scores start),
            # then batch 0's x chunks.  s-tile 0 rides HWDGE as f32.
            xnat0_s = xnat_pool.tile([P, N_ST, D], BF16, tag="xnat")
            xf0 = xnat_pool.tile([P, D], F32, tag="xf0")
            nc.sync.dma_start(out=xf0[:], in_=x_ext[0, 0:P, :])
            nc.vector.tensor_copy(xnat0_s[:, 0, :], xf0[:])
            dma_w("wk", wk_ext)
            dma_w("wq", wq_ext)
            nc.gpsimd.dma_start(
                out=xnat0_s[:, 1:4, :],
                in_=x_ext[0, P:NCH, :].rearrange("(t p) d -> p t d", p=P),
            )
            dma_x_chunk(0, 1, xnat0_s)
            dma_x_chunk(0, 2, xnat0_s)
            dma_x_chunk(0, 3, xnat0_s)
            dma_w("wv", wv_ext)
            x0_loaded = [True] * N_SC
            wk_s, wq_s, wv_s = w_tiles["wk"], w_tiles["wq"], w_tiles["wv"]

            # One-time prework: M = Wq Wk^T, stored as 16*M fp8 (raw entries
            # would be fp8-subnormal; the exp scale divides the 16 out).
            # wqT scaled by 16 at its ACT copy; wkT copies also on ACT so the
            # fill phase's DVE stays on the x transposes.
            wqT_s = w_pool.tile([P, N_DT, D], BF16, tag="wqT")
            wkT_s = w_pool.tile([P, N_DT, D], BF16, tag="wkT")
            m8_s = w_pool.tile([P, N_DT, D], F8, tag="m8")

            def m_prework_thunks():
                thunks = []

                def make_wtrans_unit(src_w, dst, t_e, scale):
                    def th():
                        tp = gp_psum.tile([P, N_DT * P], F32, tag="gp")
                        for t_a in range(N_DT):
                            nc.tensor.matmul(
                                tp[:, t_a * P : (t_a + 1) * P],
                                src_w[:, t_a, t_e * P : (t_e + 1) * P],
                                ident[:],
                                start=True,
                                stop=True,
                                skip_group_check=True,
                            )
                        if scale is None:
                            nc.scalar.copy(dst[:, t_e, :], tp[:])
                        else:
                            nc.scalar.mul(dst[:, t_e, :], tp[:], scale)

                    return th

                def make_m_group(t_a):
                    def th():
                        mp = gp_psum.tile([P, NCH], F32, tag="gp")
                        for t_e in range(N_DT):
                            nc.tensor.matmul(
                                mp[:],
                                wqT_s[:, t_e, t_a * P : (t_a + 1) * P],
                                wkT_s[:, t_e, :],
                                start=(t_e == 0),
                                stop=(t_e == N_DT - 1),
                            )
                        nc.vector.tensor_copy(m8_s[:, t_a, :], mp[:])

                    return th

                for t_e in range(N_DT):
                    thunks.append(make_wtrans_unit(wk_s, wkT_s, t_e, None))
                for t_e in range(N_DT):
                    thunks.append(make_wtrans_unit(wq_s, wqT_s, t_e, KM))
                for t_a in range(N_DT):
                    thunks.append(make_m_group(t_a))
                return thunks

            # ---------- thunk builders --------------------------------------

            def proj_thunks(b, xnat_s, loaded):
                """Transpose + G = X M projection thunks for batch b.  xt8 is
                [P, dtile, S] fp8.  gt8 layout depends on USE_SWI:
                  - SWI: [P, jpair, qt_block, 2*P] with the two d-subtiles of
                    a jpair interleaved along the last dim (stored UNreversed;
                    the HW's column reversal permutes q within blocks, which
                    every downstream consumer absorbs).
                  - plain DR: [P, dtile, S]."""
                xt8_s = xt_pool.tile([P, N_DT, S], F8, tag="xt")
                if USE_SWI:
                    gt8_s = qkv_pool.tile([P, 2, N_ST, 2 * P], F8, tag="gt")
                else:
                    gt8_s = qkv_pool.tile([P, N_DT, S], F8, tag="gt")

                def make_dma(sc):
                    def th():
                        dma_x_chunk(b, sc, xnat_s)

                    return th

                dma_th = [
                    None if loaded[sc] else make_dma(sc) for sc in range(N_SC)
                ]

                def make_trans_unit(sc, t_i):
                    def th():
                        st = sc * 4 + t_i
                        tp = gp_psum.tile([P, N_DT * P], F32, tag="gp")
                        for dt_i in range(N_DT):
                            nc.tensor.matmul(
                                tp[:, dt_i * P : (dt_i + 1) * P],
                                xnat_s[:, st, dt_i * P : (dt_i + 1) * P],
                                ident[:],
                                start=True,
                                stop=True,
                                skip_group_check=True,
                            )
                        nc.vector.tensor_copy(
                            xt8_s[:, :, st * P : (st + 1) * P],
                            tp[:].rearrange("p (t c) -> p t c", t=N_DT),
                        )

                    return th

                trans_th = [
                    [make_trans_unit(sc, t_i) for t_i in range(4)]
                    for sc in range(N_SC)
                ]

                def make_g(sc, ct):
                    def th():
                        mp = gp_psum.tile([P, NCH], F32, tag="gp")
                        for j in range(2):
                            nc.tensor.matmul(
                                mp[:],
                                m8_s[:, 2 * j : 2 * j + 2, ct * P : (ct + 1) * P],
                                xt8_s[:, 2 * j : 2 * j + 2, sc * NCH : (sc + 1) * NCH],
                                start=(j == 0),
                                stop=(j == 1),
                                perf_mode=DR,
                            )
                        if USE_SWI:
                            nc.vector.tensor_copy(
                                gt8_s[
                                    :,
                                    ct // 2,
                                    sc * 4 : (sc + 1) * 4,
                                    (ct % 2) :: 2,
                                ],
                                mp[:].rearrange("p (b q) -> p b q", b=4),
                            )
                        else:
                            nc.vector.tensor_copy(
                                gt8_s[:, ct, sc * NCH : (sc + 1) * NCH], mp[:]
                            )

                    return th

                kq_th = [
                    [make_g(sc, ct) for ct in range(N_DT)]
                    for sc in range(N_SC)
                ]
                return (gt8_s, xt8_s), dma_th, trans_th, kq_th

            def scores_stationary(gt8_s, j, qt):
                if USE_SWI:
                    return gt8_s[:, j, qt, :]
                return gt8_s[:, 2 * j : 2 * j + 2, qt * P : (qt + 1) * P]

            SC_MODE = SWI if USE_SWI else DR

            def emit_scores_half(gt8_s, xt8_s, qt, h, e2_t, rs_t):
                """One half-row pass for one q-tile: j-major into a [P,2,512]
                two-bank PSUM tile, one 1024-wide exp (fp8 out, offset
                -C_OFF), row-sum via accum_out into rs_t[:, h]."""
                par = qt % 2
                sp = sc_psum.tile([P, 2, NCH], F32, tag="sc")
                for j in range(2):
                    for i in range(2):
                        kc = 2 * h + i
                        nc.tensor.matmul(
                            sp[:, i, :],
                            scores_stationary(gt8_s, j, qt),
                            xt8_s[:, 2 * j : 2 * j + 2, kc * NCH : (kc + 1) * NCH],
                            start=(j == 0),
                            stop=(j == 1),
                            perf_mode=SC_MODE,
                        )
                nc.scalar.activation(
                    e2_t[:, par, h * 2 * NCH : (h + 1) * 2 * NCH],
                    sp[:].rearrange("p a b -> p (a b)"),
                    mybir.ActivationFunctionType.Exp,
                    scale=SCALE / KM,
                    bias=negc_t[:],
                    accum_out=rs_t[:, h : h + 1],
                )

            def emit_r(qt, rs_t, r2_t):
                """r = KR / (rowsumA + rowsumB), broadcast into the qt%2 lane
                of the pair's (interleaved) stationary tile."""
                par = qt % 2
                rtot = soft_pool.tile([P, 1], F32, tag="rtot")
                nc.vector.reduce_sum(rtot[:], rs_t[:], axis=mybir.AxisListType.X)
                rtot_s = soft_pool.tile([P, 1], F32, tag="rtots")
                nc.vector.tensor_scalar_mul(rtot_s[:], rtot[:], 1.0 / KR)
                rrec = soft_pool.tile([P, 1], F32, tag="rrec")
                nc.vector.reciprocal(rrec[:], rtot_s[:])
                if USE_SWI:
                    dst = r2_t[:, par::2]
                else:
                    dst = r2_t[:, par, :]
                nc.vector.tensor_copy(dst, rrec[:, 0:1].broadcast_to([P, P]))

            def colsum_stationary(r2_t):
                if USE_SWI:
                    return r2_t[:, :]
                return r2_t[:, 0:2, :]

            def emit_colsum_pair(w_ps, e2_t, r2_t, pair, kcs, w_off):
                for kc in kcs:
                    nc.tensor.matmul(
                        w_ps[:, kc - w_off, :],
                        colsum_stationary(r2_t),
                        e2_t[:, 0:2, kc * NCH : (kc + 1) * NCH],
                        start=(pair == 0),
                        stop=(pair == N_PAIR - 1),
                        perf_mode=SC_MODE,
                        skip_group_check=True,
                    )

            def phase_A(gt8_s, xt8_s, e2_list, rs_list, extras):
                """Key chunks {0,1} for all 16 q-tiles."""
                for qt in range(N_ST):
                    if qt % 2 == 0:
                        e2_t = e_pool.tile([P, 2, S], F8, tag="e2")
                        e2_list.append(e2_t)
                    rs_t = rs_pool.tile([P, 2], F32, tag="rs")
                    rs_list.append(rs_t)
                    emit_scores_half(gt8_s, xt8_s, qt, 0, e2_list[qt // 2], rs_t)
                    for th in extras[qt]:
                        th()

            def phase_B(gt8_s, xt8_s, e2_list, rs_list, extras):
                """Key chunks {2,3}, r pipeline, inline colsum kc{0,1}
                (2-pair lag, lazy w tile), deferred kc{2,3} sweep thunks."""
                w_holder = {}

                def get_wps():
                    if "a" not in w_holder:
                        w_ps_a = w_psum.tile([P, 2, NCH], F32, tag="w")
                        w_holder["a"] = w_ps_a
                    return w_holder["a"]

                r2_list = []
                pending = []
                for qt in range(N_ST):
                    pair = qt // 2
                    if qt % 2 == 0:
                        if USE_SWI:
                            r2_t = r2_pool.tile([P, 2 * P], F8, tag="r2")
                        else:
                            r2_t = r2_pool.tile([P, 2, P], F8, tag="r2")
                        r2_list.append(r2_t)
                    emit_scores_half(gt8_s, xt8_s, qt, 1, e2_list[pair], rs_list[qt])
                    emit_r(qt, rs_list[qt], r2_list[pair])
                    if qt % 2 == 1:
                        pending.append(pair)
                        if len(pending) > 2:
                            p = pending.pop(0)
                            emit_colsum_pair(
                                get_wps(), e2_list[p], r2_list[p], p, (0, 1), 0
                            )
                    for th in extras[qt]:
                        th()
                for p in pending:
                    emit_colsum_pair(get_wps(), e2_list[p], r2_list[p], p, (0, 1), 0)

                sweep_holder = {}

                def make_sweep_pair(pair):
                    def th():
                        if "b" not in sweep_holder:
                            w_ps_b = w_psum.tile([P, 2, NCH], F32, tag="w")
                            sweep_holder["b"] = w_ps_b
                        emit_colsum_pair(
                            sweep_holder["b"],
                            e2_list[pair],
                            r2_list[pair],
                            pair,
                            (2, 3),
                            2,
                        )

                    return th

                sweep_th = [make_sweep_pair(p) for p in range(N_PAIR)]
                return w_holder["a"], sweep_holder, sweep_th

            def final_thunks(b, w_ps_a, sweep_holder, sweep_th, xnat_s):
                """Colsum sweep kc{2,3} + w-phase: out = (w @ X) @ W_v; the
                KR pre-scale is folded into the final o_sb copy.  y_ps and
                the row->column transposes live in the gp PSUM pool."""
                w_sb = wvec_pool.tile([1, S], BF16, tag="wsb")
                y_ps = gp_psum.tile([P, NCH], F32, tag="gp")
                wt_pads = {}
                yt_pads = {}
                thunks = []

                def make_wcopy(kc):
                    def th():
                        src = w_ps_a if kc < 2 else sweep_holder["b"]
                        nc.vector.tensor_copy(
                            w_sb[:, kc * NCH : (kc + 1) * NCH],
                            src[0:1, kc % 2, :],
                        )

                    return th

                def row_to_bcast_cols(src_row, pads, key, tag):
                    tp = gp_psum.tile([P, 1], F32, tag="gp")
                    nc.tensor.matmul(
                        tp[:], src_row, one_t[0:1, 0:1], start=True, stop=True
                    )
                    pad = wvec_pool.tile([P, P], BF16, tag=tag)
                    nc.vector.tensor_copy(pad[:], tp[:, 0:1].broadcast_to([P, P]))
                    pads[key] = pad

                def make_wtrans(kt):
                    def th():
                        row_to_bcast_cols(
                            w_sb[0:1, kt * P : (kt + 1) * P],
                            wt_pads, kt, f"wtp{kt % 4}",
                        )

                    return th

                def make_ymm(st):
                    def th():
                        nc.tensor.matmul(
                            y_ps[:],
                            wt_pads[st][:],
                            xnat_s[:, st, :],
                            start=(st == 0),
                            stop=(st == N_ST - 1),
                            skip_group_check=True,
                        )

                    return th

                def epilogue_th():
                    y_sb = wvec_pool.tile([1, NCH], BF16, tag="ysb")
                    nc.vector.tensor_copy(y_sb[:], y_ps[0:1, :])
                    o_ps = gp_psum.tile([P, NCH], F32, tag="gp")
                    for c in range(N_DT):
                        row_to_bcast_cols(
                            y_sb[0:1, c * P : (c + 1) * P], yt_pads, c, f"ytp{c}"
                        )
                    for c in range(N_DT):
                        nc.tensor.matmul(
                            o_ps[:],
                            yt_pads[c][:],
                            wv_s[:, c, :],
                            start=(c == 0),
                            stop=(c == N_DT - 1),
                            skip_group_check=True,
                        )
                    o_sb = wvec_pool.tile([1, NCH], F32, tag="osb")
                    nc.vector.tensor_scalar_mul(o_sb[:], o_ps[0:1, :], 1.0 / KR)
                    nc.sync.dma_start(out=out_ext[b : b + 1, :], in_=o_sb[:])

                thunks.append(make_wcopy(0))
                thunks.append(make_wcopy(1))
                thunks.extend(sweep_th)
                thunks.append(make_wcopy(2))
                thunks.append(make_wcopy(3))
                for kt in range(N_ST):
                    thunks.append(make_wtrans(kt))
                    if kt >= 3:
                        thunks.append(make_ymm(kt - 3))
                for st in range(N_ST - 3, N_ST):
                    thunks.append(make_ymm(st))
                thunks.append(epilogue_th)
                return thunks

            def spread(thunks, n_slots):
                slots = [[] for _ in range(n_slots)]
                k = len(thunks)
                for i, th in enumerate(thunks):
                    slots[min(i * n_slots // k, n_slots - 1)].append(th)
                return slots

            # ------------------------- emission ------------------------------

            # FILL: s-tile 0 transpose (f32 path), s-tiles 1-3 transposes,
            # M prework, G s-chunk 0 — just enough for phase A0's q-tile 0.
            h0, dma0, trans0, kq0 = proj_thunks(0, xnat0_s, x0_loaded)
            g0, xt0 = h0

            def first_tile_trans_f32():
                tp = gp_psum.tile([P, N_DT * P], F32, tag="gp")
                for dt_i in range(N_DT):
                    nc.tensor.matmul(
                        tp[:, dt_i * P : (dt_i + 1) * P],
                        xf0[:, dt_i * P : (dt_i + 1) * P],
                        ident_f[:],
                        start=True,
                        stop=True,
                        skip_group_check=True,
                    )
                nc.vector.tensor_copy(
                    xt0[:, :, 0:P],
                    tp[:].rearrange("p (t c) -> p t c", t=N_DT),
                )

            first_tile_trans_f32()
            pre_th = m_prework_thunks()
            # interleave prework (ACT copies) with c0 transposes (DVE copies)
            fill_stream = []
            fill_stream.extend(pre_th[:4])  # wkT units
            fill_stream.extend(trans0[0][1:])  # s-tiles 1-3
            fill_stream.extend(pre_th[4:8])  # wqT units
            fill_stream.extend(trans0[1])  # s-tiles 4-7 (phase A needs kc1)
            fill_stream.extend(pre_th[8:])  # M groups
            fill_stream.extend(kq0[0])  # G s-chunk 0
            for th in fill_stream:
                th()

            # batch 1 proj thunks (woven into B0)
            xnat1_s = xnat_pool.tile([P, N_ST, D], BF16, tag="xnat")
            h1, dma1, trans1, kq1 = proj_thunks(1, xnat1_s, [False] * N_SC)
            g1, xt1 = h1

            # --- phase A0: extras = batch0's remaining transposes/G + batch1
            # DMA kickoff.  Deadlines: G sc1 before qt4, sc2 before qt8,
            # sc3 before qt12; trans c2/c3 before phase B0.
            slots_a0 = [[] for _ in range(N_ST)]
            slots_a0[0] = [kq0[1][0], kq0[1][1]]
            slots_a0[1] = [kq0[1][2], kq0[1][3]]
            slots_a0[2] = [trans0[2][0], trans0[2][1]]
            slots_a0[3] = [trans0[2][2], trans0[2][3]]
            slots_a0[4] = [kq0[2][0], kq0[2][1]]
            slots_a0[5] = [kq0[2][2], kq0[2][3]]
            slots_a0[6] = [trans0[3][0], trans0[3][1]]
            slots_a0[7] = [trans0[3][2], trans0[3][3]]
            slots_a0[8] = [kq0[3][0], kq0[3][1]]
            slots_a0[9] = [kq0[3][2], kq0[3][3]]
            slots_a0[10] = [dma1[0]]
            slots_a0[11] = [dma1[1]]
            dma1[0] = dma1[1] = None

            e2_0, rs_0 = [], []
            phase_A(g0, xt0, e2_0, rs_0, slots_a0)

            # --- phase B0: extras = batch1's projection (DMA c2/c3 early).
            proj1_flat = [dma1[2], dma1[3]]
            dma1[2] = dma1[3] = None
            for sc in range(N_SC):
                proj1_flat.extend(trans1[sc])
                proj1_flat.extend(kq1[sc])
            wa0, swh0, swth0 = phase_B(
                g0, xt0, e2_0, rs_0, spread(proj1_flat, N_ST)
            )

            # --- phase A1: extras = batch0's colsum sweep + w-phase.
            fin0 = final_thunks(0, wa0, swh0, swth0, xnat0_s)
            e2_1, rs_1 = [], []
            phase_A(g1, xt1, e2_1, rs_1, spread(fin0, N_ST))

            # --- phase B1: no extras.
            wa1, swh1, swth1 = phase_B(
                g1, xt1, e2_1, rs_1, [[] for _ in range(N_ST)]
            )

            # --- tail: batch1's sweep + w-phase.
            for th in final_thunks(1, wa1, swh1, swth1, xnat1_s):
                th()

    nc.compile()
    return nc


_NC_CACHE = None


def _get_nc():
    global _NC_CACHE
    if _NC_CACHE is None:
        _NC_CACHE = build_nc()
    return _NC_CACHE


def make_in_maps(inputs, W_q, W_k, W_v):
    inputs = np.ascontiguousarray(np.asarray(inputs, dtype=np.float32))
    W_q = np.ascontiguousarray(np.asarray(W_q, dtype=np.float32))
    W_k = np.ascontiguousarray(np.asarray(W_k, dtype=np.float32))
    W_v = np.ascontiguousarray(np.asarray(W_v, dtype=np.float32))
    return [
        {
            "inputs": inputs[i * B_PER_CORE : (i + 1) * B_PER_CORE],
            "W_q": W_q,
            "W_k": W_k,
            "W_v": W_v,
        }
        for i in range(N_CORES)
    ]


def kernel(**inputs) -> np.ndarray:
    nc = _get_nc()
    in_maps = make_in_maps(
        inputs["inputs"], inputs["W_q"], inputs["W_k"], inputs["W_v"]
    )
    res = run_bass_kernel_spmd(nc, in_maps, core_ids=list(range(N_CORES)))
    return np.concatenate(
        [res.results[i]["out"] for i in range(N_CORES)], axis=0
    ).astype(np.float32)


# revision 26
# speedup vs baseline: 1.4751x; 1.0005x over previous
"""Trainium2 Bass kernel for batched single-head attention with seq-sum pooling.

Reference computation (B=16, S=2048, D=512, fp32):
    q = x @ W_q ; k = x @ W_k ; v = x @ W_v          per batch  [S, D]
    scores = q @ k.T / sqrt(D)                        [S, S]
    attn = softmax(scores, axis=-1)
    out_b = sum_s (attn @ v)[s, :]                    [D]

Algebraic restructures:
1. The final sum over query positions commutes through both trailing
   matmuls: out_b = ((r^T E) @ x) @ W_v with E = exp(scores/sqrt(D)) and
   r[q] = 1/rowsum_q(E) — removes the [S,S]x[S,D] attention-value matmul
   AND the V projection.
2. scores = x M x^T with M = W_q W_k^T computed once per core — one
   G = x M projection replaces both per-batch Q/K projections.

fp8: the G projection, scores, and softmax column-sum matmuls run with
float8e4 operands in DoubleRow mode (K=256/instruction).  Exact foldings
keep fp8 in range: M stored as 16*M, E = exp(s/sqrt(D) - 2.5) (global
offset cancels through the softmax), r stored as 128*r (folded into the
final output copy).  Measured rel err 7.5e-3 (tolerance 2e-2).

Scores/colsum stationaries are stored PRE-INTERLEAVED for
DoubleRowSwInterleave: the HW weight load then reads contiguously instead
of DoubleRow's strided interleave (which disables fast-weight-load and
makes LDWEIGHTS the ~260ns/matmul bottleneck).  The interleave's column
reversal permutes scores rows (q) within each 128-block — harmless, since
every consumer (exp, row-sum, r broadcast, weighted column-sum) is
q-order-free, and all per-q tensors carry the same permutation.  The G
matmul keeps plain DoubleRow (a reversed G would misalign the scores
contraction).

Schedule (from trace analysis: the PE re-throttles 2.4->1.2GHz on idle
windows, so density is everything): each batch's score rows are computed in
two half-row passes — phase A covers key chunks {0,1}, phase B {2,3} —
which lets phase A start after only 4 transposes + 4 G chunks (~6us) and
hides the entire 8MB x DMA under compute.  Row sums accumulate per-half
via exp's accum_out into persistent per-q-tile tiles; r and the colsums
happen in phase B.  Colsum kc{0,1} accumulates inline (2-pair lag);
kc{2,3} runs as a deferred sweep.  Weave plan: batch0's remaining
transposes/G chunks fill phase A0's slack; batch1's projection fills B0;
batch0's sweep+w-phase fills A1; the only serial tails are ~8us of fill
and batch1's w-phase.  PSUM: sc 2x[P,2,512] + gp 2x1 + w 2 banks = 8.

Sharding: pure data parallelism over batch — 2 batch elements per core on
8 NeuronCores, weights replicated, no collectives.
"""

import sys

sys.path.insert(0, "/opt/trn_rl_repo")

import numpy as np

import concourse.bass as bass
import concourse.mybir as mybir
import concourse.tile as tile
from concourse import bacc
from concourse.bass_utils import run_bass_kernel_spmd
from concourse.masks import make_identity

B, S, D = 16, 2048, 512
P = 128
N_CORES = 8
B_PER_CORE = B // N_CORES  # 2
SCALE = 1.0 / float(np.sqrt(D))
KM = 16.0  # M pre-scale (exact power of 2)
KR = 128.0  # r pre-scale (exact power of 2)
C_OFF = 2.5  # global exp offset; cancels through softmax normalization

F32 = mybir.dt.float32
BF16 = mybir.dt.bfloat16
F8 = mybir.dt.float8e4
DR = mybir.MatmulPerfMode.DoubleRow
SWI = mybir.MatmulPerfMode.DoubleRowSwInterleave
USE_SWI = True  # pre-interleaved scores/colsum stationaries

N_ST = S // P  # 16 s-tiles
N_DT = D // P  # 4 d-tiles
NCH = 512  # moving free dim per matmul (one fp32 PSUM bank)
N_SC = S // NCH  # 4 s-chunks
N_KC = S // NCH  # 4 k-chunks
N_PAIR = N_ST // 2  # 8 q-tile pairs


def build_nc():
    nc = bacc.Bacc("TRN2", target_bir_lowering=False, debug=False, num_devices=N_CORES)
    x_ext = nc.dram_tensor(
        "inputs", [B_PER_CORE, S, D], F32, kind="ExternalInput"
    ).ap()
    wq_ext = nc.dram_tensor("W_q", [D, D], F32, kind="ExternalInput").ap()
    wk_ext = nc.dram_tensor("W_k", [D, D], F32, kind="ExternalInput").ap()
    wv_ext = nc.dram_tensor("W_v", [D, D], F32, kind="ExternalInput").ap()
    out_ext = nc.dram_tensor("out", [B_PER_CORE, D], F32, kind="ExternalOutput").ap()

    with tile.TileContext(nc) as tc:
        with (
            tc.tile_pool(name="const", bufs=1) as const_pool,
            tc.tile_pool(name="w", bufs=1) as w_pool,
            tc.tile_pool(name="xnat", bufs=2) as xnat_pool,
            tc.tile_pool(name="xt", bufs=2) as xt_pool,
            tc.tile_pool(name="qkv", bufs=2) as qkv_pool,
            tc.tile_pool(name="e", bufs=17) as e_pool,
            tc.tile_pool(name="soft", bufs=4) as soft_pool,
            tc.tile_pool(name="rs", bufs=36) as rs_pool,
            tc.tile_pool(name="r2", bufs=18) as r2_pool,
            tc.tile_pool(name="wvec", bufs=2) as wvec_pool,
            tc.tile_pool(name="scps", bufs=2, space="PSUM") as sc_psum,
            tc.tile_pool(name="gpps", bufs=2, space="PSUM") as gp_psum,
            tc.tile_pool(name="wps", bufs=1, space="PSUM") as w_psum,
        ):
            one_t = const_pool.tile([1, 1], BF16)
            nc.gpsimd.memset(one_t[:], 1.0)
            ident_f = const_pool.tile([P, P], F32)
            make_identity(nc, ident_f[:])
            ident = const_pool.tile([P, P], BF16)
            nc.vector.tensor_copy(ident[:], ident_f[:])
            negc_t = const_pool.tile([P, 1], F32)
            nc.gpsimd.memset(negc_t[:], -C_OFF)

            # HAM warmup: the PE clock boots throttled to 1.2GHz and only
            # un-throttles to 2.4GHz after ~3.4us of sustained matmul
            # activity (one full busy window of the hardware activity
            # monitor) — and a DoubleRow/fp8 stream SUSTAINS the warm state
            # but was never observed to CREATE it.  Dependency-free bf16
            # dummy matmuls are woven BETWEEN the fill-phase units (whose
            # pace is copy-bound, leaving PE gaps) so the PE stream is
            # gapless from t~1us without delaying real work.
            warm_mov = const_pool.tile([P, NCH], BF16)
            nc.gpsimd.memset(warm_mov[:], 0.0)
            warm_ps = sc_psum.tile([P, 2, NCH], F32, tag="sc")
            warm_i = [0]

            def emit_warm_dummy(n=1):
                for _ in range(n):
                    nc.tensor.matmul(
                        warm_ps[:, warm_i[0] % 2, :],
                        ident[:],
                        warm_mov[:],
                        start=True,
                        stop=True,
                        skip_group_check=True,
                    )
                    warm_i[0] += 1

            emit_warm_dummy(5)

            def dma_x_chunk(b, sc, xnat_s):
                nc.gpsimd.dma_start(
                    out=xnat_s[:, sc * 4 : (sc + 1) * 4, :],
                    in_=x_ext[b, sc * NCH : (sc + 1) * NCH, :].rearrange(
                        "(t p) d -> p t d", p=P
                    ),
                )

            w_tiles = {}

            def dma_w(name, ext):
                w_s = w_pool.tile([P, N_DT, D], BF16, tag=name)
                nc.gpsimd.dma_start(
                    out=w_s[:], in_=ext.rearrange("(t p) e -> p t e", p=P)
                )
                w_tiles[name] = w_s

            # SWDGE order: weights first (M prework gates the scores start),
            # then batch 0's x chunks.  s-tile 0 rides HWDGE as f32.
            xnat0_s = xnat_pool.tile([P, N_ST, D], BF16, tag="xnat")
            xf0 = xnat_pool.tile([P, D], F32, tag="xf0")
            nc.sync.dma_start(out=xf0[:], in_=x_ext[0, 0:P, :])
            nc.vector.tensor_copy(xnat0_s[:, 0, :], xf0[:])
            dma_w("wk", wk_ext)
            dma_w("wq", wq_ext)
            nc.gpsimd.dma_start(
                out=xnat0_s[:, 1:4, :],
                in_=x_ext[0, P:NCH, :].rearrange("(t p) d -> p t d", p=P),
            )
            dma_x_chunk(0, 1, xnat0_s)
            dma_x_chunk(0, 2, xnat0_s)
            dma_x_chunk(0, 3, xnat0_s)
            dma_w("wv", wv_ext)
            x0_loaded = [True] * N_SC
            wk_s, wq_s, wv_s = w_tiles["wk"], w_tiles["wq"], w_tiles["wv"]

            # One-time prework: M = Wq Wk^T, stored as 16*M fp8 (raw entries
            # would be fp8-subnormal; the exp scale divides the 16 out).
            # wqT scaled by 16 at its ACT copy; wkT copies also on ACT so the
            # fill phase's DVE stays on the x transposes.
            wqT_s = w_pool.tile([P, N_DT, D], BF16, tag="wqT")
            wkT_s = w_pool.tile([P, N_DT, D], BF16, tag="wkT")
            m8_s = w_pool.tile([P, N_DT, D], F8, tag="m8")

            def m_prework_thunks():
                thunks = []

                def make_wtrans_unit(src_w, dst, t_e, scale):
                    def th():
                        tp = gp_psum.tile([P, N_DT * P], F32, tag="gp")
                        for t_a in range(N_DT):
                            nc.tensor.matmul(
                                tp[:, t_a * P : (t_a + 1) * P],
                                src_w[:, t_a, t_e * P : (t_e + 1) * P],
                                ident[:],
                                start=True,
                                stop=True,
                                skip_group_check=True,
                            )
                        if scale is None:
                            nc.scalar.copy(dst[:, t_e, :], tp[:])
                        else:
                            nc.scalar.mul(dst[:, t_e, :], tp[:], scale)

                    return th

                def make_m_group(t_a):
                    def th():
                        mp = gp_psum.tile([P, NCH], F32, tag="gp")
                        for t_e in range(N_DT):
                            nc.tensor.matmul(
                                mp[:],
                                wqT_s[:, t_e, t_a * P : (t_a + 1) * P],
                                wkT_s[:, t_e, :],
                                start=(t_e == 0),
                                stop=(t_e == N_DT - 1),
                            )
                        # ACT copy balances the fill phase's copy load
                        # (DVE carries the x transposes + G casts)
                        nc.scalar.copy(m8_s[:, t_a, :], mp[:])

                    return th

                for t_e in range(N_DT):
                    thunks.append(make_wtrans_unit(wk_s, wkT_s, t_e, None))
                for t_e in range(N_DT):
                    thunks.append(make_wtrans_unit(wq_s, wqT_s, t_e, KM))
                for t_a in range(N_DT):
                    thunks.append(make_m_group(t_a))
                return thunks

            # ---------- thunk builders --------------------------------------

            def proj_thunks(b, xnat_s, loaded):
                """Transpose + G = X M projection thunks for batch b.  xt8 is
                [P, dtile, S] fp8.  gt8 layout depends on USE_SWI:
                  - SWI: [P, jpair, qt_block, 2*P] with the two d-subtiles of
                    a jpair interleaved along the last dim (stored UNreversed;
                    the HW's column reversal permutes q within blocks, which
                    every downstream consumer absorbs).
                  - plain DR: [P, dtile, S]."""
                xt8_s = xt_pool.tile([P, N_DT, S], F8, tag="xt")
                if USE_SWI:
                    gt8_s = qkv_pool.tile([P, 2, N_ST, 2 * P], F8, tag="gt")
                else:
                    gt8_s = qkv_pool.tile([P, N_DT, S], F8, tag="gt")

                def make_dma(sc):
                    def th():
                        dma_x_chunk(b, sc, xnat_s)

                    return th

                dma_th = [
                    None if loaded[sc] else make_dma(sc) for sc in range(N_SC)
                ]

                def make_trans_unit(sc, t_i):
                    def th():
                        st = sc * 4 + t_i
                        tp = gp_psum.tile([P, N_DT * P], F32, tag="gp")
                        for dt_i in range(N_DT):
                            nc.tensor.matmul(
                                tp[:, dt_i * P : (dt_i + 1) * P],
                                xnat_s[:, st, dt_i * P : (dt_i + 1) * P],
                                ident[:],
                                start=True,
                                stop=True,
                                skip_group_check=True,
                            )
                        nc.vector.tensor_copy(
                            xt8_s[:, :, st * P : (st + 1) * P],
                            tp[:].rearrange("p (t c) -> p t c", t=N_DT),
                        )

                    return th

                trans_th = [
                    [make_trans_unit(sc, t_i) for t_i in range(4)]
                    for sc in range(N_SC)
                ]

                def make_g(sc, ct):
                    def th():
                        mp = gp_psum.tile([P, NCH], F32, tag="gp")
                        for j in range(2):
                            nc.tensor.matmul(
                                mp[:],
                                m8_s[:, 2 * j : 2 * j + 2, ct * P : (ct + 1) * P],
                                xt8_s[:, 2 * j : 2 * j + 2, sc * NCH : (sc + 1) * NCH],
                                start=(j == 0),
                                stop=(j == 1),
                                perf_mode=DR,
                            )
                        if USE_SWI:
                            nc.vector.tensor_copy(
                                gt8_s[
                                    :,
                                    ct // 2,
                                    sc * 4 : (sc + 1) * 4,
                                    (ct % 2) :: 2,
                                ],
                                mp[:].rearrange("p (b q) -> p b q", b=4),
                            )
                        else:
                            nc.vector.tensor_copy(
                                gt8_s[:, ct, sc * NCH : (sc + 1) * NCH], mp[:]
                            )

                    return th

                kq_th = [
                    [make_g(sc, ct) for ct in range(N_DT)]
                    for sc in range(N_SC)
                ]
                return (gt8_s, xt8_s), dma_th, trans_th, kq_th

            def scores_stationary(gt8_s, j, qt):
                if USE_SWI:
                    return gt8_s[:, j, qt, :]
                return gt8_s[:, 2 * j : 2 * j + 2, qt * P : (qt + 1) * P]

            SC_MODE = SWI if USE_SWI else DR

            def emit_scores_half(gt8_s, xt8_s, qt, h, e2_t, rs_t):
                """One half-row pass for one q-tile: j-major into a [P,2,512]
                two-bank PSUM tile, one 1024-wide exp (fp8 out, offset
                -C_OFF), row-sum via accum_out into rs_t[:, h]."""
                par = qt % 2
                sp = sc_psum.tile([P, 2, NCH], F32, tag="sc")
                for j in range(2):
                    for i in range(2):
                        kc = 2 * h + i
                        nc.tensor.matmul(
                            sp[:, i, :],
                            scores_stationary(gt8_s, j, qt),
                            xt8_s[:, 2 * j : 2 * j + 2, kc * NCH : (kc + 1) * NCH],
                            start=(j == 0),
                            stop=(j == 1),
                            perf_mode=SC_MODE,
                        )
                nc.scalar.activation(
                    e2_t[:, par, h * 2 * NCH : (h + 1) * 2 * NCH],
                    sp[:].rearrange("p a b -> p (a b)"),
                    mybir.ActivationFunctionType.Exp,
                    scale=SCALE / KM,
                    bias=negc_t[:],
                    accum_out=rs_t[:, h : h + 1],
                )

            def emit_r(qt, rs_t, r2_t):
                """r = KR / (rowsumA + rowsumB), broadcast into the qt%2 lane
                of the pair's (interleaved) stationary tile."""
                par = qt % 2
                rtot = soft_pool.tile([P, 1], F32, tag="rtot")
                nc.vector.reduce_sum(rtot[:], rs_t[:], axis=mybir.AxisListType.X)
                rtot_s = soft_pool.tile([P, 1], F32, tag="rtots")
                nc.vector.tensor_scalar_mul(rtot_s[:], rtot[:], 1.0 / KR)
                rrec = soft_pool.tile([P, 1], F32, tag="rrec")
                nc.vector.reciprocal(rrec[:], rtot_s[:])
                if USE_SWI:
                    dst = r2_t[:, par::2]
                else:
                    dst = r2_t[:, par, :]
                nc.vector.tensor_copy(dst, rrec[:, 0:1].broadcast_to([P, P]))

            def colsum_stationary(r2_t):
                if USE_SWI:
                    return r2_t[:, :]
                return r2_t[:, 0:2, :]

            def emit_colsum_pair(w_ps, e2_t, r2_t, pair, kcs, w_off):
                for kc in kcs:
                    nc.tensor.matmul(
                        w_ps[:, kc - w_off, :],
                        colsum_stationary(r2_t),
                        e2_t[:, 0:2, kc * NCH : (kc + 1) * NCH],
                        start=(pair == 0),
                        stop=(pair == N_PAIR - 1),
                        perf_mode=SC_MODE,
                        skip_group_check=True,
                    )

            def phase_A(gt8_s, xt8_s, e2_list, rs_list, extras):
                """Key chunks {0,1} for all 16 q-tiles."""
                for qt in range(N_ST):
                    if qt % 2 == 0:
                        e2_t = e_pool.tile([P, 2, S], F8, tag="e2")
                        e2_list.append(e2_t)
                    rs_t = rs_pool.tile([P, 2], F32, tag="rs")
                    rs_list.append(rs_t)
                    emit_scores_half(gt8_s, xt8_s, qt, 0, e2_list[qt // 2], rs_t)
                    for th in extras[qt]:
                        th()

            def phase_B(gt8_s, xt8_s, e2_list, rs_list, extras):
                """Key chunks {2,3}, r pipeline, inline colsum kc{0,1}
                (2-pair lag, lazy w tile), deferred kc{2,3} sweep thunks."""
                w_holder = {}

                def get_wps():
                    if "a" not in w_holder:
                        w_ps_a = w_psum.tile([P, 2, NCH], F32, tag="w")
                        w_holder["a"] = w_ps_a
                    return w_holder["a"]

                r2_list = []
                pending = []
                for qt in range(N_ST):
                    pair = qt // 2
                    if qt % 2 == 0:
                        if USE_SWI:
                            r2_t = r2_pool.tile([P, 2 * P], F8, tag="r2")
                        else:
                            r2_t = r2_pool.tile([P, 2, P], F8, tag="r2")
                        r2_list.append(r2_t)
                    emit_scores_half(gt8_s, xt8_s, qt, 1, e2_list[pair], rs_list[qt])
                    emit_r(qt, rs_list[qt], r2_list[pair])
                    if qt % 2 == 1:
                        pending.append(pair)
                        if len(pending) > 2:
                            p = pending.pop(0)
                            emit_colsum_pair(
                                get_wps(), e2_list[p], r2_list[p], p, (0, 1), 0
                            )
                    for th in extras[qt]:
                        th()
                for p in pending:
                    emit_colsum_pair(get_wps(), e2_list[p], r2_list[p], p, (0, 1), 0)

                sweep_holder = {}

                def make_sweep_pair(pair):
                    def th():
                        if "b" not in sweep_holder:
                            w_ps_b = w_psum.tile([P, 2, NCH], F32, tag="w")
                            sweep_holder["b"] = w_ps_b
                        emit_colsum_pair(
                            sweep_holder["b"],
                            e2_list[pair],
                            r2_list[pair],
                            pair,
                            (2, 3),
                            2,
                        )

                    return th

                sweep_th = [make_sweep_pair(p) for p in range(N_PAIR)]
                return w_holder["a"], sweep_holder, sweep_th

            def final_thunks(b, w_ps_a, sweep_holder, sweep_th, xnat_s):
                """Colsum sweep kc{2,3} + w-phase: out = (w @ X) @ W_v; the
                KR pre-scale is folded into the final o_sb copy.  y_ps and
                the row->column transposes live in the gp PSUM pool."""
                w_sb = wvec_pool.tile([1, S], BF16, tag="wsb")
                y_ps = gp_psum.tile([P, NCH], F32, tag="gp")
                wt_pads = {}
                yt_pads = {}
                thunks = []

                def make_wcopy(kc):
                    def th():
                        src = w_ps_a if kc < 2 else sweep_holder["b"]
                        nc.vector.tensor_copy(
                            w_sb[:, kc * NCH : (kc + 1) * NCH],
                            src[0:1, kc % 2, :],
                        )

                    return th

                def row_to_bcast_cols(src_row, pads, key, tag):
                    tp = gp_psum.tile([P, 1], F32, tag="gp")
                    nc.tensor.matmul(
                        tp[:], src_row, one_t[0:1, 0:1], start=True, stop=True
                    )
                    pad = wvec_pool.tile([P, P], BF16, tag=tag)
                    nc.vector.tensor_copy(pad[:], tp[:, 0:1].broadcast_to([P, P]))
                    pads[key] = pad

                def make_wtrans(kt):
                    def th():
                        row_to_bcast_cols(
                            w_sb[0:1, kt * P : (kt + 1) * P],
                            wt_pads, kt, f"wtp{kt % 4}",
                        )

                    return th

                def make_ymm(st):
                    def th():
                        nc.tensor.matmul(
                            y_ps[:],
                            wt_pads[st][:],
                            xnat_s[:, st, :],
                            start=(st == 0),
                            stop=(st == N_ST - 1),
                            skip_group_check=True,
                        )

                    return th

                def epilogue_th():
                    y_sb = wvec_pool.tile([1, NCH], BF16, tag="ysb")
                    nc.vector.tensor_copy(y_sb[:], y_ps[0:1, :])
                    o_ps = gp_psum.tile([P, NCH], F32, tag="gp")
                    for c in range(N_DT):
                        row_to_bcast_cols(
                            y_sb[0:1, c * P : (c + 1) * P], yt_pads, c, f"ytp{c}"
                        )
                    for c in range(N_DT):
                        nc.tensor.matmul(
                            o_ps[:],
                            yt_pads[c][:],
                            wv_s[:, c, :],
                            start=(c == 0),
                            stop=(c == N_DT - 1),
                            skip_group_check=True,
                        )
                    o_sb = wvec_pool.tile([1, NCH], F32, tag="osb")
                    nc.vector.tensor_scalar_mul(o_sb[:], o_ps[0:1, :], 1.0 / KR)
                    nc.sync.dma_start(out=out_ext[b : b + 1, :], in_=o_sb[:])

                thunks.append(make_wcopy(0))
                thunks.append(make_wcopy(1))
                thunks.extend(sweep_th)
                thunks.append(make_wcopy(2))
                thunks.append(make_wcopy(3))
                for kt in range(N_ST):
                    thunks.append(make_wtrans(kt))
                    if kt >= 3:
                        thunks.append(make_ymm(kt - 3))
                for st in range(N_ST - 3, N_ST):
                    thunks.append(make_ymm(st))
                thunks.append(epilogue_th)
                return thunks

            def spread(thunks, n_slots):
                slots = [[] for _ in range(n_slots)]
                k = len(thunks)
                for i, th in enumerate(thunks):
                    slots[min(i * n_slots // k, n_slots - 1)].append(th)
                return slots

            # ------------------------- emission ------------------------------

            # FILL: s-tile 0 transpose (f32 path), s-tiles 1-3 transposes,
            # M prework, G s-chunk 0 — just enough for phase A0's q-tile 0.
            h0, dma0, trans0, kq0 = proj_thunks(0, xnat0_s, x0_loaded)
            g0, xt0 = h0

            def first_tile_trans_f32():
                tp = gp_psum.tile([P, N_DT * P], F32, tag="gp")
                for dt_i in range(N_DT):
                    nc.tensor.matmul(
                        tp[:, dt_i * P : (dt_i + 1) * P],
                        xf0[:, dt_i * P : (dt_i + 1) * P],
                        ident_f[:],
                        start=True,
                        stop=True,
                        skip_group_check=True,
                    )
                nc.vector.tensor_copy(
                    xt0[:, :, 0:P],
                    tp[:].rearrange("p (t c) -> p t c", t=N_DT),
                )

            first_tile_trans_f32()
            pre_th = m_prework_thunks()
            # interleave prework (ACT copies) with c0 transposes (DVE copies);
            # a dummy matmul after each early unit keeps the copy-bound fill
            # phase's PE stream gapless so the clock warms by ~4.5us
            fill_stream = []
            fill_stream.extend(pre_th[:4])  # wkT units
            fill_stream.extend(trans0[0][1:])  # s-tiles 1-3
            fill_stream.extend(pre_th[4:8])  # wqT units
            fill_stream.extend(trans0[1])  # s-tiles 4-7 (phase A needs kc1)
            fill_stream.extend(pre_th[8:])  # M groups
            fill_stream.extend(kq0[0])  # G s-chunk 0
            for i, th in enumerate(fill_stream):
                th()
                if i < 14:
                    emit_warm_dummy(1)

            # batch 1 proj thunks (woven into B0)
            xnat1_s = xnat_pool.tile([P, N_ST, D], BF16, tag="xnat")
            h1, dma1, trans1, kq1 = proj_thunks(1, xnat1_s, [False] * N_SC)
            g1, xt1 = h1

            # --- phase A0: extras = batch0's remaining transposes/G + batch1
            # DMA kickoff.  Deadlines: G sc1 before qt4, sc2 before qt8,
            # sc3 before qt12; trans c2/c3 before phase B0.
            slots_a0 = [[] for _ in range(N_ST)]
            slots_a0[0] = [kq0[1][0], kq0[1][1]]
            slots_a0[1] = [kq0[1][2], kq0[1][3]]
            slots_a0[2] = [trans0[2][0], trans0[2][1]]
            slots_a0[3] = [trans0[2][2], trans0[2][3]]
            slots_a0[4] = [kq0[2][0], kq0[2][1]]
            slots_a0[5] = [kq0[2][2], kq0[2][3]]
            slots_a0[6] = [trans0[3][0], trans0[3][1]]
            slots_a0[7] = [trans0[3][2], trans0[3][3]]
            slots_a0[8] = [kq0[3][0], kq0[3][1]]
            slots_a0[9] = [kq0[3][2], kq0[3][3]]
            slots_a0[10] = [dma1[0]]
            slots_a0[11] = [dma1[1]]
            dma1[0] = dma1[1] = None

            e2_0, rs_0 = [], []
            phase_A(g0, xt0, e2_0, rs_0, slots_a0)

            # --- phase B0: extras = batch1's projection (DMA c2/c3 early).
            proj1_flat = [dma1[2], dma1[3]]
            dma1[2] = dma1[3] = None
            for sc in range(N_SC):
                proj1_flat.extend(trans1[sc])
                proj1_flat.extend(kq1[sc])
            wa0, swh0, swth0 = phase_B(
                g0, xt0, e2_0, rs_0, spread(proj1_flat, N_ST)
            )

            # --- phase A1: extras = batch0's colsum sweep + w-phase.
            fin0 = final_thunks(0, wa0, swh0, swth0, xnat0_s)
            e2_1, rs_1 = [], []
            phase_A(g1, xt1, e2_1, rs_1, spread(fin0, N_ST))

            # --- phase B1: no extras.
            wa1, swh1, swth1 = phase_B(
                g1, xt1, e2_1, rs_1, [[] for _ in range(N_ST)]
            )

            # --- tail: batch1's sweep + w-phase.
            for th in final_thunks(1, wa1, swh1, swth1, xnat1_s):
                th()

    nc.compile()
    return nc


_NC_CACHE = None


def _get_nc():
    global _NC_CACHE
    if _NC_CACHE is None:
        _NC_CACHE = build_nc()
    return _NC_CACHE


def make_in_maps(inputs, W_q, W_k, W_v):
    inputs = np.ascontiguousarray(np.asarray(inputs, dtype=np.float32))
    W_q = np.ascontiguousarray(np.asarray(W_q, dtype=np.float32))
    W_k = np.ascontiguousarray(np.asarray(W_k, dtype=np.float32))
    W_v = np.ascontiguousarray(np.asarray(W_v, dtype=np.float32))
    return [
        {
            "inputs": inputs[i * B_PER_CORE : (i + 1) * B_PER_CORE],
            "W_q": W_q,
            "W_k": W_k,
            "W_v": W_v,
        }
        for i in range(N_CORES)
    ]


def kernel(**inputs) -> np.ndarray:
    nc = _get_nc()
    in_maps = make_in_maps(
        inputs["inputs"], inputs["W_q"], inputs["W_k"], inputs["W_v"]
    )
    res = run_bass_kernel_spmd(nc, in_maps, core_ids=list(range(N_CORES)))
    return np.concatenate(
        [res.results[i]["out"] for i in range(N_CORES)], axis=0
    ).astype(np.float32)


# revision 33
# speedup vs baseline: 1.5070x; 1.0217x over previous
"""Trainium2 Bass kernel for batched single-head attention with seq-sum pooling.

Reference computation (B=16, S=2048, D=512, fp32):
    q = x @ W_q ; k = x @ W_k ; v = x @ W_v          per batch  [S, D]
    scores = q @ k.T / sqrt(D)                        [S, S]
    attn = softmax(scores, axis=-1)
    out_b = sum_s (attn @ v)[s, :]                    [D]

Algebraic restructures:
1. The final sum over query positions commutes through both trailing
   matmuls: out_b = ((r^T E) @ x) @ W_v with E = exp(scores/sqrt(D)) and
   r[q] = 1/rowsum_q(E) — removes the [S,S]x[S,D] attention-value matmul
   AND the V projection.
2. scores = x M x^T with M = W_q W_k^T computed once per core — one
   G = x M projection replaces both per-batch Q/K projections.

fp8: the G projection, scores, and softmax column-sum matmuls run with
float8e4 operands in DoubleRow mode (K=256/instruction).  Exact foldings
keep fp8 in range: M stored as 16*M, E = exp(s/sqrt(D) - 2.5) (global
offset cancels through the softmax), r stored as 128*r (folded into the
final output copy).  Measured rel err 7.5e-3 (tolerance 2e-2).

Scores/colsum stationaries are stored PRE-INTERLEAVED for
DoubleRowSwInterleave: the HW weight load then reads contiguously instead
of DoubleRow's strided interleave (which disables fast-weight-load and
makes LDWEIGHTS the ~260ns/matmul bottleneck).  The interleave's column
reversal permutes scores rows (q) within each 128-block — harmless, since
every consumer (exp, row-sum, r broadcast, weighted column-sum) is
q-order-free, and all per-q tensors carry the same permutation.  The G
matmul keeps plain DoubleRow (a reversed G would misalign the scores
contraction).

Schedule (from trace analysis: the PE re-throttles 2.4->1.2GHz on idle
windows, so density is everything): each batch's score rows are computed in
two half-row passes — phase A covers key chunks {0,1}, phase B {2,3} —
which lets phase A start after only 4 transposes + 4 G chunks (~6us) and
hides the entire 8MB x DMA under compute.  Row sums accumulate per-half
via exp's accum_out into persistent per-q-tile tiles; r and the colsums
happen in phase B.  Colsum kc{0,1} accumulates inline (2-pair lag);
kc{2,3} runs as a deferred sweep.  Weave plan: batch0's remaining
transposes/G chunks fill phase A0's slack; batch1's projection fills B0;
batch0's sweep+w-phase fills A1; the only serial tails are ~8us of fill
and batch1's w-phase.  PSUM: sc 2x[P,2,512] + gp 2x1 + w 2 banks = 8.

Sharding: pure data parallelism over batch — 2 batch elements per core on
8 NeuronCores, weights replicated, no collectives.
"""

import sys

sys.path.insert(0, "/opt/trn_rl_repo")

import numpy as np

import concourse.bass as bass
import concourse.mybir as mybir
import concourse.tile as tile
from concourse import bacc
from concourse.bass_utils import run_bass_kernel_spmd
from concourse.masks import make_identity

B, S, D = 16, 2048, 512
P = 128
N_CORES = 8
B_PER_CORE = B // N_CORES  # 2
SCALE = 1.0 / float(np.sqrt(D))
KM = 16.0  # M pre-scale (exact power of 2)
KR = 128.0  # r pre-scale (exact power of 2)
C_OFF = 2.5  # global exp offset; cancels through softmax normalization

F32 = mybir.dt.float32
BF16 = mybir.dt.bfloat16
F8 = mybir.dt.float8e4
DR = mybir.MatmulPerfMode.DoubleRow
SWI = mybir.MatmulPerfMode.DoubleRowSwInterleave
USE_SWI = True  # pre-interleaved scores/colsum stationaries

N_ST = S // P  # 16 s-tiles
N_DT = D // P  # 4 d-tiles
NCH = 512  # moving free dim per matmul (one fp32 PSUM bank)
N_SC = S // NCH  # 4 s-chunks
N_KC = S // NCH  # 4 k-chunks
N_PAIR = N_ST // 2  # 8 q-tile pairs


def build_nc():
    nc = bacc.Bacc("TRN2", target_bir_lowering=False, debug=False, num_devices=N_CORES)
    x_ext = nc.dram_tensor(
        "inputs", [B_PER_CORE, S, D], F32, kind="ExternalInput"
    ).ap()
    wq_ext = nc.dram_tensor("W_q", [D, D], F32, kind="ExternalInput").ap()
    wk_ext = nc.dram_tensor("W_k", [D, D], F32, kind="ExternalInput").ap()
    wv_ext = nc.dram_tensor("W_v", [D, D], F32, kind="ExternalInput").ap()
    out_ext = nc.dram_tensor("out", [B_PER_CORE, D], F32, kind="ExternalOutput").ap()

    with tile.TileContext(nc) as tc:
        with (
            tc.tile_pool(name="const", bufs=1) as const_pool,
            tc.tile_pool(name="w", bufs=1) as w_pool,
            tc.tile_pool(name="xnat", bufs=2) as xnat_pool,
            tc.tile_pool(name="xt", bufs=2) as xt_pool,
            tc.tile_pool(name="qkv", bufs=2) as qkv_pool,
            tc.tile_pool(name="e", bufs=17) as e_pool,
            tc.tile_pool(name="soft", bufs=4) as soft_pool,
            tc.tile_pool(name="rs", bufs=36) as rs_pool,
            tc.tile_pool(name="r2", bufs=18) as r2_pool,
            tc.tile_pool(name="wvec", bufs=2) as wvec_pool,
            tc.tile_pool(name="scps", bufs=2, space="PSUM") as sc_psum,
            tc.tile_pool(name="gpps", bufs=2, space="PSUM") as gp_psum,
            tc.tile_pool(name="wps", bufs=1, space="PSUM") as w_psum,
        ):
            one_t = const_pool.tile([1, 1], BF16)
            nc.gpsimd.memset(one_t[:], 1.0)
            ident_f = const_pool.tile([P, P], F32)
            make_identity(nc, ident_f[:])
            ident = const_pool.tile([P, P], BF16)
            nc.vector.tensor_copy(ident[:], ident_f[:])
            negc_t = const_pool.tile([P, 1], F32)
            nc.gpsimd.memset(negc_t[:], -C_OFF)

            # HAM warmup: the PE clock boots throttled to 1.2GHz and only
            # un-throttles to 2.4GHz after ~3.4us of sustained matmul
            # activity (one full busy window of the hardware activity
            # monitor) — and a DoubleRow/fp8 stream SUSTAINS the warm state
            # but was never observed to CREATE it.  Dependency-free bf16
            # dummy matmuls are woven BETWEEN the fill-phase units (whose
            # pace is copy-bound, leaving PE gaps) so the PE stream is
            # gapless from t~1us without delaying real work.
            warm_mov = const_pool.tile([P, NCH], BF16)
            nc.gpsimd.memset(warm_mov[:], 0.0)
            warm_ps = sc_psum.tile([P, 2, NCH], F32, tag="sc")
            warm_i = [0]

            def emit_warm_dummy(n=1):
                for _ in range(n):
                    nc.tensor.matmul(
                        warm_ps[:, warm_i[0] % 2, :],
                        ident[:],
                        warm_mov[:],
                        start=True,
                        stop=True,
                        skip_group_check=True,
                    )
                    warm_i[0] += 1

            emit_warm_dummy(5)

            def dma_x_chunk(b, sc, xnat_s):
                nc.gpsimd.dma_start(
                    out=xnat_s[:, sc * 4 : (sc + 1) * 4, :],
                    in_=x_ext[b, sc * NCH : (sc + 1) * NCH, :].rearrange(
                        "(t p) d -> p t d", p=P
                    ),
                )

            w_tiles = {}

            def dma_w(name, ext):
                w_s = w_pool.tile([P, N_DT, D], BF16, tag=name)
                nc.gpsimd.dma_start(
                    out=w_s[:], in_=ext.rearrange("(t p) e -> p t e", p=P)
                )
                w_tiles[name] = w_s

            # DMA plan.  One SWDGE dma_start of ~1MB costs ~6us and the queue
            # is FIFO, so the x chunks monopolize it: s1-3, then batch0
            # chunks 1-3, then ALL of batch1's chunks, then wv — each landing
            # just ahead of its consumer phase.  The weights ride the two
            # parallel HWDGE queues as plain f32 (HWDGE can't cast; the
            # prework transposes consume f32 directly), so M prework starts
            # at ~4us instead of ~12.  s-tile 0 is f32 on the sync queue.
            xnat0_s = xnat_pool.tile([P, N_ST, D], BF16, tag="xnat")
            xnat1_s = xnat_pool.tile([P, N_ST, D], BF16, tag="xnat")
            xf0 = xnat_pool.tile([P, D], F32, tag="xf0")
            wkf_s = w_pool.tile([P, N_DT, D], F32, tag="wkf")
            wqf_s = w_pool.tile([P, N_DT, D], F32, tag="wqf")
            nc.scalar.dma_start(
                out=wkf_s[:], in_=wk_ext.rearrange("(t p) e -> p t e", p=P)
            )
            nc.sync.dma_start(out=xf0[:], in_=x_ext[0, 0:P, :])
            nc.sync.dma_start(
                out=wqf_s[:], in_=wq_ext.rearrange("(t p) e -> p t e", p=P)
            )
            nc.vector.tensor_copy(xnat0_s[:, 0, :], xf0[:])
            nc.gpsimd.dma_start(
                out=xnat0_s[:, 1:4, :],
                in_=x_ext[0, P:NCH, :].rearrange("(t p) d -> p t d", p=P),
            )
            dma_x_chunk(0, 1, xnat0_s)
            dma_x_chunk(0, 2, xnat0_s)
            dma_x_chunk(0, 3, xnat0_s)
            for sc in range(N_SC):
                dma_x_chunk(1, sc, xnat1_s)
            dma_w("wv", wv_ext)
            x0_loaded = [True] * N_SC
            wv_s = w_tiles["wv"]

            # One-time prework: M = Wq Wk^T, stored as 16*M fp8 (raw entries
            # would be fp8-subnormal; the exp scale divides the 16 out).
            # wqT scaled by 16 at its ACT copy; wkT copies also on ACT so the
            # fill phase's DVE stays on the x transposes.
            wqT_s = w_pool.tile([P, N_DT, D], BF16, tag="wqT")
            wkT_s = w_pool.tile([P, N_DT, D], BF16, tag="wkT")
            m8_s = w_pool.tile([P, N_DT, D], F8, tag="m8")

            def m_prework_thunks():
                thunks = []

                def make_wtrans_unit(src_w, dst, t_e, scale):
                    def th():
                        tp = gp_psum.tile([P, N_DT * P], F32, tag="gp")
                        for t_a in range(N_DT):
                            nc.tensor.matmul(
                                tp[:, t_a * P : (t_a + 1) * P],
                                src_w[:, t_a, t_e * P : (t_e + 1) * P],
                                ident_f[:],
                                start=True,
                                stop=True,
                                skip_group_check=True,
                            )
                        if scale is None:
                            nc.scalar.copy(dst[:, t_e, :], tp[:])
                        else:
                            nc.scalar.mul(dst[:, t_e, :], tp[:], scale)

                    return th

                def make_m_group(t_a):
                    def th():
                        mp = gp_psum.tile([P, NCH], F32, tag="gp")
                        for t_e in range(N_DT):
                            nc.tensor.matmul(
                                mp[:],
                                wqT_s[:, t_e, t_a * P : (t_a + 1) * P],
                                wkT_s[:, t_e, :],
                                start=(t_e == 0),
                                stop=(t_e == N_DT - 1),
                            )
                        # ACT copy balances the fill phase's copy load
                        # (DVE carries the x transposes + G casts)
                        nc.scalar.copy(m8_s[:, t_a, :], mp[:])

                    return th

                for t_e in range(N_DT):
                    thunks.append(make_wtrans_unit(wkf_s, wkT_s, t_e, None))
                for t_e in range(N_DT):
                    thunks.append(make_wtrans_unit(wqf_s, wqT_s, t_e, KM))
                for t_a in range(N_DT):
                    thunks.append(make_m_group(t_a))
                return thunks

            # ---------- thunk builders --------------------------------------

            def proj_thunks(b, xnat_s, loaded):
                """Transpose + G = X M projection thunks for batch b.  xt8 is
                [P, dtile, S] fp8.  gt8 layout depends on USE_SWI:
                  - SWI: [P, jpair, qt_block, 2*P] with the two d-subtiles of
                    a jpair interleaved along the last dim (stored UNreversed;
                    the HW's column reversal permutes q within blocks, which
                    every downstream consumer absorbs).
                  - plain DR: [P, dtile, S]."""
                xt8_s = xt_pool.tile([P, N_DT, S], F8, tag="xt")
                if USE_SWI:
                    gt8_s = qkv_pool.tile([P, 2, N_ST, 2 * P], F8, tag="gt")
                else:
                    gt8_s = qkv_pool.tile([P, N_DT, S], F8, tag="gt")

                def make_dma(sc):
                    def th():
                        dma_x_chunk(b, sc, xnat_s)

                    return th

                dma_th = [
                    None if loaded[sc] else make_dma(sc) for sc in range(N_SC)
                ]

                def make_trans_unit(sc, t_i):
                    def th():
                        st = sc * 4 + t_i
                        tp = gp_psum.tile([P, N_DT * P], F32, tag="gp")
                        for dt_i in range(N_DT):
                            nc.tensor.matmul(
                                tp[:, dt_i * P : (dt_i + 1) * P],
                                xnat_s[:, st, dt_i * P : (dt_i + 1) * P],
                                ident[:],
                                start=True,
                                stop=True,
                                skip_group_check=True,
                            )
                        nc.vector.tensor_copy(
                            xt8_s[:, :, st * P : (st + 1) * P],
                            tp[:].rearrange("p (t c) -> p t c", t=N_DT),
                        )

                    return th

                trans_th = [
                    [make_trans_unit(sc, t_i) for t_i in range(4)]
                    for sc in range(N_SC)
                ]

                def make_g(sc, ct):
                    def th():
                        mp = gp_psum.tile([P, NCH], F32, tag="gp")
                        for j in range(2):
                            nc.tensor.matmul(
                                mp[:],
                                m8_s[:, 2 * j : 2 * j + 2, ct * P : (ct + 1) * P],
                                xt8_s[:, 2 * j : 2 * j + 2, sc * NCH : (sc + 1) * NCH],
                                start=(j == 0),
                                stop=(j == 1),
                                perf_mode=DR,
                            )
                        if USE_SWI:
                            nc.vector.tensor_copy(
                                gt8_s[
                                    :,
                                    ct // 2,
                                    sc * 4 : (sc + 1) * 4,
                                    (ct % 2) :: 2,
                                ],
                                mp[:].rearrange("p (b q) -> p b q", b=4),
                            )
                        else:
                            nc.vector.tensor_copy(
                                gt8_s[:, ct, sc * NCH : (sc + 1) * NCH], mp[:]
                            )

                    return th

                kq_th = [
                    [make_g(sc, ct) for ct in range(N_DT)]
                    for sc in range(N_SC)
                ]
                return (gt8_s, xt8_s), dma_th, trans_th, kq_th

            def scores_stationary(gt8_s, j, qt):
                if USE_SWI:
                    return gt8_s[:, j, qt, :]
                return gt8_s[:, 2 * j : 2 * j + 2, qt * P : (qt + 1) * P]

            SC_MODE = SWI if USE_SWI else DR

            def emit_scores_half(gt8_s, xt8_s, qt, h, e2_t, rs_t):
                """One half-row pass for one q-tile: j-major into a [P,2,512]
                two-bank PSUM tile, one 1024-wide exp (fp8 out, offset
                -C_OFF), row-sum via accum_out into rs_t[:, h]."""
                par = qt % 2
                sp = sc_psum.tile([P, 2, NCH], F32, tag="sc")
                for j in range(2):
                    for i in range(2):
                        kc = 2 * h + i
                        nc.tensor.matmul(
                            sp[:, i, :],
                            scores_stationary(gt8_s, j, qt),
                            xt8_s[:, 2 * j : 2 * j + 2, kc * NCH : (kc + 1) * NCH],
                            start=(j == 0),
                            stop=(j == 1),
                            perf_mode=SC_MODE,
                        )
                nc.scalar.activation(
                    e2_t[:, par, h * 2 * NCH : (h + 1) * 2 * NCH],
                    sp[:].rearrange("p a b -> p (a b)"),
                    mybir.ActivationFunctionType.Exp,
                    scale=SCALE / KM,
                    bias=negc_t[:],
                    accum_out=rs_t[:, h : h + 1],
                )

            def emit_r(qt, rs_t, r2_t):
                """r = KR / (rowsumA + rowsumB), broadcast into the qt%2 lane
                of the pair's (interleaved) stationary tile."""
                par = qt % 2
                rtot = soft_pool.tile([P, 1], F32, tag="rtot")
                nc.vector.reduce_sum(rtot[:], rs_t[:], axis=mybir.AxisListType.X)
                rtot_s = soft_pool.tile([P, 1], F32, tag="rtots")
                nc.vector.tensor_scalar_mul(rtot_s[:], rtot[:], 1.0 / KR)
                rrec = soft_pool.tile([P, 1], F32, tag="rrec")
                nc.vector.reciprocal(rrec[:], rtot_s[:])
                if USE_SWI:
                    dst = r2_t[:, par::2]
                else:
                    dst = r2_t[:, par, :]
                nc.vector.tensor_copy(dst, rrec[:, 0:1].broadcast_to([P, P]))

            def colsum_stationary(r2_t):
                if USE_SWI:
                    return r2_t[:, :]
                return r2_t[:, 0:2, :]

            def emit_colsum_pair(w_ps, e2_t, r2_t, pair, kcs, w_off):
                for kc in kcs:
                    nc.tensor.matmul(
                        w_ps[:, kc - w_off, :],
                        colsum_stationary(r2_t),
                        e2_t[:, 0:2, kc * NCH : (kc + 1) * NCH],
                        start=(pair == 0),
                        stop=(pair == N_PAIR - 1),
                        perf_mode=SC_MODE,
                        skip_group_check=True,
                    )

            def phase_A(gt8_s, xt8_s, e2_list, rs_list, extras):
                """Key chunks {0,1} for all 16 q-tiles."""
                for qt in range(N_ST):
                    if qt % 2 == 0:
                        e2_t = e_pool.tile([P, 2, S], F8, tag="e2")
                        e2_list.append(e2_t)
                    rs_t = rs_pool.tile([P, 2], F32, tag="rs")
                    rs_list.append(rs_t)
                    emit_scores_half(gt8_s, xt8_s, qt, 0, e2_list[qt // 2], rs_t)
                    for th in extras[qt]:
                        th()

            def phase_B(gt8_s, xt8_s, e2_list, rs_list, extras):
                """Key chunks {2,3}, r pipeline, inline colsum kc{0,1}
                (2-pair lag, lazy w tile), deferred kc{2,3} sweep thunks."""
                w_holder = {}

                def get_wps():
                    if "a" not in w_holder:
                        w_ps_a = w_psum.tile([P, 2, NCH], F32, tag="w")
                        w_holder["a"] = w_ps_a
                    return w_holder["a"]

                r2_list = []
                pending = []
                for qt in range(N_ST):
                    pair = qt // 2
                    if qt % 2 == 0:
                        if USE_SWI:
                            r2_t = r2_pool.tile([P, 2 * P], F8, tag="r2")
                        else:
                            r2_t = r2_pool.tile([P, 2, P], F8, tag="r2")
                        r2_list.append(r2_t)
                    emit_scores_half(gt8_s, xt8_s, qt, 1, e2_list[pair], rs_list[qt])
                    emit_r(qt, rs_list[qt], r2_list[pair])
                    if qt % 2 == 1:
                        pending.append(pair)
                        if len(pending) > 2:
                            p = pending.pop(0)
                            emit_colsum_pair(
                                get_wps(), e2_list[p], r2_list[p], p, (0, 1), 0
                            )
                    for th in extras[qt]:
                        th()
                for p in pending:
                    emit_colsum_pair(get_wps(), e2_list[p], r2_list[p], p, (0, 1), 0)

                sweep_holder = {}

                def make_sweep_pair(pair):
                    def th():
                        if "b" not in sweep_holder:
                            w_ps_b = w_psum.tile([P, 2, NCH], F32, tag="w")
                            sweep_holder["b"] = w_ps_b
                        emit_colsum_pair(
                            sweep_holder["b"],
                            e2_list[pair],
                            r2_list[pair],
                            pair,
                            (2, 3),
                            2,
                        )

                    return th

                sweep_th = [make_sweep_pair(p) for p in range(N_PAIR)]
                return w_holder["a"], sweep_holder, sweep_th

            def final_thunks(b, w_ps_a, sweep_holder, sweep_th, xnat_s, y_in_w=False):
                """Colsum sweep kc{2,3} + w-phase: out = (w @ X) @ W_v; the
                KR pre-scale is folded into the final o_sb copy.  y_in_w puts
                the y accumulator in the (by then free) w PSUM pool so both
                gp buffers rotate the row->column transposes — without it the
                exposed tail chain serializes at ~560ns/step through one gp
                buffer.  (Only legal for the LAST batch: it adds a w-pool
                allocation.)"""
                w_sb = wvec_pool.tile([1, S], BF16, tag="wsb")
                y_holder = {}

                def get_yps():
                    if "mm" not in y_holder:
                        if y_in_w:
                            y_ps_w = w_psum.tile([P, 2, NCH], F32, tag="w")
                            y_holder["mm"] = y_ps_w[:, 0, :]
                            y_holder["row"] = y_ps_w[0:1, 0, :]
                        else:
                            y_ps_g = gp_psum.tile([P, NCH], F32, tag="gp")
                            y_holder["mm"] = y_ps_g[:]
                            y_holder["row"] = y_ps_g[0:1, :]
                    return y_holder
                wt_pads = {}
                yt_pads = {}
                thunks = []

                def make_wcopy(kc):
                    def th():
                        src = w_ps_a if kc < 2 else sweep_holder["b"]
                        nc.vector.tensor_copy(
                            w_sb[:, kc * NCH : (kc + 1) * NCH],
                            src[0:1, kc % 2, :],
                        )

                    return th

                def row_to_bcast_cols(src_row, pads, key, tag):
                    tp = gp_psum.tile([P, 1], F32, tag="gp")
                    nc.tensor.matmul(
                        tp[:], src_row, one_t[0:1, 0:1], start=True, stop=True
                    )
                    pad = wvec_pool.tile([P, P], BF16, tag=tag)
                    nc.vector.tensor_copy(pad[:], tp[:, 0:1].broadcast_to([P, P]))
                    pads[key] = pad

                def make_wtrans(kt):
                    def th():
                        row_to_bcast_cols(
                            w_sb[0:1, kt * P : (kt + 1) * P],
                            wt_pads, kt, f"wtp{kt % 4}",
                        )

                    return th

                def make_ymm(st):
                    def th():
                        nc.tensor.matmul(
                            get_yps()["mm"],
                            wt_pads[st][:],
                            xnat_s[:, st, :],
                            start=(st == 0),
                            stop=(st == N_ST - 1),
                            skip_group_check=True,
                        )

                    return th

                def epilogue_th():
                    y_sb = wvec_pool.tile([1, NCH], BF16, tag="ysb")
                    nc.vector.tensor_copy(y_sb[:], get_yps()["row"])
                    o_ps = gp_psum.tile([P, NCH], F32, tag="gp")
                    for c in range(N_DT):
                        row_to_bcast_cols(
                            y_sb[0:1, c * P : (c + 1) * P], yt_pads, c, f"ytp{c}"
                        )
                    for c in range(N_DT):
                        nc.tensor.matmul(
                            o_ps[:],
                            yt_pads[c][:],
                            wv_s[:, c, :],
                            start=(c == 0),
                            stop=(c == N_DT - 1),
                            skip_group_check=True,
                        )
                    o_sb = wvec_pool.tile([1, NCH], F32, tag="osb")
                    nc.vector.tensor_scalar_mul(o_sb[:], o_ps[0:1, :], 1.0 / KR)
                    nc.sync.dma_start(out=out_ext[b : b + 1, :], in_=o_sb[:])

                thunks.append(make_wcopy(0))
                thunks.append(make_wcopy(1))
                thunks.extend(sweep_th)
                thunks.append(make_wcopy(2))
                thunks.append(make_wcopy(3))
                for kt in range(N_ST):
                    thunks.append(make_wtrans(kt))
                    if kt >= 3:
                        thunks.append(make_ymm(kt - 3))
                for st in range(N_ST - 3, N_ST):
                    thunks.append(make_ymm(st))
                thunks.append(epilogue_th)
                return thunks

            def spread(thunks, n_slots):
                slots = [[] for _ in range(n_slots)]
                k = len(thunks)
                for i, th in enumerate(thunks):
                    slots[min(i * n_slots // k, n_slots - 1)].append(th)
                return slots

            # ------------------------- emission ------------------------------

            # FILL: s-tile 0 transpose (f32 path), s-tiles 1-3 transposes,
            # M prework, G s-chunk 0 — just enough for phase A0's q-tile 0.
            h0, dma0, trans0, kq0 = proj_thunks(0, xnat0_s, x0_loaded)
            g0, xt0 = h0

            def first_tile_trans_f32():
                tp = gp_psum.tile([P, N_DT * P], F32, tag="gp")
                for dt_i in range(N_DT):
                    nc.tensor.matmul(
                        tp[:, dt_i * P : (dt_i + 1) * P],
                        xf0[:, dt_i * P : (dt_i + 1) * P],
                        ident_f[:],
                        start=True,
                        stop=True,
                        skip_group_check=True,
                    )
                nc.vector.tensor_copy(
                    xt0[:, :, 0:P],
                    tp[:].rearrange("p (t c) -> p t c", t=N_DT),
                )

            first_tile_trans_f32()
            pre_th = m_prework_thunks()
            # interleave prework (ACT copies) with c0 transposes (DVE copies);
            # a dummy matmul after each early unit keeps the copy-bound fill
            # phase's PE stream gapless so the clock warms by ~4.5us
            fill_stream = []
            fill_stream.extend(pre_th[:4])  # wkT units
            fill_stream.extend(trans0[0][1:])  # s-tiles 1-3
            fill_stream.extend(pre_th[4:8])  # wqT units
            fill_stream.extend(trans0[1])  # s-tiles 4-7 (phase A needs kc1)
            fill_stream.extend(pre_th[8:])  # M groups
            fill_stream.extend(kq0[0])  # G s-chunk 0
            for i, th in enumerate(fill_stream):
                th()
                if i < 14:
                    emit_warm_dummy(1)

            # batch 1 proj thunks (woven into B0; all DMAs already queued)
            h1, dma1, trans1, kq1 = proj_thunks(1, xnat1_s, [True] * N_SC)
            g1, xt1 = h1

            # --- phase A0: extras = batch0's remaining transposes/G.
            # Deadlines: G sc1 before qt4, sc2 before qt8, sc3 before qt12;
            # trans c2/c3 before phase B0.
            slots_a0 = [[] for _ in range(N_ST)]
            slots_a0[0] = [kq0[1][0], kq0[1][1]]
            slots_a0[1] = [kq0[1][2], kq0[1][3]]
            slots_a0[2] = [trans0[2][0], trans0[2][1]]
            slots_a0[3] = [trans0[2][2], trans0[2][3]]
            slots_a0[4] = [kq0[2][0], kq0[2][1]]
            slots_a0[5] = [kq0[2][2], kq0[2][3]]
            slots_a0[6] = [trans0[3][0], trans0[3][1]]
            slots_a0[7] = [trans0[3][2], trans0[3][3]]
            slots_a0[8] = [kq0[3][0], kq0[3][1]]
            slots_a0[9] = [kq0[3][2], kq0[3][3]]

            e2_0, rs_0 = [], []
            phase_A(g0, xt0, e2_0, rs_0, slots_a0)

            # --- phase B0: extras = batch1's projection.
            proj1_flat = []
            for sc in range(N_SC):
                proj1_flat.extend(trans1[sc])
                proj1_flat.extend(kq1[sc])
            wa0, swh0, swth0 = phase_B(
                g0, xt0, e2_0, rs_0, spread(proj1_flat, N_ST)
            )

            # --- phase A1: extras = batch0's colsum sweep + w-phase.  The
            # sweep matmuls go ONE PAIR PER SLOT — a solid block would park
            # 16 matmuls ahead of the scores stream in the in-order PE queue
            # and starve ACT for ~3.5us.
            fin0 = final_thunks(0, wa0, swh0, swth0, xnat0_s)
            slots_a1 = [[] for _ in range(N_ST)]
            slots_a1[0] = fin0[0:3]  # wcopy0, wcopy1, sweep pair0
            for p in range(1, N_PAIR):
                slots_a1[p] = [fin0[2 + p]]  # sweep pair p
            slots_a1[8].extend(fin0[10:12])  # wcopy2, wcopy3
            rest = fin0[12:]
            k = len(rest)
            for i, th in enumerate(rest):
                slots_a1[8 + min(i * 8 // k, 7)].append(th)
            e2_1, rs_1 = [], []
            phase_A(g1, xt1, e2_1, rs_1, slots_a1)

            # --- phase B1: no extras.
            wa1, swh1, swth1 = phase_B(
                g1, xt1, e2_1, rs_1, [[] for _ in range(N_ST)]
            )

            # --- tail: batch1's sweep + w-phase (y in the free w banks).
            for th in final_thunks(1, wa1, swh1, swth1, xnat1_s, y_in_w=True):
                th()

    nc.compile()
    return nc


_NC_CACHE = None


def _get_nc():
    global _NC_CACHE
    if _NC_CACHE is None:
        _NC_CACHE = build_nc()
    return _NC_CACHE


def make_in_maps(inputs, W_q, W_k, W_v):
    inputs = np.ascontiguousarray(np.asarray(inputs, dtype=np.float32))
    W_q = np.ascontiguousarray(np.asarray(W_q, dtype=np.float32))
    W_k = np.ascontiguousarray(np.asarray(W_k, dtype=np.float32))
    W_v = np.ascontiguousarray(np.asarray(W_v, dtype=np.float32))
    return [
        {
            "inputs": inputs[i * B_PER_CORE : (i + 1) * B_PER_CORE],
            "W_q": W_q,
            "W_k": W_k,
            "W_v": W_v,
        }
        for i in range(N_CORES)
    ]


def kernel(**inputs) -> np.ndarray:
    nc = _get_nc()
    in_maps = make_in_maps(
        inputs["inputs"], inputs["W_q"], inputs["W_k"], inputs["W_v"]
    )
    res = run_bass_kernel_spmd(nc, in_maps, core_ids=list(range(N_CORES)))
    return np.concatenate(
        [res.results[i]["out"] for i in range(N_CORES)], axis=0
    ).astype(np.float32)


# revision 40
# speedup vs baseline: 1.5447x; 1.0250x over previous
"""Trainium2 Bass kernel for batched single-head attention with seq-sum pooling.

Reference computation (B=16, S=2048, D=512, fp32):
    q = x @ W_q ; k = x @ W_k ; v = x @ W_v          per batch  [S, D]
    scores = q @ k.T / sqrt(D)                        [S, S]
    attn = softmax(scores, axis=-1)
    out_b = sum_s (attn @ v)[s, :]                    [D]

Algebraic restructures:
1. The final sum over query positions commutes through both trailing
   matmuls: out_b = ((r^T E) @ x) @ W_v with E = exp(scores/sqrt(D)) and
   r[q] = 1/rowsum_q(E) — removes the [S,S]x[S,D] attention-value matmul
   AND the V projection.
2. scores = x M x^T with M = W_q W_k^T computed once per core — one
   G = x M projection replaces both per-batch Q/K projections.

fp8: the G projection, scores, and softmax column-sum matmuls run with
float8e4 operands in DoubleRow mode (K=256/instruction).  Exact foldings
keep fp8 in range: M stored as 16*M, E = exp(s/sqrt(D) - 2.5) (global
offset cancels through the softmax), r stored as 128*r (folded into the
final output copy).  Measured rel err 7.5e-3 (tolerance 2e-2).

Scores/colsum stationaries are stored PRE-INTERLEAVED for
DoubleRowSwInterleave: the HW weight load then reads contiguously instead
of DoubleRow's strided interleave (which disables fast-weight-load and
makes LDWEIGHTS the ~260ns/matmul bottleneck).  The interleave's column
reversal permutes scores rows (q) within each 128-block — harmless, since
every consumer (exp, row-sum, r broadcast, weighted column-sum) is
q-order-free, and all per-q tensors carry the same permutation.  The G
matmul keeps plain DoubleRow (a reversed G would misalign the scores
contraction).

Schedule (from trace analysis: the PE re-throttles 2.4->1.2GHz on idle
windows, so density is everything): each batch's score rows are computed in
two half-row passes — phase A covers key chunks {0,1}, phase B {2,3} —
which lets phase A start after only 4 transposes + 4 G chunks (~6us) and
hides the entire 8MB x DMA under compute.  Row sums accumulate per-half
via exp's accum_out into persistent per-q-tile tiles; r and the colsums
happen in phase B.  Colsum kc{0,1} accumulates inline (2-pair lag);
kc{2,3} runs as a deferred sweep.  Weave plan: batch0's remaining
transposes/G chunks fill phase A0's slack; batch1's projection fills B0;
batch0's sweep+w-phase fills A1; the only serial tails are ~8us of fill
and batch1's w-phase.  PSUM: sc 2x[P,2,512] + gp 2x1 + w 2 banks = 8.

Sharding: pure data parallelism over batch — 2 batch elements per core on
8 NeuronCores, weights replicated, no collectives.
"""

import sys

sys.path.insert(0, "/opt/trn_rl_repo")

import numpy as np

import concourse.bass as bass
import concourse.mybir as mybir
import concourse.tile as tile
from concourse import bacc
from concourse.bass_utils import run_bass_kernel_spmd
from concourse.masks import make_identity

B, S, D = 16, 2048, 512
P = 128
N_CORES = 8
B_PER_CORE = B // N_CORES  # 2
SCALE = 1.0 / float(np.sqrt(D))
KM = 16.0  # M pre-scale (exact power of 2)
KR = 128.0  # r pre-scale (exact power of 2)
C_OFF = 2.5  # global exp offset; cancels through softmax normalization

F32 = mybir.dt.float32
BF16 = mybir.dt.bfloat16
F8 = mybir.dt.float8e4
DR = mybir.MatmulPerfMode.DoubleRow
SWI = mybir.MatmulPerfMode.DoubleRowSwInterleave
USE_SWI = True  # pre-interleaved scores/colsum stationaries

N_ST = S // P  # 16 s-tiles
N_DT = D // P  # 4 d-tiles
NCH = 512  # moving free dim per matmul (one fp32 PSUM bank)
N_SC = S // NCH  # 4 s-chunks
N_KC = S // NCH  # 4 k-chunks
N_PAIR = N_ST // 2  # 8 q-tile pairs


def build_nc():
    nc = bacc.Bacc("TRN2", target_bir_lowering=False, debug=False, num_devices=N_CORES)
    x_ext = nc.dram_tensor(
        "inputs", [B_PER_CORE, S, D], F32, kind="ExternalInput"
    ).ap()
    wq_ext = nc.dram_tensor("W_q", [D, D], F32, kind="ExternalInput").ap()
    wk_ext = nc.dram_tensor("W_k", [D, D], F32, kind="ExternalInput").ap()
    wv_ext = nc.dram_tensor("W_v", [D, D], F32, kind="ExternalInput").ap()
    out_ext = nc.dram_tensor("out", [B_PER_CORE, D], F32, kind="ExternalOutput").ap()

    with tile.TileContext(nc) as tc:
        with (
            tc.tile_pool(name="const", bufs=1) as const_pool,
            tc.tile_pool(name="w", bufs=1) as w_pool,
            tc.tile_pool(name="xnat", bufs=2) as xnat_pool,
            tc.tile_pool(name="xt", bufs=2) as xt_pool,
            tc.tile_pool(name="qkv", bufs=2) as qkv_pool,
            tc.tile_pool(name="e", bufs=17) as e_pool,
            tc.tile_pool(name="soft", bufs=4) as soft_pool,
            tc.tile_pool(name="rs", bufs=36) as rs_pool,
            tc.tile_pool(name="r2", bufs=18) as r2_pool,
            tc.tile_pool(name="wvec", bufs=2) as wvec_pool,
            tc.tile_pool(name="scps", bufs=2, space="PSUM") as sc_psum,
            tc.tile_pool(name="gpps", bufs=2, space="PSUM") as gp_psum,
            tc.tile_pool(name="wps", bufs=1, space="PSUM") as w_psum,
        ):
            one_t = const_pool.tile([1, 1], BF16)
            nc.gpsimd.memset(one_t[:], 1.0)
            ident_f = const_pool.tile([P, P], F32)
            make_identity(nc, ident_f[:])
            ident = const_pool.tile([P, P], BF16)
            nc.vector.tensor_copy(ident[:], ident_f[:])
            negc_t = const_pool.tile([P, 1], F32)
            nc.gpsimd.memset(negc_t[:], -C_OFF)

            # HAM warmup: the PE clock boots throttled to 1.2GHz and only
            # un-throttles to 2.4GHz after ~3.4us of sustained matmul
            # activity (one full busy window of the hardware activity
            # monitor) — and a DoubleRow/fp8 stream SUSTAINS the warm state
            # but was never observed to CREATE it.  Dependency-free bf16
            # dummy matmuls are woven BETWEEN the fill-phase units (whose
            # pace is copy-bound, leaving PE gaps) so the PE stream is
            # gapless from t~1us without delaying real work.
            warm_mov = const_pool.tile([P, NCH], BF16)
            nc.gpsimd.memset(warm_mov[:], 0.0)
            warm_ps = sc_psum.tile([P, 2, NCH], F32, tag="sc")
            warm_i = [0]

            def emit_warm_dummy(n=1):
                for _ in range(n):
                    nc.tensor.matmul(
                        warm_ps[:, warm_i[0] % 2, :],
                        ident[:],
                        warm_mov[:],
                        start=True,
                        stop=True,
                        skip_group_check=True,
                    )
                    warm_i[0] += 1

            emit_warm_dummy(5)

            def dma_x_chunk(b, sc, xnat_s):
                nc.gpsimd.dma_start(
                    out=xnat_s[:, sc * 4 : (sc + 1) * 4, :],
                    in_=x_ext[b, sc * NCH : (sc + 1) * NCH, :].rearrange(
                        "(t p) d -> p t d", p=P
                    ),
                )

            w_tiles = {}

            def dma_w(name, ext):
                w_s = w_pool.tile([P, N_DT, D], BF16, tag=name)
                nc.gpsimd.dma_start(
                    out=w_s[:], in_=ext.rearrange("(t p) e -> p t e", p=P)
                )
                w_tiles[name] = w_s

            # DMA plan.  One SWDGE dma_start of ~1MB costs ~6us and the queue
            # is FIFO, so the x chunks monopolize it: s1-3, then batch0
            # chunks 1-3, then ALL of batch1's chunks, then wv — each landing
            # just ahead of its consumer phase.  The weights ride the two
            # parallel HWDGE queues as plain f32 (HWDGE can't cast; the
            # prework transposes consume f32 directly), so M prework starts
            # at ~4us instead of ~12.  s-tile 0 is f32 on the sync queue.
            xnat0_s = xnat_pool.tile([P, N_ST, D], BF16, tag="xnat")
            xnat1_s = xnat_pool.tile([P, N_ST, D], BF16, tag="xnat")
            xf0 = xnat_pool.tile([P, D], F32, tag="xf0")
            wkf_s = w_pool.tile([P, N_DT, D], F32, tag="wkf")
            wqf_s = w_pool.tile([P, N_DT, D], F32, tag="wqf")
            nc.scalar.dma_start(
                out=wkf_s[:], in_=wk_ext.rearrange("(t p) e -> p t e", p=P)
            )
            nc.sync.dma_start(out=xf0[:], in_=x_ext[0, 0:P, :])
            nc.sync.dma_start(
                out=wqf_s[:], in_=wq_ext.rearrange("(t p) e -> p t e", p=P)
            )
            nc.vector.tensor_copy(xnat0_s[:, 0, :], xf0[:])
            nc.gpsimd.dma_start(
                out=xnat0_s[:, 1:4, :],
                in_=x_ext[0, P:NCH, :].rearrange("(t p) d -> p t d", p=P),
            )
            dma_x_chunk(0, 1, xnat0_s)
            dma_x_chunk(0, 2, xnat0_s)
            dma_x_chunk(0, 3, xnat0_s)
            for sc in range(N_SC):
                dma_x_chunk(1, sc, xnat1_s)
            dma_w("wv", wv_ext)
            x0_loaded = [True] * N_SC
            wv_s = w_tiles["wv"]

            # One-time prework: M = Wq Wk^T, stored as 16*M fp8 (raw entries
            # would be fp8-subnormal; the exp scale divides the 16 out).
            # wqT scaled by 16 at its ACT copy; wkT copies also on ACT so the
            # fill phase's DVE stays on the x transposes.
            wqT_s = w_pool.tile([P, N_DT, D], BF16, tag="wqT")
            wkT_s = w_pool.tile([P, N_DT, D], BF16, tag="wkT")
            m8_s = w_pool.tile([P, N_DT, D], F8, tag="m8")

            # During the fill the sc and w PSUM pools are idle; cycling the
            # fill units across all three pools gives 4-5 concurrent
            # unit-copy lanes instead of serializing ~20 copies through gp's
            # two banks.
            fill_pools = [gp_psum, sc_psum, gp_psum, w_psum]
            fill_pi = [0]

            def next_fill_pool():
                p = fill_pools[fill_pi[0] % len(fill_pools)]
                fill_pi[0] += 1
                return p

            a0_pools = [gp_psum, w_psum]
            a0_pi = [0]

            def pool_for(ctx):
                # "fill": sc/w pools are idle -> 4 lanes.  "a0": scores own
                # sc, colsum hasn't started -> gp + w.  "b0": only gp free.
                if ctx == "fill":
                    return next_fill_pool()
                if ctx == "a0":
                    p = a0_pools[a0_pi[0] % 2]
                    a0_pi[0] += 1
                    return p
                return gp_psum

            def pool_tag(pool):
                # reuse each pool's canonical tag: a new tag would get its
                # own buffer ring and blow the 8-bank PSUM budget
                if pool is sc_psum:
                    return "sc"
                if pool is w_psum:
                    return "w"
                return "gp"

            def m_prework_thunks():
                thunks = []

                def make_wtrans_unit(src_w, dst, t_e, scale):
                    def th():
                        pool = next_fill_pool()
                        tp = pool.tile([P, N_DT * P], F32, tag=pool_tag(pool))
                        for t_a in range(N_DT):
                            nc.tensor.matmul(
                                tp[:, t_a * P : (t_a + 1) * P],
                                src_w[:, t_a, t_e * P : (t_e + 1) * P],
                                ident_f[:],
                                start=True,
                                stop=True,
                                skip_group_check=True,
                            )
                        # wkT on ACT, wqT (scaled) on DVE: two parallel
                        # copy chains for the prework
                        if scale is None:
                            nc.scalar.copy(dst[:, t_e, :], tp[:])
                        else:
                            nc.vector.tensor_scalar_mul(dst[:, t_e, :], tp[:], scale)

                    return th

                def make_m_group(t_a):
                    def th():
                        pool = next_fill_pool()
                        mp = pool.tile([P, NCH], F32, tag=pool_tag(pool))
                        for t_e in range(N_DT):
                            nc.tensor.matmul(
                                mp[:],
                                wqT_s[:, t_e, t_a * P : (t_a + 1) * P],
                                wkT_s[:, t_e, :],
                                start=(t_e == 0),
                                stop=(t_e == N_DT - 1),
                            )
                        nc.scalar.copy(m8_s[:, t_a, :], mp[:])

                    return th

                for t_e in range(N_DT):
                    thunks.append(make_wtrans_unit(wkf_s, wkT_s, t_e, None))
                for t_e in range(N_DT):
                    thunks.append(make_wtrans_unit(wqf_s, wqT_s, t_e, KM))
                for t_a in range(N_DT):
                    thunks.append(make_m_group(t_a))
                return thunks

            # ---------- thunk builders --------------------------------------

            def proj_thunks(b, xnat_s, loaded, unit_ctx=None):
                """Transpose + G = X M projection thunks for batch b.  xt8 is
                [P, dtile, S] fp8.  gt8 layout depends on USE_SWI:
                  - SWI: [P, jpair, qt_block, 2*P] with the two d-subtiles of
                    a jpair interleaved along the last dim (stored UNreversed;
                    the HW's column reversal permutes q within blocks, which
                    every downstream consumer absorbs).
                  - plain DR: [P, dtile, S]."""
                xt8_s = xt_pool.tile([P, N_DT, S], F8, tag="xt")
                if USE_SWI:
                    gt8_s = qkv_pool.tile([P, 2, N_ST, 2 * P], F8, tag="gt")
                else:
                    gt8_s = qkv_pool.tile([P, N_DT, S], F8, tag="gt")

                def make_dma(sc):
                    def th():
                        dma_x_chunk(b, sc, xnat_s)

                    return th

                dma_th = [
                    None if loaded[sc] else make_dma(sc) for sc in range(N_SC)
                ]

                def make_trans_unit(sc, t_i):
                    def th():
                        st = sc * 4 + t_i
                        pool = pool_for(unit_ctx[sc] if unit_ctx else "b0")
                        tp = pool.tile([P, N_DT * P], F32, tag=pool_tag(pool))
                        for dt_i in range(N_DT):
                            nc.tensor.matmul(
                                tp[:, dt_i * P : (dt_i + 1) * P],
                                xnat_s[:, st, dt_i * P : (dt_i + 1) * P],
                                ident[:],
                                start=True,
                                stop=True,
                                skip_group_check=True,
                            )
                        nc.vector.tensor_copy(
                            xt8_s[:, :, st * P : (st + 1) * P],
                            tp[:].rearrange("p (t c) -> p t c", t=N_DT),
                        )

                    return th

                trans_th = [
                    [make_trans_unit(sc, t_i) for t_i in range(4)]
                    for sc in range(N_SC)
                ]

                def make_g(sc, ct):
                    def th():
                        pool = pool_for(unit_ctx[sc] if unit_ctx else "b0")
                        mp = pool.tile([P, NCH], F32, tag=pool_tag(pool))
                        for j in range(2):
                            nc.tensor.matmul(
                                mp[:],
                                m8_s[:, 2 * j : 2 * j + 2, ct * P : (ct + 1) * P],
                                xt8_s[:, 2 * j : 2 * j + 2, sc * NCH : (sc + 1) * NCH],
                                start=(j == 0),
                                stop=(j == 1),
                                perf_mode=DR,
                            )
                        if USE_SWI:
                            nc.vector.tensor_copy(
                                gt8_s[
                                    :,
                                    ct // 2,
                                    sc * 4 : (sc + 1) * 4,
                                    (ct % 2) :: 2,
                                ],
                                mp[:].rearrange("p (b q) -> p b q", b=4),
                            )
                        else:
                            nc.vector.tensor_copy(
                                gt8_s[:, ct, sc * NCH : (sc + 1) * NCH], mp[:]
                            )

                    return th

                kq_th = [
                    [make_g(sc, ct) for ct in range(N_DT)]
                    for sc in range(N_SC)
                ]
                return (gt8_s, xt8_s), dma_th, trans_th, kq_th

            def scores_stationary(gt8_s, j, qt):
                if USE_SWI:
                    return gt8_s[:, j, qt, :]
                return gt8_s[:, 2 * j : 2 * j + 2, qt * P : (qt + 1) * P]

            SC_MODE = SWI if USE_SWI else DR

            def emit_scores_half(gt8_s, xt8_s, qt, h, e2_t, rs_t):
                """One half-row pass for one q-tile: j-major into a [P,2,512]
                two-bank PSUM tile, one 1024-wide exp (fp8 out, offset
                -C_OFF), row-sum via accum_out into rs_t[:, h]."""
                par = qt % 2
                sp = sc_psum.tile([P, 2, NCH], F32, tag="sc")
                for j in range(2):
                    for i in range(2):
                        kc = 2 * h + i
                        nc.tensor.matmul(
                            sp[:, i, :],
                            scores_stationary(gt8_s, j, qt),
                            xt8_s[:, 2 * j : 2 * j + 2, kc * NCH : (kc + 1) * NCH],
                            start=(j == 0),
                            stop=(j == 1),
                            perf_mode=SC_MODE,
                        )
                nc.scalar.activation(
                    e2_t[:, par, h * 2 * NCH : (h + 1) * 2 * NCH],
                    sp[:].rearrange("p a b -> p (a b)"),
                    mybir.ActivationFunctionType.Exp,
                    scale=SCALE / KM,
                    bias=negc_t[:],
                    accum_out=rs_t[:, h : h + 1],
                )

            def emit_r(qt, rs_t, r2_t):
                """r = KR / (rowsumA + rowsumB), broadcast into the qt%2 lane
                of the pair's (interleaved) stationary tile."""
                par = qt % 2
                rtot = soft_pool.tile([P, 1], F32, tag="rtot")
                nc.vector.reduce_sum(rtot[:], rs_t[:], axis=mybir.AxisListType.X)
                rtot_s = soft_pool.tile([P, 1], F32, tag="rtots")
                nc.vector.tensor_scalar_mul(rtot_s[:], rtot[:], 1.0 / KR)
                rrec = soft_pool.tile([P, 1], F32, tag="rrec")
                nc.vector.reciprocal(rrec[:], rtot_s[:])
                if USE_SWI:
                    dst = r2_t[:, par::2]
                else:
                    dst = r2_t[:, par, :]
                nc.vector.tensor_copy(dst, rrec[:, 0:1].broadcast_to([P, P]))

            def colsum_stationary(r2_t):
                if USE_SWI:
                    return r2_t[:, :]
                return r2_t[:, 0:2, :]

            def emit_colsum_pair(w_ps, e2_t, r2_t, pair, kcs, w_off):
                for kc in kcs:
                    nc.tensor.matmul(
                        w_ps[:, kc - w_off, :],
                        colsum_stationary(r2_t),
                        e2_t[:, 0:2, kc * NCH : (kc + 1) * NCH],
                        start=(pair == 0),
                        stop=(pair == N_PAIR - 1),
                        perf_mode=SC_MODE,
                        skip_group_check=True,
                    )

            def phase_A(gt8_s, xt8_s, e2_list, rs_list, extras):
                """Key chunks {0,1} for all 16 q-tiles."""
                for qt in range(N_ST):
                    if qt % 2 == 0:
                        e2_t = e_pool.tile([P, 2, S], F8, tag="e2")
                        e2_list.append(e2_t)
                    rs_t = rs_pool.tile([P, 2], F32, tag="rs")
                    rs_list.append(rs_t)
                    emit_scores_half(gt8_s, xt8_s, qt, 0, e2_list[qt // 2], rs_t)
                    for th in extras[qt]:
                        th()

            def phase_B(gt8_s, xt8_s, e2_list, rs_list, extras):
                """Key chunks {2,3}, r pipeline, inline colsum kc{0,1}
                (2-pair lag, lazy w tile), deferred kc{2,3} sweep thunks."""
                w_holder = {}

                def get_wps():
                    if "a" not in w_holder:
                        w_ps_a = w_psum.tile([P, 2, NCH], F32, tag="w")
                        w_holder["a"] = w_ps_a
                    return w_holder["a"]

                r2_list = []
                pending = []
                for qt in range(N_ST):
                    pair = qt // 2
                    if qt % 2 == 0:
                        if USE_SWI:
                            r2_t = r2_pool.tile([P, 2 * P], F8, tag="r2")
                        else:
                            r2_t = r2_pool.tile([P, 2, P], F8, tag="r2")
                        r2_list.append(r2_t)
                    emit_scores_half(gt8_s, xt8_s, qt, 1, e2_list[pair], rs_list[qt])
                    emit_r(qt, rs_list[qt], r2_list[pair])
                    if qt % 2 == 1:
                        pending.append(pair)
                        if len(pending) > 2:
                            p = pending.pop(0)
                            emit_colsum_pair(
                                get_wps(), e2_list[p], r2_list[p], p, (0, 1), 0
                            )
                    for th in extras[qt]:
                        th()
                for p in pending:
                    emit_colsum_pair(get_wps(), e2_list[p], r2_list[p], p, (0, 1), 0)

                sweep_holder = {}

                def make_sweep_pair(pair):
                    def th():
                        if "b" not in sweep_holder:
                            w_ps_b = w_psum.tile([P, 2, NCH], F32, tag="w")
                            sweep_holder["b"] = w_ps_b
                        emit_colsum_pair(
                            sweep_holder["b"],
                            e2_list[pair],
                            r2_list[pair],
                            pair,
                            (2, 3),
                            2,
                        )

                    return th

                sweep_th = [make_sweep_pair(p) for p in range(N_PAIR)]
                return w_holder["a"], sweep_holder, sweep_th

            def final_thunks(b, w_ps_a, sweep_holder, sweep_th, xnat_s, y_in_w=False):
                """Colsum sweep kc{2,3} + w-phase: out = (w @ X) @ W_v; the
                KR pre-scale is folded into the final o_sb copy.  y_in_w puts
                the y accumulator in the (by then free) w PSUM pool so both
                gp buffers rotate the row->column transposes — without it the
                exposed tail chain serializes at ~560ns/step through one gp
                buffer.  (Only legal for the LAST batch: it adds a w-pool
                allocation.)"""
                w_sb = wvec_pool.tile([1, S], BF16, tag="wsb")
                y_holder = {}

                def get_yps():
                    if "mm" not in y_holder:
                        if y_in_w:
                            y_ps_w = w_psum.tile([P, 2, NCH], F32, tag="w")
                            y_holder["mm"] = y_ps_w[:, 0, :]
                            y_holder["row"] = y_ps_w[0:1, 0, :]
                        else:
                            y_ps_g = gp_psum.tile([P, NCH], F32, tag="gp")
                            y_holder["mm"] = y_ps_g[:]
                            y_holder["row"] = y_ps_g[0:1, :]
                    return y_holder
                wt_pads = {}
                yt_pads = {}
                thunks = []

                def make_wcopy(kc):
                    def th():
                        src = w_ps_a if kc < 2 else sweep_holder["b"]
                        nc.vector.tensor_copy(
                            w_sb[:, kc * NCH : (kc + 1) * NCH],
                            src[0:1, kc % 2, :],
                        )

                    return th

                def row_to_bcast_cols(src_row, pads, key, tag):
                    tp = gp_psum.tile([P, 1], F32, tag="gp")
                    nc.tensor.matmul(
                        tp[:], src_row, one_t[0:1, 0:1], start=True, stop=True
                    )
                    pad = wvec_pool.tile([P, P], BF16, tag=tag)
                    nc.vector.tensor_copy(pad[:], tp[:, 0:1].broadcast_to([P, P]))
                    pads[key] = pad

                def make_wtrans(kt):
                    def th():
                        row_to_bcast_cols(
                            w_sb[0:1, kt * P : (kt + 1) * P],
                            wt_pads, kt, f"wtp{kt % 4}",
                        )

                    return th

                def make_ymm(st):
                    def th():
                        nc.tensor.matmul(
                            get_yps()["mm"],
                            wt_pads[st][:],
                            xnat_s[:, st, :],
                            start=(st == 0),
                            stop=(st == N_ST - 1),
                            skip_group_check=True,
                        )

                    return th

                def epilogue_th():
                    y_sb = wvec_pool.tile([1, NCH], BF16, tag="ysb")
                    nc.vector.tensor_copy(y_sb[:], get_yps()["row"])
                    o_ps = gp_psum.tile([P, NCH], F32, tag="gp")
                    for c in range(N_DT):
                        row_to_bcast_cols(
                            y_sb[0:1, c * P : (c + 1) * P], yt_pads, c, f"ytp{c}"
                        )
                    for c in range(N_DT):
                        nc.tensor.matmul(
                            o_ps[:],
                            yt_pads[c][:],
                            wv_s[:, c, :],
                            start=(c == 0),
                            stop=(c == N_DT - 1),
                            skip_group_check=True,
                        )
                    o_sb = wvec_pool.tile([1, NCH], F32, tag="osb")
                    nc.vector.tensor_scalar_mul(o_sb[:], o_ps[0:1, :], 1.0 / KR)
                    nc.sync.dma_start(out=out_ext[b : b + 1, :], in_=o_sb[:])

                thunks.append(make_wcopy(0))
                thunks.append(make_wcopy(1))
                thunks.extend(sweep_th)
                thunks.append(make_wcopy(2))
                thunks.append(make_wcopy(3))
                for kt in range(N_ST):
                    thunks.append(make_wtrans(kt))
                    if kt >= 3:
                        thunks.append(make_ymm(kt - 3))
                for st in range(N_ST - 3, N_ST):
                    thunks.append(make_ymm(st))
                thunks.append(epilogue_th)
                return thunks

            def spread(thunks, n_slots):
                slots = [[] for _ in range(n_slots)]
                k = len(thunks)
                for i, th in enumerate(thunks):
                    slots[min(i * n_slots // k, n_slots - 1)].append(th)
                return slots

            # ------------------------- emission ------------------------------

            # FILL: s-tile 0 transpose (f32 path), s-tiles 1-3 transposes,
            # M prework, G s-chunk 0 — just enough for phase A0's q-tile 0.
            h0, dma0, trans0, kq0 = proj_thunks(
                0, xnat0_s, x0_loaded, unit_ctx=["fill", "fill", "a0", "a0"]
            )
            g0, xt0 = h0

            def first_tile_trans_f32():
                tp = gp_psum.tile([P, N_DT * P], F32, tag="gp")
                for dt_i in range(N_DT):
                    nc.tensor.matmul(
                        tp[:, dt_i * P : (dt_i + 1) * P],
                        xf0[:, dt_i * P : (dt_i + 1) * P],
                        ident_f[:],
                        start=True,
                        stop=True,
                        skip_group_check=True,
                    )
                nc.vector.tensor_copy(
                    xt0[:, :, 0:P],
                    tp[:].rearrange("p (t c) -> p t c", t=N_DT),
                )

            first_tile_trans_f32()
            pre_th = m_prework_thunks()
            # interleave prework (ACT copies) with c0 transposes (DVE copies);
            # a dummy matmul after each early unit keeps the copy-bound fill
            # phase's PE stream gapless so the clock warms by ~4.5us
            fill_stream = []
            fill_stream.extend(pre_th[:4])  # wkT units
            fill_stream.extend(trans0[0][1:])  # s-tiles 1-3
            fill_stream.extend(pre_th[4:8])  # wqT units
            fill_stream.extend(trans0[1])  # s-tiles 4-7 (phase A needs kc1)
            fill_stream.extend(pre_th[8:])  # M groups
            fill_stream.extend(kq0[0])  # G s-chunk 0
            for i, th in enumerate(fill_stream):
                th()
                if i < 14:
                    emit_warm_dummy(1)

            # batch 1 proj thunks (woven into B0; all DMAs already queued)
            h1, dma1, trans1, kq1 = proj_thunks(1, xnat1_s, [True] * N_SC)
            g1, xt1 = h1

            # --- phase A0: extras = batch0's remaining transposes/G.
            # Deadlines: G sc1 before qt4, sc2 before qt8, sc3 before qt12;
            # trans c2/c3 before phase B0.
            slots_a0 = [[] for _ in range(N_ST)]
            slots_a0[0] = [kq0[1][0], kq0[1][1]]
            slots_a0[1] = [kq0[1][2], kq0[1][3]]
            slots_a0[2] = [trans0[2][0], trans0[2][1]]
            slots_a0[3] = [trans0[2][2], trans0[2][3]]
            slots_a0[4] = [kq0[2][0], kq0[2][1]]
            slots_a0[5] = [kq0[2][2], kq0[2][3]]
            slots_a0[6] = [trans0[3][0], trans0[3][1]]
            slots_a0[7] = [trans0[3][2], trans0[3][3]]
            slots_a0[8] = [kq0[3][0], kq0[3][1]]
            slots_a0[9] = [kq0[3][2], kq0[3][3]]

            e2_0, rs_0 = [], []
            phase_A(g0, xt0, e2_0, rs_0, slots_a0)

            # --- phase B0: extras = batch1's projection.
            proj1_flat = []
            for sc in range(N_SC):
                proj1_flat.extend(trans1[sc])
                proj1_flat.extend(kq1[sc])
            wa0, swh0, swth0 = phase_B(
                g0, xt0, e2_0, rs_0, spread(proj1_flat, N_ST)
            )

            # --- phase A1: extras = batch0's colsum sweep + w-phase.  The
            # sweep matmuls go ONE PAIR PER SLOT — a solid block would park
            # 16 matmuls ahead of the scores stream in the in-order PE queue
            # and starve ACT for ~3.5us.
            fin0 = final_thunks(0, wa0, swh0, swth0, xnat0_s)
            slots_a1 = [[] for _ in range(N_ST)]
            slots_a1[0] = fin0[0:3]  # wcopy0, wcopy1, sweep pair0
            for p in range(1, N_PAIR):
                slots_a1[p] = [fin0[2 + p]]  # sweep pair p
            slots_a1[8].extend(fin0[10:12])  # wcopy2, wcopy3
            rest = fin0[12:]
            k = len(rest)
            for i, th in enumerate(rest):
                slots_a1[8 + min(i * 8 // k, 7)].append(th)
            e2_1, rs_1 = [], []
            phase_A(g1, xt1, e2_1, rs_1, slots_a1)

            # --- phase B1: no extras.
            wa1, swh1, swth1 = phase_B(
                g1, xt1, e2_1, rs_1, [[] for _ in range(N_ST)]
            )

            # --- tail: batch1's sweep + w-phase (y in the free w banks).
            for th in final_thunks(1, wa1, swh1, swth1, xnat1_s, y_in_w=True):
                th()

    nc.compile()
    return nc


_NC_CACHE = None


def _get_nc():
    global _NC_CACHE
    if _NC_CACHE is None:
        _NC_CACHE = build_nc()
    return _NC_CACHE


def make_in_maps(inputs, W_q, W_k, W_v):
    inputs = np.ascontiguousarray(np.asarray(inputs, dtype=np.float32))
    W_q = np.ascontiguousarray(np.asarray(W_q, dtype=np.float32))
    W_k = np.ascontiguousarray(np.asarray(W_k, dtype=np.float32))
    W_v = np.ascontiguousarray(np.asarray(W_v, dtype=np.float32))
    return [
        {
            "inputs": inputs[i * B_PER_CORE : (i + 1) * B_PER_CORE],
            "W_q": W_q,
            "W_k": W_k,
            "W_v": W_v,
        }
        for i in range(N_CORES)
    ]


def kernel(**inputs) -> np.ndarray:
    nc = _get_nc()
    in_maps = make_in_maps(
        inputs["inputs"], inputs["W_q"], inputs["W_k"], inputs["W_v"]
    )
    res = run_bass_kernel_spmd(nc, in_maps, core_ids=list(range(N_CORES)))
    return np.concatenate(
        [res.results[i]["out"] for i in range(N_CORES)], axis=0
    ).astype(np.float32)
